# revision 19
# baseline (speedup 1.0000x reference)
"""Bass/Trainium2 kernel for nn_BigramLanguageModel (6-layer GPT, B=128 T=256 C=384 H=6 V=65).

Strategy: pure data-parallel over batch across 8 NeuronCores (16 batch rows each),
weights replicated. Per core, a fully fused transformer forward:
  - residual stream token-major fp32 in SBUF ([128 tok, 384] tiles)
  - matmul operands fp16 (1 cy/row on PE), fp32 PSUM accumulation
  - LayerNorm stats via bn_stats/bn_aggr (token-major), scale/bias folded into the
    PSUM->SBUF copy after the PE transpose to feature-major
  - attention scores computed transposed [s, t]; softmax denominator via a ones
    column appended to V (row 64 of the AV matmul output); causal mask via
    gpsimd.affine_select; no max-subtraction (scores are tiny; softmax invariant)
  - embedding gather as one-hot matmul (one-hot via DMA-broadcast + is_equal)
  - loss (mean NLL) computed on device per core; host sums 8 partials
"""

import numpy as np

V, B, T, C, H, L = 65, 128, 256, 384, 6, 6
HS, FF = C // H, 4 * C
EPS = 1e-5
NCORES = 8
BSH = B // NCORES          # batch rows per core = 16
NTOK = BSH * T             # tokens per core = 4096
NT = NTOK // 128           # token tiles per core = 32
NG = NT // 4               # groups (512 tokens = 2 batch rows) = 8
NC_CH = C // 128           # 3 feature chunks
NF_CH = FF // 128          # 12 ff chunks
SCALE = C ** (-0.5)

_CACHE = {}


def _build_program(nlayers=L, do_attn=True, do_mlp=True):
    import concourse.bacc as bacc
    import concourse.bass as bass
    import concourse.mybir as mybir
    import concourse.tile as tile
    from concourse.masks import make_identity

    f16 = mybir.dt.float16
    f32 = mybir.dt.float32
    i32 = mybir.dt.int32
    AF = mybir.ActivationFunctionType
    ALU = mybir.AluOpType

    nc = bacc.Bacc(target_bir_lowering=False)

    # ---- DRAM I/O ----
    idx_d = nc.dram_tensor("idx", [BSH, T], i32, kind="ExternalInput")
    tgt_d = nc.dram_tensor("targets", [BSH, T], i32, kind="ExternalInput")
    tok_d = nc.dram_tensor("tok_emb", [V, C], f16, kind="ExternalInput")
    pos_d = nc.dram_tensor("pos_emb", [T, C], f16, kind="ExternalInput")
    wqkv_d = nc.dram_tensor("wqkv", [L, C, 3 * C], f16, kind="ExternalInput")
    projw_d = nc.dram_tensor("proj_w", [L, C, C], f16, kind="ExternalInput")
    projb_d = nc.dram_tensor("proj_b", [1, L * C], f16, kind="ExternalInput")
    w1_d = nc.dram_tensor("w1", [L, C, FF], f16, kind="ExternalInput")
    b1_d = nc.dram_tensor("b1", [L, FF], f32, kind="ExternalInput")
    w2_d = nc.dram_tensor("w2", [L, FF, C], f16, kind="ExternalInput")
    b2_d = nc.dram_tensor("b2", [1, L * C], f16, kind="ExternalInput")
    ln1s_d = nc.dram_tensor("ln1_s", [L, C], f32, kind="ExternalInput")
    ln1b_d = nc.dram_tensor("ln1_b", [L, C], f32, kind="ExternalInput")
    ln2s_d = nc.dram_tensor("ln2_s", [L, C], f32, kind="ExternalInput")
    ln2b_d = nc.dram_tensor("ln2_b", [L, C], f32, kind="ExternalInput")
    lnfs_d = nc.dram_tensor("lnf_s", [C], f32, kind="ExternalInput")
    lnfb_d = nc.dram_tensor("lnf_b", [C], f32, kind="ExternalInput")
    headw_d = nc.dram_tensor("head_w", [C, V], f16, kind="ExternalInput")
    headb_d = nc.dram_tensor("head_b", [1, V], f16, kind="ExternalInput")
    logits_d = nc.dram_tensor("logits", [NTOK, V], f32, kind="ExternalOutput")
    loss_d = nc.dram_tensor("loss_sum", [1, 1], f32, kind="ExternalOutput")

    with tile.TileContext(nc) as tc:
        import contextlib
        ctx = contextlib.ExitStack()
        with ctx:
            const = ctx.enter_context(tc.tile_pool(name="const", bufs=1))
            wA = ctx.enter_context(tc.tile_pool(name="wA", bufs=2))
            wB = ctx.enter_context(tc.tile_pool(name="wB", bufs=2))
            resid = ctx.enter_context(tc.tile_pool(name="resid", bufs=1))
            tmg = ctx.enter_context(tc.tile_pool(name="tmg", bufs=2))      # token-major group tiles
            fmg = ctx.enter_context(tc.tile_pool(name="fmg", bufs=2))      # feature-major group tiles
            qkp = ctx.enter_context(tc.tile_pool(name="qkp", bufs=2))
            vp = ctx.enter_context(tc.tile_pool(name="vp", bufs=2))
            exl = ctx.enter_context(tc.tile_pool(name="exl", bufs=3))
            hp = ctx.enter_context(tc.tile_pool(name="hp", bufs=1))
            stat = ctx.enter_context(tc.tile_pool(name="stat", bufs=4))
            small = ctx.enter_context(tc.tile_pool(name="small", bufs=2))
            outp = ctx.enter_context(tc.tile_pool(name="outp", bufs=4))
            ps_big = ctx.enter_context(tc.tile_pool(name="ps_big", bufs=2, space="PSUM"))
            ps_tr = ctx.enter_context(tc.tile_pool(name="ps_tr", bufs=2, space="PSUM"))
            ps_sc = ctx.enter_context(tc.tile_pool(name="ps_sc", bufs=2, space="PSUM"))
            ps_av = ctx.enter_context(tc.tile_pool(name="ps_av", bufs=2, space="PSUM"))

            # ---- constants / params resident in SBUF ----
            ident = const.tile([128, 128], f16)
            make_identity(nc, ident)
            ones_r = const.tile([1, 128], f16)       # K=1 lhsT for bias matmuls
            nc.vector.memset(ones_r, 1.0)
            ones_c32 = const.tile([128, 1], f32)     # K=128 lhsT for loss column-sum
            nc.vector.memset(ones_c32, 1.0)
            eps_t = const.tile([128, 1], f32)
            nc.vector.memset(eps_t, EPS)
            viota_p = const.tile([V, 512], i32)      # value = partition idx
            nc.gpsimd.iota(viota_p, pattern=[[0, 512]], base=0, channel_multiplier=1)
            viota_r = const.tile([128, V], f32)      # value = free idx (exact in f32)
            nc.gpsimd.iota(viota_r, pattern=[[1, V]], base=0, channel_multiplier=0,
                           allow_small_or_imprecise_dtypes=True)

            tok_sb = const.tile([V, C], f16)
            nc.sync.dma_start(out=tok_sb, in_=tok_d[:, :])
            pos_sb = const.tile([128, 2, C], f16)
            nc.sync.dma_start(out=pos_sb, in_=pos_d.rearrange("(i p) c -> p i c", p=128))
            ln1s_sb = const.tile([128, L, NC_CH], f32)
            nc.sync.dma_start(out=ln1s_sb, in_=ln1s_d.rearrange("l (i p) -> p l i", p=128))
            ln1b_sb = const.tile([128, L, NC_CH], f32)
            nc.sync.dma_start(out=ln1b_sb, in_=ln1b_d.rearrange("l (i p) -> p l i", p=128))
            ln2s_sb = const.tile([128, L, NC_CH], f32)
            nc.sync.dma_start(out=ln2s_sb, in_=ln2s_d.rearrange("l (i p) -> p l i", p=128))
            ln2b_sb = const.tile([128, L, NC_CH], f32)
            nc.sync.dma_start(out=ln2b_sb, in_=ln2b_d.rearrange("l (i p) -> p l i", p=128))
            lnfs_sb = const.tile([128, NC_CH], f32)
            nc.sync.dma_start(out=lnfs_sb, in_=lnfs_d.rearrange("(i p) -> p i", p=128))
            lnfb_sb = const.tile([128, NC_CH], f32)
            nc.sync.dma_start(out=lnfb_sb, in_=lnfb_d.rearrange("(i p) -> p i", p=128))
            b1_sb = const.tile([128, L, NF_CH], f32)
            nc.sync.dma_start(out=b1_sb, in_=b1_d.rearrange("l (i p) -> p l i", p=128))
            headw_sb = const.tile([128, NC_CH, V], f16)
            nc.sync.dma_start(out=headw_sb, in_=headw_d.rearrange("(i p) v -> p i v", p=128))
            headb_sb = const.tile([1, V], f16)
            nc.sync.dma_start(out=headb_sb, in_=headb_d[:, :])
            tgt_i = const.tile([128, NT], i32)
            nc.sync.dma_start(out=tgt_i, in_=tgt_d.rearrange("b (n p) -> p (b n)", p=128))
            tgt_sb = const.tile([128, NT], f32)
            nc.vector.tensor_copy(tgt_sb, tgt_i)
            loss_cols = const.tile([128, NT], f32)

            x_tm = resid.tile([128, NT, C], f32)     # persistent residual stream

            # ================= embedding =================
            with nc.named_scope("embed"):
                for g in range(NG):
                    idxb = small.tile([V, 512], i32, tag="idxb")
                    nc.sync.dma_start(
                        out=idxb,
                        in_=bass.AP(idx_d, g * 512, [[0, V], [1, 512]]),
                    )
                    onehot = small.tile([V, 512], f16, tag="onehot")
                    nc.vector.tensor_tensor(out=onehot, in0=idxb, in1=viota_p, op=ALU.is_equal)
                    for j in range(4):
                        i = g * 4 + j
                        ps = ps_big.tile([128, 512], f32, tag="big")
                        nc.tensor.matmul(ps[:, 0:C], onehot[:, j * 128:(j + 1) * 128], tok_sb,
                                         start=True, stop=True)
                        nc.vector.tensor_add(x_tm[:, i, :], ps[:, 0:C], pos_sb[:, i % 2, :])

            # helper: LayerNorm (token-major stats) + transpose to feature-major with
            # scale/bias folded into the PSUM->SBUF copy. Returns [128, NC_CH, 512] f16.
            def ln_to_fm(g, s_ap, b_ap, tag):
                mv = stat.tile([128, 4, 2], f32, tag="mv")
                for j in range(4):
                    i = g * 4 + j
                    st6 = stat.tile([128, 6], f32, tag="st6")
                    nc.vector.bn_stats(out=st6, in_=x_tm[:, i, :])
                    nc.vector.bn_aggr(out=mv[:, j, :], in_=st6)
                std = stat.tile([128, 4], f32, tag="std")
                nc.scalar.activation(out=std, in_=mv[:, :, 1], func=AF.Sqrt,
                                     bias=eps_t, scale=1.0)
                rstd = stat.tile([128, 4], f32, tag="rstd")
                nc.vector.reciprocal_approx_fast(out=rstd, in_=std)
                nmr = stat.tile([128, 4], f32, tag="nmr")
                nc.vector.tensor_mul(nmr, mv[:, :, 0], rstd)
                nc.vector.tensor_scalar_mul(nmr, nmr, -1.0)
                nn_tm = tmg.tile([128, 4, C], f16, tag="nn_tm")
                for j in range(4):
                    i = g * 4 + j
                    nc.scalar.activation(out=nn_tm[:, j, :], in_=x_tm[:, i, :],
                                         func=AF.Identity, bias=nmr[:, j:j + 1],
                                         scale=rstd[:, j:j + 1])
                nn_fm = fmg.tile([128, NC_CH, 512], f16, tag=tag)
                for ci in range(NC_CH):
                    ps = ps_tr.tile([128, 512], f16, tag="tr")
                    for j in range(4):
                        nc.tensor.transpose(ps[:, j * 128:(j + 1) * 128],
                                            nn_tm[:, j, ci * 128:(ci + 1) * 128], ident)
                    nc.scalar.activation(out=nn_fm[:, ci, :], in_=ps, func=AF.Identity,
                                         bias=b_ap[:, ci:ci + 1], scale=s_ap[:, ci:ci + 1])
                return nn_fm

            # ================= layers =================
            for l in range(nlayers):
                wqkv_t = wA.tile([128, NC_CH, 3 * C], f16, tag="wqkv")
                nc.sync.dma_start(out=wqkv_t,
                                  in_=wqkv_d[l, :, :].rearrange("(i p) o -> p i o", p=128))
                projw_t = wA.tile([128, NC_CH, C], f16, tag="projw")
                nc.sync.dma_start(out=projw_t,
                                  in_=projw_d[l, :, :].rearrange("(i p) o -> p i o", p=128))
                w1_t = wB.tile([128, NC_CH, FF], f16, tag="w1")
                nc.sync.dma_start(out=w1_t,
                                  in_=w1_d[l, :, :].rearrange("(i p) o -> p i o", p=128))
                w2_t = wB.tile([128, NF_CH, C], f16, tag="w2")
                nc.sync.dma_start(out=w2_t,
                                  in_=w2_d[l, :, :].rearrange("(i p) o -> p i o", p=128))
                projb_sb = wA.tile([1, C], f16, tag="projb")
                nc.sync.dma_start(out=projb_sb, in_=projb_d[:, l * C:(l + 1) * C])
                b2_sb = wA.tile([1, C], f16, tag="b2")
                nc.sync.dma_start(out=b2_sb, in_=b2_d[:, l * C:(l + 1) * C])

                # ---- phase A: attention ----
                with nc.named_scope(f"l{l}_attn"):
                    for g in range(NG if do_attn else 0):
                        nn_fm = ln_to_fm(g, ln1s_sb[:, l, :], ln1b_sb[:, l, :], "nn_fm")
                        # q, k feature-major [128, ch, 512]
                        q_sb = qkp.tile([128, NC_CH, 512], f16, tag="q")
                        k_sb = qkp.tile([128, NC_CH, 512], f16, tag="k")
                        for dst, off in ((q_sb, 0), (k_sb, C)):
                            for m in range(NC_CH):
                                ps = ps_big.tile([128, 512], f32, tag="big")
                                for ci in range(NC_CH):
                                    nc.tensor.matmul(
                                        ps, wqkv_t[:, ci, off + m * 128:off + (m + 1) * 128],
                                        nn_fm[:, ci, :],
                                        start=(ci == 0), stop=(ci == NC_CH - 1))
                                nc.vector.tensor_copy(dst[:, m, :], ps)
                        # v token-major, augmented with ones column per head [128, 4, H, HS+1]
                        v_sb = vp.tile([128, 4, H, HS + 1], f16, tag="v")
                        for j in range(4):
                            ps = ps_big.tile([128, 512], f32, tag="big")
                            for ci in range(NC_CH):
                                nc.tensor.matmul(ps[:, 0:C],
                                                 nn_fm[:, ci, j * 128:(j + 1) * 128],
                                                 wqkv_t[:, ci, 2 * C:3 * C],
                                                 start=(ci == 0), stop=(ci == NC_CH - 1))
                            nc.vector.tensor_copy(
                                v_sb[:, j, :, 0:HS],
                                ps[:, 0:C].rearrange("p (h d) -> p h d", h=H))
                            nc.gpsimd.memset(v_sb[:, j, :, HS:HS + 1], 1.0)
                        # attention per (batch-in-group, head) -> att token-major
                        att_tm = tmg.tile([128, 4, C], f16, tag="att_tm")
                        for bl in range(2):
                            toff = bl * 256
                            for h in range(H):
                                ch, hp_ = h // 2, (h % 2) * HS
                                sc = ps_sc.tile([128, 384], f32, tag="sc")
                                nc.tensor.matmul(sc[:, 0:256],
                                                 k_sb[hp_:hp_ + HS, ch, toff:toff + 128],
                                                 q_sb[hp_:hp_ + HS, ch, toff:toff + 256],
                                                 start=True, stop=True)
                                nc.tensor.matmul(sc[:, 256:384],
                                                 k_sb[hp_:hp_ + HS, ch, toff + 128:toff + 256],
                                                 q_sb[hp_:hp_ + HS, ch, toff + 128:toff + 256],
                                                 start=True, stop=True)
                                ex = exl.tile([128, 384], f16, tag="ex")
                                nc.scalar.activation(out=ex, in_=sc, func=AF.Exp, scale=SCALE)
                                nc.gpsimd.affine_select(
                                    out=ex[:, 0:256], in_=ex[:, 0:256],
                                    compare_op=ALU.is_ge, fill=0.0, base=0,
                                    pattern=[[1, 256]], channel_multiplier=-1)
                                nc.gpsimd.affine_select(
                                    out=ex[:, 256:384], in_=ex[:, 256:384],
                                    compare_op=ALU.is_ge, fill=0.0, base=0,
                                    pattern=[[1, 128]], channel_multiplier=-1)
                                av = ps_av.tile([128, 2, V], f32, tag="av")
                                nc.tensor.matmul(av[:, 0, :], ex[:, 0:128],
                                                 v_sb[:, bl * 2, h, :], start=True, stop=True)
                                nc.tensor.matmul(av[:, 1, :], ex[:, 128:256],
                                                 v_sb[:, bl * 2, h, :], start=True, stop=False)
                                nc.tensor.matmul(av[:, 1, :], ex[:, 256:384],
                                                 v_sb[:, bl * 2 + 1, h, :], start=False, stop=True)
                                for tt in range(2):
                                    r = stat.tile([128, 1], f32, tag="r")
                                    nc.vector.reciprocal_approx_fast(
                                        out=r, in_=av[:, tt, HS:HS + 1])
                                    nc.scalar.activation(
                                        out=att_tm[:, bl * 2 + tt, h * HS:(h + 1) * HS],
                                        in_=av[:, tt, 0:HS], func=AF.Copy, scale=r)
                        # att -> feature-major
                        att_fm = fmg.tile([128, NC_CH, 512], f16, tag="att_fm")
                        for ci in range(NC_CH):
                            ps = ps_tr.tile([128, 512], f16, tag="tr")
                            for j in range(4):
                                nc.tensor.transpose(ps[:, j * 128:(j + 1) * 128],
                                                    att_tm[:, j, ci * 128:(ci + 1) * 128], ident)
                            nc.vector.tensor_copy(att_fm[:, ci, :], ps)
                        # proj + residual
                        for j in range(4):
                            i = g * 4 + j
                            ps = ps_big.tile([128, 512], f32, tag="big")
                            for ci in range(NC_CH):
                                nc.tensor.matmul(ps[:, 0:C], att_fm[:, ci, j * 128:(j + 1) * 128],
                                                 projw_t[:, ci, :],
                                                 start=(ci == 0), stop=False)
                            nc.tensor.matmul(ps[:, 0:C], ones_r, projb_sb,
                                             start=False, stop=True)
                            nc.vector.tensor_add(x_tm[:, i, :], x_tm[:, i, :], ps[:, 0:C])

                # ---- phase B: MLP ----
                with nc.named_scope(f"l{l}_mlp"):
                    for g in range(NG if do_mlp else 0):
                        nn_fm = ln_to_fm(g, ln2s_sb[:, l, :], ln2b_sb[:, l, :], "nn_fm")
                        h_sb = hp.tile([128, NF_CH, 512], f16, tag="h")
                        for f in range(NF_CH):
                            ps = ps_big.tile([128, 512], f32, tag="big")
                            for ci in range(NC_CH):
                                nc.tensor.matmul(ps, w1_t[:, ci, f * 128:(f + 1) * 128],
                                                 nn_fm[:, ci, :],
                                                 start=(ci == 0), stop=(ci == NC_CH - 1))
                            nc.scalar.activation(out=h_sb[:, f, :], in_=ps, func=AF.Relu,
                                                 bias=b1_sb[:, l, f:f + 1], scale=1.0)
                        for j in range(4):
                            i = g * 4 + j
                            ps = ps_big.tile([128, 512], f32, tag="big")
                            for fi in range(NF_CH):
                                nc.tensor.matmul(ps[:, 0:C], h_sb[:, fi, j * 128:(j + 1) * 128],
                                                 w2_t[:, fi, :],
                                                 start=(fi == 0), stop=False)
                            nc.tensor.matmul(ps[:, 0:C], ones_r, b2_sb,
                                             start=False, stop=True)
                            nc.vector.tensor_add(x_tm[:, i, :], x_tm[:, i, :], ps[:, 0:C])

            # ================= final LN + head + loss =================
            with nc.named_scope("head"):
                for g in range(NG):
                    xf_fm = ln_to_fm(g, lnfs_sb, lnfb_sb, "nn_fm")
                    for j in range(4):
                        i = g * 4 + j
                        psl = ps_av.tile([128, 2, V], f32, tag="av")
                        for ci in range(NC_CH):
                            nc.tensor.matmul(psl[:, 0, :], xf_fm[:, ci, j * 128:(j + 1) * 128],
                                             headw_sb[:, ci, :],
                                             start=(ci == 0), stop=False)
                        nc.tensor.matmul(psl[:, 0, :], ones_r, headb_sb,
                                         start=False, stop=True)
                        lg = outp.tile([128, V], f32, tag="lg")
                        nc.scalar.activation(out=lg, in_=psl[:, 0, :], func=AF.Copy)
                        nc.sync.dma_start(out=logits_d[i * 128:(i + 1) * 128, :], in_=lg)
                        # loss pieces: lse - logits[target]
                        sel = outp.tile([128, V], f32, tag="sel")
                        nc.vector.tensor_scalar(out=sel, in0=viota_r,
                                                scalar1=tgt_sb[:, i:i + 1], scalar2=None,
                                                op0=ALU.is_equal)
                        scr = outp.tile([128, V], f32, tag="scr")
                        pick = stat.tile([128, 1], f32, tag="pick")
                        nc.vector.tensor_mul(scr, lg, sel)
                        nc.vector.reduce_sum(pick, scr, axis=mybir.AxisListType.X)
                        ex2 = outp.tile([128, V], f32, tag="ex2")
                        se = stat.tile([128, 1], f32, tag="se")
                        nc.scalar.activation(out=ex2, in_=lg, func=AF.Exp, accum_out=se)
                        lse = stat.tile([128, 1], f32, tag="lse")
                        nc.scalar.activation(out=lse, in_=se, func=AF.Ln)
                        nc.vector.tensor_sub(loss_cols[:, i:i + 1], lse, pick)
                pst = ps_sc.tile([1, NT], f32, tag="sc")
                nc.tensor.matmul(pst, ones_c32, loss_cols, start=True, stop=True)
                tot = stat.tile([1, 1], f32, tag="tot")
                nc.vector.reduce_sum(tot, pst, axis=mybir.AxisListType.X)
                nc.sync.dma_start(out=loss_d[:, :], in_=tot)

    nc.finalize()
    return nc


def _get_program():
    if "nc" not in _CACHE:
        _CACHE["nc"] = _build_program()
    return _CACHE["nc"]


def _prep_weights(inputs):
    f16 = np.float16

    def hcd(w):  # [L,H,C,HS] -> [L, C, H*HS]
        return np.ascontiguousarray(w.transpose(0, 2, 1, 3)).reshape(L, C, C)

    wqkv = np.concatenate(
        [hcd(np.asarray(inputs["wq"])), hcd(np.asarray(inputs["wk"])),
         hcd(np.asarray(inputs["wv"]))], axis=2).astype(f16)
    shared = {
        "tok_emb": np.asarray(inputs["tok_emb"]).astype(f16),
        "pos_emb": np.asarray(inputs["pos_emb"]).astype(f16),
        "wqkv": np.ascontiguousarray(wqkv),
        "proj_w": np.asarray(inputs["proj_w"]).astype(f16),
        "proj_b": np.asarray(inputs["proj_b"]).astype(f16).reshape(1, L * C),
        "w1": np.asarray(inputs["w1"]).astype(f16),
        "b1": np.asarray(inputs["b1"]).astype(np.float32),
        "w2": np.asarray(inputs["w2"]).astype(f16),
        "b2": np.asarray(inputs["b2"]).astype(f16).reshape(1, L * C),
        "ln1_s": np.asarray(inputs["ln1_s"]).astype(np.float32),
        "ln1_b": np.asarray(inputs["ln1_b"]).astype(np.float32),
        "ln2_s": np.asarray(inputs["ln2_s"]).astype(np.float32),
        "ln2_b": np.asarray(inputs["ln2_b"]).astype(np.float32),
        "lnf_s": np.asarray(inputs["lnf_s"]).astype(np.float32),
        "lnf_b": np.asarray(inputs["lnf_b"]).astype(np.float32),
        "head_w": np.asarray(inputs["head_w"]).astype(f16),
        "head_b": np.asarray(inputs["head_b"]).astype(f16).reshape(1, V),
    }
    return shared


def kernel(**inputs):
    from concourse.bass_utils import run_bass_kernel_spmd

    nc = _get_program()
    shared = _prep_weights(inputs)
    idx = np.ascontiguousarray(np.asarray(inputs["idx"], dtype=np.int32))
    tgt = np.ascontiguousarray(np.asarray(inputs["targets"], dtype=np.int32))

    in_maps = []
    for c in range(NCORES):
        m = dict(shared)
        m["idx"] = np.ascontiguousarray(idx[c * BSH:(c + 1) * BSH])
        m["targets"] = np.ascontiguousarray(tgt[c * BSH:(c + 1) * BSH])
        in_maps.append(m)

    res = run_bass_kernel_spmd(nc, in_maps, core_ids=list(range(NCORES)), trace=False)
    logits = np.concatenate([r["logits"] for r in res.results], axis=0)
    loss = sum(float(r["loss_sum"][0, 0]) for r in res.results) / (B * T)
    return logits.astype(np.float32), np.array(loss, dtype=np.float32)


# revision 23
# speedup vs baseline: 1.0054x; 1.0054x over previous
"""Bass/Trainium2 kernel for nn_BigramLanguageModel (6-layer GPT, B=128 T=256 C=384 H=6 V=65).

Strategy: pure data-parallel over batch across 8 NeuronCores (16 batch rows each),
weights replicated. Per core, a fully fused transformer forward:
  - residual stream token-major fp32 in SBUF ([128 tok, 384] tiles)
  - matmul operands fp16 (1 cy/row on PE), fp32 PSUM accumulation
  - LayerNorm stats via bn_stats/bn_aggr (token-major), scale/bias folded into the
    PSUM->SBUF copy after the PE transpose to feature-major
  - attention scores computed transposed [s, t]; softmax denominator via a ones
    column appended to V (row 64 of the AV matmul output); causal mask via
    gpsimd.affine_select; no max-subtraction (scores are tiny; softmax invariant)
  - embedding gather as one-hot matmul (one-hot via DMA-broadcast + is_equal)
  - loss (mean NLL) computed on device per core; host sums 8 partials
"""

import numpy as np

V, B, T, C, H, L = 65, 128, 256, 384, 6, 6
HS, FF = C // H, 4 * C
EPS = 1e-5
NCORES = 8
BSH = B // NCORES          # batch rows per core = 16
NTOK = BSH * T             # tokens per core = 4096
NT = NTOK // 128           # token tiles per core = 32
NG = NT // 4               # groups (512 tokens = 2 batch rows) = 8
NC_CH = C // 128           # 3 feature chunks
NF_CH = FF // 128          # 12 ff chunks
SCALE = C ** (-0.5)

_CACHE = {}


def _build_program(nlayers=L, do_attn=True, do_mlp=True):
    import concourse.bacc as bacc
    import concourse.bass as bass
    import concourse.mybir as mybir
    import concourse.tile as tile
    from concourse.masks import make_identity

    f16 = mybir.dt.float16
    f32 = mybir.dt.float32
    i32 = mybir.dt.int32
    AF = mybir.ActivationFunctionType
    ALU = mybir.AluOpType

    nc = bacc.Bacc(target_bir_lowering=False)

    # ---- DRAM I/O ----
    idx_d = nc.dram_tensor("idx", [BSH, T], i32, kind="ExternalInput")
    tgt_d = nc.dram_tensor("targets", [BSH, T], i32, kind="ExternalInput")
    tok_d = nc.dram_tensor("tok_emb", [V, C], f16, kind="ExternalInput")
    pos_d = nc.dram_tensor("pos_emb", [T, C], f16, kind="ExternalInput")
    wqkv_d = nc.dram_tensor("wqkv", [L, C, 3 * C], f16, kind="ExternalInput")
    projw_d = nc.dram_tensor("proj_w", [L, C, C], f16, kind="ExternalInput")
    projb_d = nc.dram_tensor("proj_b", [1, L * C], f16, kind="ExternalInput")
    w1_d = nc.dram_tensor("w1", [L, C, FF], f16, kind="ExternalInput")
    b1_d = nc.dram_tensor("b1", [L, FF], f32, kind="ExternalInput")
    w2_d = nc.dram_tensor("w2", [L, FF, C], f16, kind="ExternalInput")
    b2_d = nc.dram_tensor("b2", [1, L * C], f16, kind="ExternalInput")
    ln1s_d = nc.dram_tensor("ln1_s", [L, C], f32, kind="ExternalInput")
    ln1b_d = nc.dram_tensor("ln1_b", [L, C], f32, kind="ExternalInput")
    ln2s_d = nc.dram_tensor("ln2_s", [L, C], f32, kind="ExternalInput")
    ln2b_d = nc.dram_tensor("ln2_b", [L, C], f32, kind="ExternalInput")
    lnfs_d = nc.dram_tensor("lnf_s", [C], f32, kind="ExternalInput")
    lnfb_d = nc.dram_tensor("lnf_b", [C], f32, kind="ExternalInput")
    headw_d = nc.dram_tensor("head_w", [C, V], f16, kind="ExternalInput")
    headb_d = nc.dram_tensor("head_b", [1, V], f16, kind="ExternalInput")
    logits_d = nc.dram_tensor("logits", [NTOK, V], f32, kind="ExternalOutput")
    loss_d = nc.dram_tensor("loss_sum", [1, 1], f32, kind="ExternalOutput")

    with tile.TileContext(nc) as tc:
        import contextlib
        ctx = contextlib.ExitStack()
        with ctx:
            const = ctx.enter_context(tc.tile_pool(name="const", bufs=1))
            wA = ctx.enter_context(tc.tile_pool(name="wA", bufs=2))
            wB = ctx.enter_context(tc.tile_pool(name="wB", bufs=2))
            resid = ctx.enter_context(tc.tile_pool(name="resid", bufs=1))
            tmg = ctx.enter_context(tc.tile_pool(name="tmg", bufs=2))      # token-major group tiles
            fmg = ctx.enter_context(tc.tile_pool(name="fmg", bufs=2))      # feature-major group tiles
            qkp = ctx.enter_context(tc.tile_pool(name="qkp", bufs=2))
            vp = ctx.enter_context(tc.tile_pool(name="vp", bufs=2))
            exl = ctx.enter_context(tc.tile_pool(name="exl", bufs=3))
            hp = ctx.enter_context(tc.tile_pool(name="hp", bufs=2))
            stat = ctx.enter_context(tc.tile_pool(name="stat", bufs=4))
            small = ctx.enter_context(tc.tile_pool(name="small", bufs=2))
            outp = ctx.enter_context(tc.tile_pool(name="outp", bufs=4))
            ps_big = ctx.enter_context(tc.tile_pool(name="ps_big", bufs=2, space="PSUM"))
            ps_tr = ctx.enter_context(tc.tile_pool(name="ps_tr", bufs=2, space="PSUM"))
            ps_sc = ctx.enter_context(tc.tile_pool(name="ps_sc", bufs=2, space="PSUM"))
            ps_av = ctx.enter_context(tc.tile_pool(name="ps_av", bufs=2, space="PSUM"))

            # ---- constants / params resident in SBUF ----
            ident = const.tile([128, 128], f16)
            make_identity(nc, ident)
            ones_r = const.tile([1, 128], f16)       # K=1 lhsT for bias matmuls
            nc.vector.memset(ones_r, 1.0)
            ones_c32 = const.tile([128, 1], f32)     # K=128 lhsT for loss column-sum
            nc.vector.memset(ones_c32, 1.0)
            eps_t = const.tile([128, 1], f32)
            nc.vector.memset(eps_t, EPS)
            viota_p = const.tile([V, 512], i32)      # value = partition idx
            nc.gpsimd.iota(viota_p, pattern=[[0, 512]], base=0, channel_multiplier=1)
            viota_r = const.tile([128, V], f32)      # value = free idx (exact in f32)
            nc.gpsimd.iota(viota_r, pattern=[[1, V]], base=0, channel_multiplier=0,
                           allow_small_or_imprecise_dtypes=True)
            # causal triangle mask: tri[s, t] = 1.0 if t >= s else 0.0
            tri = const.tile([128, 128], f16)
            nc.vector.memset(tri, 1.0)
            nc.gpsimd.affine_select(out=tri, in_=tri, compare_op=ALU.is_ge, fill=0.0,
                                    base=0, pattern=[[1, 128]], channel_multiplier=-1)

            tok_sb = const.tile([V, C], f16)
            nc.sync.dma_start(out=tok_sb, in_=tok_d[:, :])
            pos_sb = const.tile([128, 2, C], f16)
            nc.sync.dma_start(out=pos_sb, in_=pos_d.rearrange("(i p) c -> p i c", p=128))
            ln1s_sb = const.tile([128, L, NC_CH], f32)
            nc.sync.dma_start(out=ln1s_sb, in_=ln1s_d.rearrange("l (i p) -> p l i", p=128))
            ln1b_sb = const.tile([128, L, NC_CH], f32)
            nc.sync.dma_start(out=ln1b_sb, in_=ln1b_d.rearrange("l (i p) -> p l i", p=128))
            ln2s_sb = const.tile([128, L, NC_CH], f32)
            nc.sync.dma_start(out=ln2s_sb, in_=ln2s_d.rearrange("l (i p) -> p l i", p=128))
            ln2b_sb = const.tile([128, L, NC_CH], f32)
            nc.sync.dma_start(out=ln2b_sb, in_=ln2b_d.rearrange("l (i p) -> p l i", p=128))
            lnfs_sb = const.tile([128, NC_CH], f32)
            nc.sync.dma_start(out=lnfs_sb, in_=lnfs_d.rearrange("(i p) -> p i", p=128))
            lnfb_sb = const.tile([128, NC_CH], f32)
            nc.sync.dma_start(out=lnfb_sb, in_=lnfb_d.rearrange("(i p) -> p i", p=128))
            b1_sb = const.tile([128, L, NF_CH], f32)
            nc.sync.dma_start(out=b1_sb, in_=b1_d.rearrange("l (i p) -> p l i", p=128))
            headw_sb = const.tile([128, NC_CH, V], f16)
            nc.sync.dma_start(out=headw_sb, in_=headw_d.rearrange("(i p) v -> p i v", p=128))
            headb_sb = const.tile([1, V], f16)
            nc.sync.dma_start(out=headb_sb, in_=headb_d[:, :])
            tgt_i = const.tile([128, NT], i32)
            nc.sync.dma_start(out=tgt_i, in_=tgt_d.rearrange("b (n p) -> p (b n)", p=128))
            tgt_sb = const.tile([128, NT], f32)
            nc.vector.tensor_copy(tgt_sb, tgt_i)
            loss_cols = const.tile([128, NT], f32)

            x_tm = resid.tile([128, NT, C], f32)     # persistent residual stream

            # ================= embedding =================
            with nc.named_scope("embed"):
                for g in range(NG):
                    idxb = small.tile([V, 512], i32, tag="idxb")
                    nc.sync.dma_start(
                        out=idxb,
                        in_=bass.AP(idx_d, g * 512, [[0, V], [1, 512]]),
                    )
                    onehot = small.tile([V, 512], f16, tag="onehot")
                    nc.vector.tensor_tensor(out=onehot, in0=idxb, in1=viota_p, op=ALU.is_equal)
                    for j in range(4):
                        i = g * 4 + j
                        ps = ps_big.tile([128, 512], f32, tag="big")
                        nc.tensor.matmul(ps[:, 0:C], onehot[:, j * 128:(j + 1) * 128], tok_sb,
                                         start=True, stop=True)
                        nc.vector.tensor_add(x_tm[:, i, :], ps[:, 0:C], pos_sb[:, i % 2, :])

            # helper: LayerNorm (token-major stats) + transpose to feature-major with
            # scale/bias folded into the PSUM->SBUF copy. Returns [128, NC_CH, 512] f16.
            def ln_to_fm(g, s_ap, b_ap, tag):
                mv = stat.tile([128, 4, 2], f32, tag="mv")
                for j in range(4):
                    i = g * 4 + j
                    st6 = stat.tile([128, 6], f32, tag="st6")
                    nc.vector.bn_stats(out=st6, in_=x_tm[:, i, :])
                    nc.vector.bn_aggr(out=mv[:, j, :], in_=st6)
                std = stat.tile([128, 4], f32, tag="std")
                nc.scalar.activation(out=std, in_=mv[:, :, 1], func=AF.Sqrt,
                                     bias=eps_t, scale=1.0)
                rstd = stat.tile([128, 4], f32, tag="rstd")
                nc.vector.reciprocal_approx_fast(out=rstd, in_=std)
                nmr = stat.tile([128, 4], f32, tag="nmr")
                nc.vector.tensor_mul(nmr, mv[:, :, 0], rstd)
                nc.vector.tensor_scalar_mul(nmr, nmr, -1.0)
                nn_tm = tmg.tile([128, 4, C], f16, tag="nn_tm")
                for j in range(4):
                    i = g * 4 + j
                    nc.scalar.activation(out=nn_tm[:, j, :], in_=x_tm[:, i, :],
                                         func=AF.Identity, bias=nmr[:, j:j + 1],
                                         scale=rstd[:, j:j + 1])
                nn_fm = fmg.tile([128, NC_CH, 512], f16, tag=tag)
                for ci in range(NC_CH):
                    ps = ps_tr.tile([128, 512], f16, tag="tr")
                    for j in range(4):
                        nc.tensor.transpose(ps[:, j * 128:(j + 1) * 128],
                                            nn_tm[:, j, ci * 128:(ci + 1) * 128], ident)
                    nc.scalar.activation(out=nn_fm[:, ci, :], in_=ps, func=AF.Identity,
                                         bias=b_ap[:, ci:ci + 1], scale=s_ap[:, ci:ci + 1])
                return nn_fm

            # ================= layers =================
            for l in range(nlayers):
                wqkv_t = wA.tile([128, NC_CH, 3 * C], f16, tag="wqkv")
                nc.sync.dma_start(out=wqkv_t,
                                  in_=wqkv_d[l, :, :].rearrange("(i p) o -> p i o", p=128))
                projw_t = wA.tile([128, NC_CH, C], f16, tag="projw")
                nc.sync.dma_start(out=projw_t,
                                  in_=projw_d[l, :, :].rearrange("(i p) o -> p i o", p=128))
                w1_t = wB.tile([128, NC_CH, FF], f16, tag="w1")
                nc.sync.dma_start(out=w1_t,
                                  in_=w1_d[l, :, :].rearrange("(i p) o -> p i o", p=128))
                w2_t = wB.tile([128, NF_CH, C], f16, tag="w2")
                nc.sync.dma_start(out=w2_t,
                                  in_=w2_d[l, :, :].rearrange("(i p) o -> p i o", p=128))
                projb_sb = wA.tile([1, C], f16, tag="projb")
                nc.sync.dma_start(out=projb_sb, in_=projb_d[:, l * C:(l + 1) * C])
                b2_sb = wA.tile([1, C], f16, tag="b2")
                nc.sync.dma_start(out=b2_sb, in_=b2_d[:, l * C:(l + 1) * C])

                # ---- phase A: attention ----
                with nc.named_scope(f"l{l}_attn"):
                    for g in range(NG if do_attn else 0):
                        nn_fm = ln_to_fm(g, ln1s_sb[:, l, :], ln1b_sb[:, l, :], "nn_fm")
                        # q, k feature-major [128, ch, 512]
                        q_sb = qkp.tile([128, NC_CH, 512], f16, tag="q")
                        k_sb = qkp.tile([128, NC_CH, 512], f16, tag="k")
                        for dst, off in ((q_sb, 0), (k_sb, C)):
                            for m in range(NC_CH):
                                ps = ps_big.tile([128, 512], f32, tag="big")
                                for ci in range(NC_CH):
                                    nc.tensor.matmul(
                                        ps, wqkv_t[:, ci, off + m * 128:off + (m + 1) * 128],
                                        nn_fm[:, ci, :],
                                        start=(ci == 0), stop=(ci == NC_CH - 1))
                                nc.vector.tensor_copy(dst[:, m, :], ps)
                        # v token-major, augmented with ones column per head [128, 4, H, HS+1]
                        v_sb = vp.tile([128, 4, H, HS + 1], f16, tag="v")
                        for j in range(4):
                            ps = ps_big.tile([128, 512], f32, tag="big")
                            for ci in range(NC_CH):
                                nc.tensor.matmul(ps[:, 0:C],
                                                 nn_fm[:, ci, j * 128:(j + 1) * 128],
                                                 wqkv_t[:, ci, 2 * C:3 * C],
                                                 start=(ci == 0), stop=(ci == NC_CH - 1))
                            nc.vector.tensor_copy(
                                v_sb[:, j, :, 0:HS],
                                ps[:, 0:C].rearrange("p (h d) -> p h d", h=H))
                            nc.vector.memset(v_sb[:, j, :, HS:HS + 1], 1.0)
                        # attention per (batch-in-group, head) -> att token-major
                        att_tm = tmg.tile([128, 4, C], f16, tag="att_tm")
                        for bl in range(2):
                            toff = bl * 256
                            for h in range(H):
                                ch, hp_ = h // 2, (h % 2) * HS
                                sc = ps_sc.tile([128, 384], f32, tag="sc")
                                nc.tensor.matmul(sc[:, 0:256],
                                                 k_sb[hp_:hp_ + HS, ch, toff:toff + 128],
                                                 q_sb[hp_:hp_ + HS, ch, toff:toff + 256],
                                                 start=True, stop=True)
                                nc.tensor.matmul(sc[:, 256:384],
                                                 k_sb[hp_:hp_ + HS, ch, toff + 128:toff + 256],
                                                 q_sb[hp_:hp_ + HS, ch, toff + 128:toff + 256],
                                                 start=True, stop=True)
                                ex = exl.tile([128, 384], f16, tag="ex")
                                nc.scalar.activation(out=ex, in_=sc, func=AF.Exp, scale=SCALE)
                                # mask diagonal blocks (cols 0:128 and 256:384) in one
                                # DVE op: view as [128, 2, 128] with block step 256,
                                # broadcast tri along the block dim (step 0)
                                exv = ex.rearrange("p (a c) -> p a c", a=3)
                                exd = bass.AP(exv.tensor, exv.offset,
                                              [exv.ap[0], [2 * exv.ap[1][0], 2], exv.ap[2]])
                                trib = bass.AP(tri.tensor, tri.offset,
                                               [tri.ap[0], [0, 2], tri.ap[1]])
                                nc.vector.tensor_mul(exd, exd, trib)
                                av = ps_av.tile([128, 2, V], f32, tag="av")
                                nc.tensor.matmul(av[:, 0, :], ex[:, 0:128],
                                                 v_sb[:, bl * 2, h, :], start=True, stop=True)
                                nc.tensor.matmul(av[:, 1, :], ex[:, 128:256],
                                                 v_sb[:, bl * 2, h, :], start=True, stop=False)
                                nc.tensor.matmul(av[:, 1, :], ex[:, 256:384],
                                                 v_sb[:, bl * 2 + 1, h, :], start=False, stop=True)
                                for tt in range(2):
                                    r = stat.tile([128, 1], f32, tag="r")
                                    nc.vector.reciprocal_approx_fast(
                                        out=r, in_=av[:, tt, HS:HS + 1])
                                    nc.scalar.activation(
                                        out=att_tm[:, bl * 2 + tt, h * HS:(h + 1) * HS],
                                        in_=av[:, tt, 0:HS], func=AF.Copy, scale=r)
                        # att -> feature-major
                        att_fm = fmg.tile([128, NC_CH, 512], f16, tag="att_fm")
                        for ci in range(NC_CH):
                            ps = ps_tr.tile([128, 512], f16, tag="tr")
                            for j in range(4):
                                nc.tensor.transpose(ps[:, j * 128:(j + 1) * 128],
                                                    att_tm[:, j, ci * 128:(ci + 1) * 128], ident)
                            nc.vector.tensor_copy(att_fm[:, ci, :], ps)
                        # proj + residual
                        for j in range(4):
                            i = g * 4 + j
                            ps = ps_big.tile([128, 512], f32, tag="big")
                            for ci in range(NC_CH):
                                nc.tensor.matmul(ps[:, 0:C], att_fm[:, ci, j * 128:(j + 1) * 128],
                                                 projw_t[:, ci, :],
                                                 start=(ci == 0), stop=False)
                            nc.tensor.matmul(ps[:, 0:C], ones_r, projb_sb,
                                             start=False, stop=True)
                            nc.vector.tensor_add(x_tm[:, i, :], x_tm[:, i, :], ps[:, 0:C])

                # ---- phase B: MLP ----
                with nc.named_scope(f"l{l}_mlp"):
                    for g in range(NG if do_mlp else 0):
                        nn_fm = ln_to_fm(g, ln2s_sb[:, l, :], ln2b_sb[:, l, :], "nn_fm")
                        h_sb = hp.tile([128, NF_CH, 512], f16, tag="h")
                        for f in range(NF_CH):
                            ps = ps_big.tile([128, 512], f32, tag="big")
                            for ci in range(NC_CH):
                                nc.tensor.matmul(ps, w1_t[:, ci, f * 128:(f + 1) * 128],
                                                 nn_fm[:, ci, :],
                                                 start=(ci == 0), stop=(ci == NC_CH - 1))
                            nc.scalar.activation(out=h_sb[:, f, :], in_=ps, func=AF.Relu,
                                                 bias=b1_sb[:, l, f:f + 1], scale=1.0)
                        for j in range(4):
                            i = g * 4 + j
                            ps = ps_big.tile([128, 512], f32, tag="big")
                            for fi in range(NF_CH):
                                nc.tensor.matmul(ps[:, 0:C], h_sb[:, fi, j * 128:(j + 1) * 128],
                                                 w2_t[:, fi, :],
                                                 start=(fi == 0), stop=False)
                            nc.tensor.matmul(ps[:, 0:C], ones_r, b2_sb,
                                             start=False, stop=True)
                            nc.vector.tensor_add(x_tm[:, i, :], x_tm[:, i, :], ps[:, 0:C])

            # ================= final LN + head + loss =================
            with nc.named_scope("head"):
                for g in range(NG):
                    xf_fm = ln_to_fm(g, lnfs_sb, lnfb_sb, "nn_fm")
                    for j in range(4):
                        i = g * 4 + j
                        psl = ps_av.tile([128, 2, V], f32, tag="av")
                        for ci in range(NC_CH):
                            nc.tensor.matmul(psl[:, 0, :], xf_fm[:, ci, j * 128:(j + 1) * 128],
                                             headw_sb[:, ci, :],
                                             start=(ci == 0), stop=False)
                        nc.tensor.matmul(psl[:, 0, :], ones_r, headb_sb,
                                         start=False, stop=True)
                        lg = outp.tile([128, V], f32, tag="lg")
                        nc.scalar.activation(out=lg, in_=psl[:, 0, :], func=AF.Copy)
                        nc.sync.dma_start(out=logits_d[i * 128:(i + 1) * 128, :], in_=lg)
                        # loss pieces: lse - logits[target]
                        sel = outp.tile([128, V], f32, tag="sel")
                        nc.vector.tensor_scalar(out=sel, in0=viota_r,
                                                scalar1=tgt_sb[:, i:i + 1], scalar2=None,
                                                op0=ALU.is_equal)
                        scr = outp.tile([128, V], f32, tag="scr")
                        pick = stat.tile([128, 1], f32, tag="pick")
                        nc.vector.tensor_mul(scr, lg, sel)
                        nc.vector.reduce_sum(pick, scr, axis=mybir.AxisListType.X)
                        ex2 = outp.tile([128, V], f32, tag="ex2")
                        se = stat.tile([128, 1], f32, tag="se")
                        nc.scalar.activation(out=ex2, in_=lg, func=AF.Exp, accum_out=se)
                        lse = stat.tile([128, 1], f32, tag="lse")
                        nc.scalar.activation(out=lse, in_=se, func=AF.Ln)
                        nc.vector.tensor_sub(loss_cols[:, i:i + 1], lse, pick)
                pst = ps_sc.tile([1, NT], f32, tag="sc")
                nc.tensor.matmul(pst, ones_c32, loss_cols, start=True, stop=True)
                tot = stat.tile([1, 1], f32, tag="tot")
                nc.vector.reduce_sum(tot, pst, axis=mybir.AxisListType.X)
                nc.sync.dma_start(out=loss_d[:, :], in_=tot)

    nc.finalize()
    return nc


def _get_program():
    if "nc" not in _CACHE:
        _CACHE["nc"] = _build_program()
    return _CACHE["nc"]


def _prep_weights(inputs):
    f16 = np.float16

    def hcd(w):  # [L,H,C,HS] -> [L, C, H*HS]
        return np.ascontiguousarray(w.transpose(0, 2, 1, 3)).reshape(L, C, C)

    wqkv = np.concatenate(
        [hcd(np.asarray(inputs["wq"])), hcd(np.asarray(inputs["wk"])),
         hcd(np.asarray(inputs["wv"]))], axis=2).astype(f16)
    shared = {
        "tok_emb": np.asarray(inputs["tok_emb"]).astype(f16),
        "pos_emb": np.asarray(inputs["pos_emb"]).astype(f16),
        "wqkv": np.ascontiguousarray(wqkv),
        "proj_w": np.asarray(inputs["proj_w"]).astype(f16),
        "proj_b": np.asarray(inputs["proj_b"]).astype(f16).reshape(1, L * C),
        "w1": np.asarray(inputs["w1"]).astype(f16),
        "b1": np.asarray(inputs["b1"]).astype(np.float32),
        "w2": np.asarray(inputs["w2"]).astype(f16),
        "b2": np.asarray(inputs["b2"]).astype(f16).reshape(1, L * C),
        "ln1_s": np.asarray(inputs["ln1_s"]).astype(np.float32),
        "ln1_b": np.asarray(inputs["ln1_b"]).astype(np.float32),
        "ln2_s": np.asarray(inputs["ln2_s"]).astype(np.float32),
        "ln2_b": np.asarray(inputs["ln2_b"]).astype(np.float32),
        "lnf_s": np.asarray(inputs["lnf_s"]).astype(np.float32),
        "lnf_b": np.asarray(inputs["lnf_b"]).astype(np.float32),
        "head_w": np.asarray(inputs["head_w"]).astype(f16),
        "head_b": np.asarray(inputs["head_b"]).astype(f16).reshape(1, V),
    }
    return shared


def kernel(**inputs):
    from concourse.bass_utils import run_bass_kernel_spmd

    nc = _get_program()
    shared = _prep_weights(inputs)
    idx = np.ascontiguousarray(np.asarray(inputs["idx"], dtype=np.int32))
    tgt = np.ascontiguousarray(np.asarray(inputs["targets"], dtype=np.int32))

    in_maps = []
    for c in range(NCORES):
        m = dict(shared)
        m["idx"] = np.ascontiguousarray(idx[c * BSH:(c + 1) * BSH])
        m["targets"] = np.ascontiguousarray(tgt[c * BSH:(c + 1) * BSH])
        in_maps.append(m)

    res = run_bass_kernel_spmd(nc, in_maps, core_ids=list(range(NCORES)), trace=False)
    logits = np.concatenate([r["logits"] for r in res.results], axis=0)
    loss = sum(float(r["loss_sum"][0, 0]) for r in res.results) / (B * T)
    return logits.astype(np.float32), np.array(loss, dtype=np.float32)


# revision 31
# speedup vs baseline: 1.1914x; 1.1850x over previous
"""Bass/Trainium2 kernel for nn_BigramLanguageModel (6-layer GPT, B=128 T=256 C=384 H=6 V=65).

Strategy: pure data-parallel over batch across 8 NeuronCores (16 batch rows each),
weights replicated. Per core, a fully fused transformer forward:
  - residual stream token-major fp32 in SBUF ([128 tok, 384] tiles)
  - matmul operands fp16 (1 cy/row on PE), fp32 PSUM accumulation
  - LayerNorm stats via bn_stats/bn_aggr (token-major), scale/bias folded into the
    PSUM->SBUF copy after the PE transpose to feature-major
  - attention scores computed transposed [s, t]; softmax denominator via a ones
    column appended to V (row 64 of the AV matmul output); causal mask via
    gpsimd.affine_select; no max-subtraction (scores are tiny; softmax invariant)
  - embedding gather as one-hot matmul (one-hot via DMA-broadcast + is_equal)
  - loss (mean NLL) computed on device per core; host sums 8 partials
"""

import numpy as np

V, B, T, C, H, L = 65, 128, 256, 384, 6, 6
HS, FF = C // H, 4 * C
EPS = 1e-5
NCORES = 8
BSH = B // NCORES          # batch rows per core = 16
NTOK = BSH * T             # tokens per core = 4096
NT = NTOK // 128           # token tiles per core = 32
NG = NT // 4               # groups (512 tokens = 2 batch rows) = 8
NC_CH = C // 128           # 3 feature chunks
NF_CH = FF // 128          # 12 ff chunks
SCALE = C ** (-0.5)

_CACHE = {}


def _build_program(nlayers=L, do_attn=True, do_mlp=True, zero_pb=False, zero_b2=False):
    import concourse.bacc as bacc
    import concourse.bass as bass
    import concourse.mybir as mybir
    import concourse.tile as tile
    from concourse.masks import make_identity

    f16 = mybir.dt.float16
    f32 = mybir.dt.float32
    i32 = mybir.dt.int32
    AF = mybir.ActivationFunctionType
    ALU = mybir.AluOpType

    nc = bacc.Bacc(target_bir_lowering=False)

    # ---- DRAM I/O ----
    idx_d = nc.dram_tensor("idx", [BSH, T], i32, kind="ExternalInput")
    tgt_d = nc.dram_tensor("targets", [BSH, T], i32, kind="ExternalInput")
    tok_d = nc.dram_tensor("tok_emb", [V, C], f16, kind="ExternalInput")
    pos_d = nc.dram_tensor("pos_emb", [T, C], f16, kind="ExternalInput")
    wqkv_d = nc.dram_tensor("wqkv", [L, C, 3 * C], f16, kind="ExternalInput")
    projw_d = nc.dram_tensor("proj_w", [L, C, C], f16, kind="ExternalInput")
    projb_d = None if zero_pb else nc.dram_tensor("proj_b", [1, L * C], f16,
                                                  kind="ExternalInput")
    w1_d = nc.dram_tensor("w1", [L, C, FF], f16, kind="ExternalInput")
    b1_d = nc.dram_tensor("b1", [L, FF], f32, kind="ExternalInput")
    w2_d = nc.dram_tensor("w2", [L, FF, C], f16, kind="ExternalInput")
    b2_d = None if zero_b2 else nc.dram_tensor("b2", [1, L * C], f16,
                                               kind="ExternalInput")
    ln1s_d = nc.dram_tensor("ln1_s", [L, C], f32, kind="ExternalInput")
    ln1b_d = nc.dram_tensor("ln1_b", [L, C], f32, kind="ExternalInput")
    ln2s_d = nc.dram_tensor("ln2_s", [L, C], f32, kind="ExternalInput")
    ln2b_d = nc.dram_tensor("ln2_b", [L, C], f32, kind="ExternalInput")
    lnfs_d = nc.dram_tensor("lnf_s", [C], f32, kind="ExternalInput")
    lnfb_d = nc.dram_tensor("lnf_b", [C], f32, kind="ExternalInput")
    headw_d = nc.dram_tensor("head_w", [C, V], f16, kind="ExternalInput")
    headb_d = nc.dram_tensor("head_b", [1, V], f16, kind="ExternalInput")
    logits_d = nc.dram_tensor("logits", [NTOK, V], f32, kind="ExternalOutput")
    loss_d = nc.dram_tensor("loss_sum", [1, 1], f32, kind="ExternalOutput")

    with tile.TileContext(nc) as tc:
        import contextlib
        ctx = contextlib.ExitStack()
        with ctx:
            const = ctx.enter_context(tc.tile_pool(name="const", bufs=1))
            wA = ctx.enter_context(tc.tile_pool(name="wA", bufs=2))
            wB = ctx.enter_context(tc.tile_pool(name="wB", bufs=2))
            resid = ctx.enter_context(tc.tile_pool(name="resid", bufs=1))
            tmg = ctx.enter_context(tc.tile_pool(name="tmg", bufs=2))      # token-major group tiles
            fmg = ctx.enter_context(tc.tile_pool(name="fmg", bufs=2))      # feature-major group tiles
            qkp = ctx.enter_context(tc.tile_pool(name="qkp", bufs=2))
            vp = ctx.enter_context(tc.tile_pool(name="vp", bufs=2))
            exl = ctx.enter_context(tc.tile_pool(name="exl", bufs=3))
            hp = ctx.enter_context(tc.tile_pool(name="hp", bufs=2))
            stat = ctx.enter_context(tc.tile_pool(name="stat", bufs=4))
            small = ctx.enter_context(tc.tile_pool(name="small", bufs=2))
            outp = ctx.enter_context(tc.tile_pool(name="outp", bufs=4))
            ps_big = ctx.enter_context(tc.tile_pool(name="ps_big", bufs=2, space="PSUM"))
            ps_tr = ctx.enter_context(tc.tile_pool(name="ps_tr", bufs=2, space="PSUM"))
            ps_sc = ctx.enter_context(tc.tile_pool(name="ps_sc", bufs=2, space="PSUM"))
            ps_av = ctx.enter_context(tc.tile_pool(name="ps_av", bufs=2, space="PSUM"))

            # ---- constants / params resident in SBUF ----
            ident = const.tile([128, 128], f16)
            make_identity(nc, ident)
            ones_r = const.tile([1, 128], f16)       # K=1 lhsT for bias matmuls
            nc.vector.memset(ones_r, 1.0)
            ones_c32 = const.tile([128, 1], f32)     # K=128 lhsT for loss column-sum
            nc.vector.memset(ones_c32, 1.0)
            eps_t = const.tile([128, 1], f32)
            nc.vector.memset(eps_t, EPS)
            viota_p = const.tile([V, 512], i32)      # value = partition idx
            nc.gpsimd.iota(viota_p, pattern=[[0, 512]], base=0, channel_multiplier=1)
            viota_r = const.tile([128, V], f32)      # value = free idx (exact in f32)
            nc.gpsimd.iota(viota_r, pattern=[[1, V]], base=0, channel_multiplier=0,
                           allow_small_or_imprecise_dtypes=True)
            # causal triangle mask: tri[s, t] = 1.0 if t >= s else 0.0
            tri = const.tile([128, 128], f16)
            nc.vector.memset(tri, 1.0)
            nc.gpsimd.affine_select(out=tri, in_=tri, compare_op=ALU.is_ge, fill=0.0,
                                    base=0, pattern=[[1, 128]], channel_multiplier=-1)

            tok_sb = const.tile([V, C], f16)
            nc.sync.dma_start(out=tok_sb, in_=tok_d[:, :])
            pos_sb = const.tile([128, 2, C], f16)
            nc.sync.dma_start(out=pos_sb, in_=pos_d.rearrange("(i p) c -> p i c", p=128))
            ln1s_sb = const.tile([128, L, NC_CH], f32)
            nc.sync.dma_start(out=ln1s_sb, in_=ln1s_d.rearrange("l (i p) -> p l i", p=128))
            ln1b_sb = const.tile([128, L, NC_CH], f32)
            nc.sync.dma_start(out=ln1b_sb, in_=ln1b_d.rearrange("l (i p) -> p l i", p=128))
            ln2s_sb = const.tile([128, L, NC_CH], f32)
            nc.sync.dma_start(out=ln2s_sb, in_=ln2s_d.rearrange("l (i p) -> p l i", p=128))
            ln2b_sb = const.tile([128, L, NC_CH], f32)
            nc.sync.dma_start(out=ln2b_sb, in_=ln2b_d.rearrange("l (i p) -> p l i", p=128))
            lnfs_sb = const.tile([128, NC_CH], f32)
            nc.sync.dma_start(out=lnfs_sb, in_=lnfs_d.rearrange("(i p) -> p i", p=128))
            lnfb_sb = const.tile([128, NC_CH], f32)
            nc.sync.dma_start(out=lnfb_sb, in_=lnfb_d.rearrange("(i p) -> p i", p=128))
            b1_sb = const.tile([128, L, NF_CH], f32)
            nc.sync.dma_start(out=b1_sb, in_=b1_d.rearrange("l (i p) -> p l i", p=128))
            headw_sb = const.tile([128, NC_CH, V], f16)
            nc.sync.dma_start(out=headw_sb, in_=headw_d.rearrange("(i p) v -> p i v", p=128))
            headb_sb = const.tile([1, V], f16)
            nc.sync.dma_start(out=headb_sb, in_=headb_d[:, :])
            tgt_i = const.tile([128, NT], i32)
            nc.sync.dma_start(out=tgt_i, in_=tgt_d.rearrange("b (n p) -> p (b n)", p=128))
            tgt_sb = const.tile([128, NT], f32)
            nc.vector.tensor_copy(tgt_sb, tgt_i)
            loss_cols = const.tile([128, NT], f32)

            x_tm = resid.tile([128, NT, C], f32)     # persistent residual stream

            # ================= embedding =================
            with nc.named_scope("embed"):
                for g in range(NG):
                    idxb = small.tile([V, 512], i32, tag="idxb")
                    nc.sync.dma_start(
                        out=idxb,
                        in_=bass.AP(idx_d, g * 512, [[0, V], [1, 512]]),
                    )
                    onehot = small.tile([V, 512], f16, tag="onehot")
                    nc.vector.tensor_tensor(out=onehot, in0=idxb, in1=viota_p, op=ALU.is_equal)
                    for j in range(4):
                        i = g * 4 + j
                        ps = ps_big.tile([128, 512], f32, tag="big")
                        nc.tensor.matmul(ps[:, 0:C], onehot[:, j * 128:(j + 1) * 128], tok_sb,
                                         start=True, stop=True)
                        nc.vector.tensor_add(x_tm[:, i, :], ps[:, 0:C], pos_sb[:, i % 2, :])

            # helper: LayerNorm (token-major stats) + transpose to feature-major with
            # scale/bias folded into the PSUM->SBUF copy. Returns [128, NC_CH, 512] f16.
            # rstd = exp(-0.5*ln(var+eps)) keeps ScalarE on one activation table
            # (natural_log_exp set) - Sqrt would force a table swap per group.
            def ln_to_fm(g, s_ap, b_ap, tag):
                mv = stat.tile([128, 4, 2], f32, tag="mv")
                for j in range(4):
                    i = g * 4 + j
                    st6 = stat.tile([128, 6], f32, tag="st6")
                    nc.vector.bn_stats(out=st6, in_=x_tm[:, i, :])
                    nc.vector.bn_aggr(out=mv[:, j, :], in_=st6)
                lnv = stat.tile([128, 4], f32, tag="lnv")
                nc.scalar.activation(out=lnv, in_=mv[:, :, 1], func=AF.Ln,
                                     bias=eps_t, scale=1.0)
                rstd = stat.tile([128, 4], f32, tag="rstd")
                nc.scalar.activation(out=rstd, in_=lnv, func=AF.Exp, scale=-0.5)
                nn_tm = tmg.tile([128, 4, C], f16, tag="nn_tm")
                for j in range(4):
                    i = g * 4 + j
                    nc.vector.tensor_scalar(out=nn_tm[:, j, :], in0=x_tm[:, i, :],
                                            scalar1=mv[:, j, 0:1], scalar2=rstd[:, j:j + 1],
                                            op0=ALU.subtract, op1=ALU.mult)
                nn_fm = fmg.tile([128, NC_CH, 512], f16, tag=tag)
                for ci in range(NC_CH):
                    ps = ps_tr.tile([128, 512], f16, tag="tr")
                    for j in range(4):
                        nc.tensor.transpose(ps[:, j * 128:(j + 1) * 128],
                                            nn_tm[:, j, ci * 128:(ci + 1) * 128], ident)
                    nc.scalar.activation(out=nn_fm[:, ci, :], in_=ps, func=AF.Identity,
                                         bias=b_ap[:, ci:ci + 1], scale=s_ap[:, ci:ci + 1])
                return nn_fm

            # ================= layers =================
            for l in range(nlayers):
                wqkv_t = wA.tile([128, NC_CH, 3 * C], f16, tag="wqkv")
                nc.sync.dma_start(out=wqkv_t,
                                  in_=wqkv_d[l, :, :].rearrange("(i p) o -> p i o", p=128))
                projw_t = wA.tile([128, NC_CH, C], f16, tag="projw")
                nc.sync.dma_start(out=projw_t,
                                  in_=projw_d[l, :, :].rearrange("(i p) o -> p i o", p=128))
                w1_t = wB.tile([128, NC_CH, FF], f16, tag="w1")
                nc.sync.dma_start(out=w1_t,
                                  in_=w1_d[l, :, :].rearrange("(i p) o -> p i o", p=128))
                w2_t = wB.tile([128, NF_CH, C], f16, tag="w2")
                nc.sync.dma_start(out=w2_t,
                                  in_=w2_d[l, :, :].rearrange("(i p) o -> p i o", p=128))
                if not zero_pb:
                    projb_sb = wA.tile([1, C], f16, tag="projb")
                    nc.sync.dma_start(out=projb_sb, in_=projb_d[:, l * C:(l + 1) * C])
                if not zero_b2:
                    b2_sb = wA.tile([1, C], f16, tag="b2")
                    nc.sync.dma_start(out=b2_sb, in_=b2_d[:, l * C:(l + 1) * C])

                # ---- phase A: attention ----
                with nc.named_scope(f"l{l}_attn"):
                    for g in range(NG if do_attn else 0):
                        nn_fm = ln_to_fm(g, ln1s_sb[:, l, :], ln1b_sb[:, l, :], "nn_fm")
                        # q, k feature-major [128, ch, 512]
                        q_sb = qkp.tile([128, NC_CH, 512], f16, tag="q")
                        k_sb = qkp.tile([128, NC_CH, 512], f16, tag="k")
                        for dst, off in ((q_sb, 0), (k_sb, C)):
                            for m in range(NC_CH):
                                ps = ps_big.tile([128, 512], f32, tag="big")
                                for ci in range(NC_CH):
                                    nc.tensor.matmul(
                                        ps, wqkv_t[:, ci, off + m * 128:off + (m + 1) * 128],
                                        nn_fm[:, ci, :],
                                        start=(ci == 0), stop=(ci == NC_CH - 1))
                                nc.vector.tensor_copy(dst[:, m, :], ps)
                        # v token-major, augmented with ones column per head [128, 4, H, HS+1]
                        v_sb = vp.tile([128, 4, H, HS + 1], f16, tag="v")
                        for j in range(4):
                            ps = ps_big.tile([128, 512], f32, tag="big")
                            for ci in range(NC_CH):
                                nc.tensor.matmul(ps[:, 0:C],
                                                 nn_fm[:, ci, j * 128:(j + 1) * 128],
                                                 wqkv_t[:, ci, 2 * C:3 * C],
                                                 start=(ci == 0), stop=(ci == NC_CH - 1))
                            nc.vector.tensor_copy(
                                v_sb[:, j, :, 0:HS],
                                ps[:, 0:C].rearrange("p (h d) -> p h d", h=H))
                            nc.vector.memset(v_sb[:, j, :, HS:HS + 1], 1.0)
                        # attention per (batch-in-group, head) -> att token-major
                        att_tm = tmg.tile([128, 4, C], f16, tag="att_tm")
                        for bl in range(2):
                            toff = bl * 256
                            for ch in range(NC_CH):
                                # scores for head pair (2ch, 2ch+1) run concurrently in
                                # the PE array via row tiling (K=64 each at rows 0/64)
                                scp = [ps_sc.tile([128, 384], f32, tag="sc",
                                                  name=f"sc{hh2}")
                                       for hh2 in range(2)]
                                for hh, sc in enumerate(scp):
                                    o = hh * HS
                                    nc.tensor.matmul(sc[:, 0:256],
                                                     k_sb[o:o + HS, ch, toff:toff + 128],
                                                     q_sb[o:o + HS, ch, toff:toff + 256],
                                                     start=True, stop=True,
                                                     tile_position=(o, 0))
                                    nc.tensor.matmul(sc[:, 256:384],
                                                     k_sb[o:o + HS, ch, toff + 128:toff + 256],
                                                     q_sb[o:o + HS, ch, toff + 128:toff + 256],
                                                     start=True, stop=True,
                                                     tile_position=(o, 0))
                                for hh, sc in enumerate(scp):
                                    h = 2 * ch + hh
                                    ex = exl.tile([128, 384], f16, tag="ex")
                                    nc.scalar.activation(out=ex, in_=sc, func=AF.Exp,
                                                         scale=SCALE)
                                    # mask diagonal blocks (cols 0:128 / 256:384) in one
                                    # op: view [128, 2, 128] block step 256, tri
                                    # broadcast along block dim (step 0)
                                    exv = ex.rearrange("p (a c) -> p a c", a=3)
                                    exd = bass.AP(exv.tensor, exv.offset,
                                                  [exv.ap[0], [2 * exv.ap[1][0], 2],
                                                   exv.ap[2]])
                                    trib = bass.AP(tri.tensor, tri.offset,
                                                   [tri.ap[0], [0, 2], tri.ap[1]])
                                    nc.gpsimd.tensor_mul(exd, exd, trib)
                                    av = ps_av.tile([128, 2, V], f32, tag="av")
                                    nc.tensor.matmul(av[:, 0, :], ex[:, 0:128],
                                                     v_sb[:, bl * 2, h, :],
                                                     start=True, stop=True)
                                    nc.tensor.matmul(av[:, 1, :], ex[:, 128:256],
                                                     v_sb[:, bl * 2, h, :],
                                                     start=True, stop=False)
                                    nc.tensor.matmul(av[:, 1, :], ex[:, 256:384],
                                                     v_sb[:, bl * 2 + 1, h, :],
                                                     start=False, stop=True)
                                    r2 = stat.tile([128, 2], f32, tag="r2")
                                    nc.vector.reciprocal_approx_fast(
                                        out=r2, in_=av[:, :, HS:HS + 1])
                                    for tt in range(2):
                                        nc.vector.tensor_scalar_mul(
                                            att_tm[:, bl * 2 + tt, h * HS:(h + 1) * HS],
                                            av[:, tt, 0:HS], r2[:, tt:tt + 1])
                        # att -> feature-major
                        att_fm = fmg.tile([128, NC_CH, 512], f16, tag="att_fm")
                        for ci in range(NC_CH):
                            ps = ps_tr.tile([128, 512], f16, tag="tr")
                            for j in range(4):
                                nc.tensor.transpose(ps[:, j * 128:(j + 1) * 128],
                                                    att_tm[:, j, ci * 128:(ci + 1) * 128], ident)
                            nc.vector.tensor_copy(att_fm[:, ci, :], ps)
                        # proj + residual
                        for j in range(4):
                            i = g * 4 + j
                            ps = ps_big.tile([128, 512], f32, tag="big")
                            for ci in range(NC_CH):
                                nc.tensor.matmul(ps[:, 0:C], att_fm[:, ci, j * 128:(j + 1) * 128],
                                                 projw_t[:, ci, :], start=(ci == 0),
                                                 stop=(zero_pb and ci == NC_CH - 1))
                            if not zero_pb:
                                nc.tensor.matmul(ps[:, 0:C], ones_r, projb_sb,
                                                 start=False, stop=True)
                            nc.vector.tensor_add(x_tm[:, i, :], x_tm[:, i, :], ps[:, 0:C])

                # ---- phase B: MLP ----
                with nc.named_scope(f"l{l}_mlp"):
                    for g in range(NG if do_mlp else 0):
                        nn_fm = ln_to_fm(g, ln2s_sb[:, l, :], ln2b_sb[:, l, :], "nn_fm")
                        h_sb = hp.tile([128, NF_CH, 512], f16, tag="h")
                        for f in range(NF_CH):
                            ps = ps_big.tile([128, 512], f32, tag="big")
                            for ci in range(NC_CH):
                                nc.tensor.matmul(ps, w1_t[:, ci, f * 128:(f + 1) * 128],
                                                 nn_fm[:, ci, :],
                                                 start=(ci == 0), stop=(ci == NC_CH - 1))
                            nc.scalar.activation(out=h_sb[:, f, :], in_=ps, func=AF.Relu,
                                                 bias=b1_sb[:, l, f:f + 1], scale=1.0)
                        for j in range(4):
                            i = g * 4 + j
                            ps = ps_big.tile([128, 512], f32, tag="big")
                            for fi in range(NF_CH):
                                nc.tensor.matmul(ps[:, 0:C], h_sb[:, fi, j * 128:(j + 1) * 128],
                                                 w2_t[:, fi, :], start=(fi == 0),
                                                 stop=(zero_b2 and fi == NF_CH - 1))
                            if not zero_b2:
                                nc.tensor.matmul(ps[:, 0:C], ones_r, b2_sb,
                                                 start=False, stop=True)
                            nc.vector.tensor_add(x_tm[:, i, :], x_tm[:, i, :], ps[:, 0:C])

            # ================= final LN + head + loss =================
            with nc.named_scope("head"):
                for g in range(NG):
                    xf_fm = ln_to_fm(g, lnfs_sb, lnfb_sb, "nn_fm")
                    for j in range(4):
                        i = g * 4 + j
                        psl = ps_av.tile([128, 2, V], f32, tag="av")
                        for ci in range(NC_CH):
                            nc.tensor.matmul(psl[:, 0, :], xf_fm[:, ci, j * 128:(j + 1) * 128],
                                             headw_sb[:, ci, :],
                                             start=(ci == 0), stop=False)
                        nc.tensor.matmul(psl[:, 0, :], ones_r, headb_sb,
                                         start=False, stop=True)
                        lg = outp.tile([128, V], f32, tag="lg")
                        nc.scalar.activation(out=lg, in_=psl[:, 0, :], func=AF.Copy)
                        nc.sync.dma_start(out=logits_d[i * 128:(i + 1) * 128, :], in_=lg)
                        # loss pieces: lse - logits[target]
                        sel = outp.tile([128, V], f32, tag="sel")
                        nc.vector.tensor_scalar(out=sel, in0=viota_r,
                                                scalar1=tgt_sb[:, i:i + 1], scalar2=None,
                                                op0=ALU.is_equal)
                        scr = outp.tile([128, V], f32, tag="scr")
                        pick = stat.tile([128, 1], f32, tag="pick")
                        nc.vector.tensor_mul(scr, lg, sel)
                        nc.vector.reduce_sum(pick, scr, axis=mybir.AxisListType.X)
                        ex2 = outp.tile([128, V], f32, tag="ex2")
                        se = stat.tile([128, 1], f32, tag="se")
                        nc.scalar.activation(out=ex2, in_=lg, func=AF.Exp, accum_out=se)
                        lse = stat.tile([128, 1], f32, tag="lse")
                        nc.scalar.activation(out=lse, in_=se, func=AF.Ln)
                        nc.vector.tensor_sub(loss_cols[:, i:i + 1], lse, pick)
                pst = ps_sc.tile([1, NT], f32, tag="sc")
                nc.tensor.matmul(pst, ones_c32, loss_cols, start=True, stop=True)
                tot = stat.tile([1, 1], f32, tag="tot")
                nc.vector.reduce_sum(tot, pst, axis=mybir.AxisListType.X)
                nc.sync.dma_start(out=loss_d[:, :], in_=tot)

    nc.finalize()
    return nc


def _get_program(zero_pb=False, zero_b2=False):
    key = ("nc", zero_pb, zero_b2)
    if key not in _CACHE:
        _CACHE[key] = _build_program(zero_pb=zero_pb, zero_b2=zero_b2)
    return _CACHE[key]


def _prep_weights(inputs):
    f16 = np.float16

    def hcd(w):  # [L,H,C,HS] -> [L, C, H*HS]
        return np.ascontiguousarray(w.transpose(0, 2, 1, 3)).reshape(L, C, C)

    wqkv = np.concatenate(
        [hcd(np.asarray(inputs["wq"])), hcd(np.asarray(inputs["wk"])),
         hcd(np.asarray(inputs["wv"]))], axis=2).astype(f16)
    shared = {
        "tok_emb": np.asarray(inputs["tok_emb"]).astype(f16),
        "pos_emb": np.asarray(inputs["pos_emb"]).astype(f16),
        "wqkv": np.ascontiguousarray(wqkv),
        "proj_w": np.asarray(inputs["proj_w"]).astype(f16),
        "proj_b": np.asarray(inputs["proj_b"]).astype(f16).reshape(1, L * C),
        "w1": np.asarray(inputs["w1"]).astype(f16),
        "b1": np.asarray(inputs["b1"]).astype(np.float32),
        "w2": np.asarray(inputs["w2"]).astype(f16),
        "b2": np.asarray(inputs["b2"]).astype(f16).reshape(1, L * C),
        "ln1_s": np.asarray(inputs["ln1_s"]).astype(np.float32),
        "ln1_b": np.asarray(inputs["ln1_b"]).astype(np.float32),
        "ln2_s": np.asarray(inputs["ln2_s"]).astype(np.float32),
        "ln2_b": np.asarray(inputs["ln2_b"]).astype(np.float32),
        "lnf_s": np.asarray(inputs["lnf_s"]).astype(np.float32),
        "lnf_b": np.asarray(inputs["lnf_b"]).astype(np.float32),
        "head_w": np.asarray(inputs["head_w"]).astype(f16),
        "head_b": np.asarray(inputs["head_b"]).astype(f16).reshape(1, V),
    }
    return shared


def _prepare_run(inputs):
    zero_pb = not np.any(np.asarray(inputs["proj_b"]))
    zero_b2 = not np.any(np.asarray(inputs["b2"]))
    nc = _get_program(zero_pb=zero_pb, zero_b2=zero_b2)
    shared = _prep_weights(inputs)
    if zero_pb:
        shared.pop("proj_b")
    if zero_b2:
        shared.pop("b2")
    idx = np.ascontiguousarray(np.asarray(inputs["idx"], dtype=np.int32))
    tgt = np.ascontiguousarray(np.asarray(inputs["targets"], dtype=np.int32))
    in_maps = []
    for c in range(NCORES):
        m = dict(shared)
        m["idx"] = np.ascontiguousarray(idx[c * BSH:(c + 1) * BSH])
        m["targets"] = np.ascontiguousarray(tgt[c * BSH:(c + 1) * BSH])
        in_maps.append(m)
    return nc, in_maps


def kernel(**inputs):
    from concourse.bass_utils import run_bass_kernel_spmd

    nc, in_maps = _prepare_run(inputs)
    res = run_bass_kernel_spmd(nc, in_maps, core_ids=list(range(NCORES)), trace=False)
    logits = np.concatenate([r["logits"] for r in res.results], axis=0)
    loss = sum(float(r["loss_sum"][0, 0]) for r in res.results) / (B * T)
    return logits.astype(np.float32), np.array(loss, dtype=np.float32)


# revision 32
# speedup vs baseline: 1.3048x; 1.0952x over previous
"""Bass/Trainium2 kernel for nn_BigramLanguageModel (6-layer GPT, B=128 T=256 C=384 H=6 V=65).

Strategy: pure data-parallel over batch across 8 NeuronCores (16 batch rows each),
weights replicated. Per core, a fully fused transformer forward:
  - residual stream token-major fp32 in SBUF ([128 tok, 384] tiles)
  - matmul operands fp16 (1 cy/row on PE), fp32 PSUM accumulation
  - LayerNorm stats via bn_stats/bn_aggr (token-major), scale/bias folded into the
    PSUM->SBUF copy after the PE transpose to feature-major
  - attention scores computed transposed [s, t]; softmax denominator via a ones
    column appended to V (row 64 of the AV matmul output); causal mask via
    gpsimd.affine_select; no max-subtraction (scores are tiny; softmax invariant)
  - embedding gather as one-hot matmul (one-hot via DMA-broadcast + is_equal)
  - loss (mean NLL) computed on device per core; host sums 8 partials
"""

import numpy as np

V, B, T, C, H, L = 65, 128, 256, 384, 6, 6
HS, FF = C // H, 4 * C
EPS = 1e-5
NCORES = 8
BSH = B // NCORES          # batch rows per core = 16
NTOK = BSH * T             # tokens per core = 4096
NT = NTOK // 128           # token tiles per core = 32
NG = NT // 4               # groups (512 tokens = 2 batch rows) = 8
NC_CH = C // 128           # 3 feature chunks
NF_CH = FF // 128          # 12 ff chunks
SCALE = C ** (-0.5)

_CACHE = {}


def _build_program(nlayers=L, do_attn=True, do_mlp=True, zero_pb=False, zero_b2=False):
    import concourse.bacc as bacc
    import concourse.bass as bass
    import concourse.mybir as mybir
    import concourse.tile as tile
    from concourse.masks import make_identity

    f16 = mybir.dt.float16
    f32 = mybir.dt.float32
    i32 = mybir.dt.int32
    AF = mybir.ActivationFunctionType
    ALU = mybir.AluOpType

    nc = bacc.Bacc(target_bir_lowering=False)

    # ---- DRAM I/O ----
    idx_d = nc.dram_tensor("idx", [BSH, T], i32, kind="ExternalInput")
    tgt_d = nc.dram_tensor("targets", [BSH, T], i32, kind="ExternalInput")
    tok_d = nc.dram_tensor("tok_emb", [V, C], f16, kind="ExternalInput")
    pos_d = nc.dram_tensor("pos_emb", [T, C], f16, kind="ExternalInput")
    wqkv_d = nc.dram_tensor("wqkv", [L, C, 3 * C], f16, kind="ExternalInput")
    projw_d = nc.dram_tensor("proj_w", [L, C, C], f16, kind="ExternalInput")
    projb_d = None if zero_pb else nc.dram_tensor("proj_b", [1, L * C], f16,
                                                  kind="ExternalInput")
    w1_d = nc.dram_tensor("w1", [L, C, FF], f16, kind="ExternalInput")
    b1_d = nc.dram_tensor("b1", [L, FF], f32, kind="ExternalInput")
    w2_d = nc.dram_tensor("w2", [L, FF, C], f16, kind="ExternalInput")
    b2_d = None if zero_b2 else nc.dram_tensor("b2", [1, L * C], f16,
                                               kind="ExternalInput")
    ln1s_d = nc.dram_tensor("ln1_s", [L, C], f32, kind="ExternalInput")
    ln1b_d = nc.dram_tensor("ln1_b", [L, C], f32, kind="ExternalInput")
    ln2s_d = nc.dram_tensor("ln2_s", [L, C], f32, kind="ExternalInput")
    ln2b_d = nc.dram_tensor("ln2_b", [L, C], f32, kind="ExternalInput")
    lnfs_d = nc.dram_tensor("lnf_s", [C], f32, kind="ExternalInput")
    lnfb_d = nc.dram_tensor("lnf_b", [C], f32, kind="ExternalInput")
    headw_d = nc.dram_tensor("head_w", [C, V], f16, kind="ExternalInput")
    headb_d = nc.dram_tensor("head_b", [1, V], f16, kind="ExternalInput")
    logits_d = nc.dram_tensor("logits", [NTOK, V], f32, kind="ExternalOutput")
    loss_d = nc.dram_tensor("loss_sum", [1, 1], f32, kind="ExternalOutput")

    with tile.TileContext(nc) as tc:
        import contextlib
        ctx = contextlib.ExitStack()
        with ctx:
            const = ctx.enter_context(tc.tile_pool(name="const", bufs=1))
            wA = ctx.enter_context(tc.tile_pool(name="wA", bufs=2))
            wB = ctx.enter_context(tc.tile_pool(name="wB", bufs=2))
            resid = ctx.enter_context(tc.tile_pool(name="resid", bufs=1))
            tmg = ctx.enter_context(tc.tile_pool(name="tmg", bufs=2))      # token-major group tiles
            fmg = ctx.enter_context(tc.tile_pool(name="fmg", bufs=2))      # feature-major group tiles
            qkp = ctx.enter_context(tc.tile_pool(name="qkp", bufs=2))
            vp = ctx.enter_context(tc.tile_pool(name="vp", bufs=2))
            exl = ctx.enter_context(tc.tile_pool(name="exl", bufs=3))
            hp = ctx.enter_context(tc.tile_pool(name="hp", bufs=2))
            stat = ctx.enter_context(tc.tile_pool(name="stat", bufs=4))
            small = ctx.enter_context(tc.tile_pool(name="small", bufs=2))
            outp = ctx.enter_context(tc.tile_pool(name="outp", bufs=4))
            ps_big = ctx.enter_context(tc.tile_pool(name="ps_big", bufs=2, space="PSUM"))
            ps_tr = ctx.enter_context(tc.tile_pool(name="ps_tr", bufs=2, space="PSUM"))
            ps_sc = ctx.enter_context(tc.tile_pool(name="ps_sc", bufs=2, space="PSUM"))
            ps_av = ctx.enter_context(tc.tile_pool(name="ps_av", bufs=2, space="PSUM"))

            # pin ScalarE's activation table to natural_log_exp_and_others (covers
            # Copy/Identity/Exp/Ln/Relu) so the table-load pass inserts no swaps
            nc.scalar.add_instruction(mybir.InstLoadActFuncSet(
                name=nc.get_next_instruction_name(), act_func_set_id=6, ins=[], outs=[]))

            # ---- constants / params resident in SBUF ----
            ident = const.tile([128, 128], f16)
            make_identity(nc, ident)
            ones_r = const.tile([1, 128], f16)       # K=1 lhsT for bias matmuls
            nc.vector.memset(ones_r, 1.0)
            ones_c32 = const.tile([128, 1], f32)     # K=128 lhsT for loss column-sum
            nc.vector.memset(ones_c32, 1.0)
            eps_t = const.tile([128, 1], f32)
            nc.vector.memset(eps_t, EPS)
            viota_p = const.tile([V, 512], i32)      # value = partition idx
            nc.gpsimd.iota(viota_p, pattern=[[0, 512]], base=0, channel_multiplier=1)
            viota_r = const.tile([128, V], f32)      # value = free idx (exact in f32)
            nc.gpsimd.iota(viota_r, pattern=[[1, V]], base=0, channel_multiplier=0,
                           allow_small_or_imprecise_dtypes=True)
            # causal triangle mask: tri[s, t] = 1.0 if t >= s else 0.0
            tri = const.tile([128, 128], f16)
            nc.vector.memset(tri, 1.0)
            nc.gpsimd.affine_select(out=tri, in_=tri, compare_op=ALU.is_ge, fill=0.0,
                                    base=0, pattern=[[1, 128]], channel_multiplier=-1)

            tok_sb = const.tile([V, C], f16)
            nc.sync.dma_start(out=tok_sb, in_=tok_d[:, :])
            pos_sb = const.tile([128, 2, C], f16)
            nc.sync.dma_start(out=pos_sb, in_=pos_d.rearrange("(i p) c -> p i c", p=128))
            ln1s_sb = const.tile([128, L, NC_CH], f32)
            nc.sync.dma_start(out=ln1s_sb, in_=ln1s_d.rearrange("l (i p) -> p l i", p=128))
            ln1b_sb = const.tile([128, L, NC_CH], f32)
            nc.sync.dma_start(out=ln1b_sb, in_=ln1b_d.rearrange("l (i p) -> p l i", p=128))
            ln2s_sb = const.tile([128, L, NC_CH], f32)
            nc.sync.dma_start(out=ln2s_sb, in_=ln2s_d.rearrange("l (i p) -> p l i", p=128))
            ln2b_sb = const.tile([128, L, NC_CH], f32)
            nc.sync.dma_start(out=ln2b_sb, in_=ln2b_d.rearrange("l (i p) -> p l i", p=128))
            lnfs_sb = const.tile([128, NC_CH], f32)
            nc.sync.dma_start(out=lnfs_sb, in_=lnfs_d.rearrange("(i p) -> p i", p=128))
            lnfb_sb = const.tile([128, NC_CH], f32)
            nc.sync.dma_start(out=lnfb_sb, in_=lnfb_d.rearrange("(i p) -> p i", p=128))
            b1_sb = const.tile([128, L, NF_CH], f32)
            nc.sync.dma_start(out=b1_sb, in_=b1_d.rearrange("l (i p) -> p l i", p=128))
            headw_sb = const.tile([128, NC_CH, V], f16)
            nc.sync.dma_start(out=headw_sb, in_=headw_d.rearrange("(i p) v -> p i v", p=128))
            headb_sb = const.tile([1, V], f16)
            nc.sync.dma_start(out=headb_sb, in_=headb_d[:, :])
            tgt_i = const.tile([128, NT], i32)
            nc.sync.dma_start(out=tgt_i, in_=tgt_d.rearrange("b (n p) -> p (b n)", p=128))
            tgt_sb = const.tile([128, NT], f32)
            nc.vector.tensor_copy(tgt_sb, tgt_i)
            loss_cols = const.tile([128, NT], f32)

            x_tm = resid.tile([128, NT, C], f32)     # persistent residual stream

            # ================= embedding =================
            with nc.named_scope("embed"):
                for g in range(NG):
                    idxb = small.tile([V, 512], i32, tag="idxb")
                    nc.sync.dma_start(
                        out=idxb,
                        in_=bass.AP(idx_d, g * 512, [[0, V], [1, 512]]),
                    )
                    onehot = small.tile([V, 512], f16, tag="onehot")
                    nc.vector.tensor_tensor(out=onehot, in0=idxb, in1=viota_p, op=ALU.is_equal)
                    for j in range(4):
                        i = g * 4 + j
                        ps = ps_big.tile([128, 512], f32, tag="big")
                        nc.tensor.matmul(ps[:, 0:C], onehot[:, j * 128:(j + 1) * 128], tok_sb,
                                         start=True, stop=True)
                        nc.vector.tensor_add(x_tm[:, i, :], ps[:, 0:C], pos_sb[:, i % 2, :])

            # helper: LayerNorm (token-major stats) + transpose to feature-major with
            # scale/bias folded into the PSUM->SBUF copy. Returns [128, NC_CH, 512] f16.
            # rstd = exp(-0.5*ln(var+eps)) keeps ScalarE on one activation table
            # (natural_log_exp set) - Sqrt would force a table swap per group.
            def ln_to_fm(g, s_ap, b_ap, tag):
                mv = stat.tile([128, 4, 2], f32, tag="mv")
                for j in range(4):
                    i = g * 4 + j
                    st6 = stat.tile([128, 6], f32, tag="st6")
                    nc.vector.bn_stats(out=st6, in_=x_tm[:, i, :])
                    nc.vector.bn_aggr(out=mv[:, j, :], in_=st6)
                lnv = stat.tile([128, 4], f32, tag="lnv")
                nc.scalar.activation(out=lnv, in_=mv[:, :, 1], func=AF.Ln,
                                     bias=eps_t, scale=1.0)
                rstd = stat.tile([128, 4], f32, tag="rstd")
                nc.scalar.activation(out=rstd, in_=lnv, func=AF.Exp, scale=-0.5)
                nn_tm = tmg.tile([128, 4, C], f16, tag="nn_tm")
                for j in range(4):
                    i = g * 4 + j
                    nc.vector.tensor_scalar(out=nn_tm[:, j, :], in0=x_tm[:, i, :],
                                            scalar1=mv[:, j, 0:1], scalar2=rstd[:, j:j + 1],
                                            op0=ALU.subtract, op1=ALU.mult)
                nn_fm = fmg.tile([128, NC_CH, 512], f16, tag=tag)
                for ci in range(NC_CH):
                    ps = ps_tr.tile([128, 512], f16, tag="tr")
                    for j in range(4):
                        nc.tensor.transpose(ps[:, j * 128:(j + 1) * 128],
                                            nn_tm[:, j, ci * 128:(ci + 1) * 128], ident)
                    nc.scalar.activation(out=nn_fm[:, ci, :], in_=ps, func=AF.Identity,
                                         bias=b_ap[:, ci:ci + 1], scale=s_ap[:, ci:ci + 1])
                return nn_fm

            # ================= layers =================
            for l in range(nlayers):
                wqkv_t = wA.tile([128, NC_CH, 3 * C], f16, tag="wqkv")
                nc.sync.dma_start(out=wqkv_t,
                                  in_=wqkv_d[l, :, :].rearrange("(i p) o -> p i o", p=128))
                projw_t = wA.tile([128, NC_CH, C], f16, tag="projw")
                nc.sync.dma_start(out=projw_t,
                                  in_=projw_d[l, :, :].rearrange("(i p) o -> p i o", p=128))
                w1_t = wB.tile([128, NC_CH, FF], f16, tag="w1")
                nc.sync.dma_start(out=w1_t,
                                  in_=w1_d[l, :, :].rearrange("(i p) o -> p i o", p=128))
                w2_t = wB.tile([128, NF_CH, C], f16, tag="w2")
                nc.sync.dma_start(out=w2_t,
                                  in_=w2_d[l, :, :].rearrange("(i p) o -> p i o", p=128))
                if not zero_pb:
                    projb_sb = wA.tile([1, C], f16, tag="projb")
                    nc.sync.dma_start(out=projb_sb, in_=projb_d[:, l * C:(l + 1) * C])
                if not zero_b2:
                    b2_sb = wA.tile([1, C], f16, tag="b2")
                    nc.sync.dma_start(out=b2_sb, in_=b2_d[:, l * C:(l + 1) * C])

                # ---- phase A: attention ----
                with nc.named_scope(f"l{l}_attn"):
                    for g in range(NG if do_attn else 0):
                        nn_fm = ln_to_fm(g, ln1s_sb[:, l, :], ln1b_sb[:, l, :], "nn_fm")
                        # q, k feature-major [128, ch, 512]
                        q_sb = qkp.tile([128, NC_CH, 512], f16, tag="q")
                        k_sb = qkp.tile([128, NC_CH, 512], f16, tag="k")
                        for dst, off in ((q_sb, 0), (k_sb, C)):
                            for m in range(NC_CH):
                                ps = ps_big.tile([128, 512], f32, tag="big")
                                for ci in range(NC_CH):
                                    nc.tensor.matmul(
                                        ps, wqkv_t[:, ci, off + m * 128:off + (m + 1) * 128],
                                        nn_fm[:, ci, :],
                                        start=(ci == 0), stop=(ci == NC_CH - 1))
                                nc.scalar.copy(dst[:, m, :], ps)
                        # v token-major, augmented with ones column per head [128, 4, H, HS+1]
                        v_sb = vp.tile([128, 4, H, HS + 1], f16, tag="v")
                        for j in range(4):
                            ps = ps_big.tile([128, 512], f32, tag="big")
                            for ci in range(NC_CH):
                                nc.tensor.matmul(ps[:, 0:C],
                                                 nn_fm[:, ci, j * 128:(j + 1) * 128],
                                                 wqkv_t[:, ci, 2 * C:3 * C],
                                                 start=(ci == 0), stop=(ci == NC_CH - 1))
                            nc.scalar.copy(
                                v_sb[:, j, :, 0:HS],
                                ps[:, 0:C].rearrange("p (h d) -> p h d", h=H))
                            nc.vector.memset(v_sb[:, j, :, HS:HS + 1], 1.0)
                        # attention per (batch-in-group, head) -> att token-major
                        att_tm = tmg.tile([128, 4, C], f16, tag="att_tm")
                        for bl in range(2):
                            toff = bl * 256
                            for ch in range(NC_CH):
                                # scores for head pair (2ch, 2ch+1) run concurrently in
                                # the PE array via row tiling (K=64 each at rows 0/64)
                                scp = [ps_sc.tile([128, 384], f32, tag="sc",
                                                  name=f"sc{hh2}")
                                       for hh2 in range(2)]
                                for hh, sc in enumerate(scp):
                                    o = hh * HS
                                    nc.tensor.matmul(sc[:, 0:256],
                                                     k_sb[o:o + HS, ch, toff:toff + 128],
                                                     q_sb[o:o + HS, ch, toff:toff + 256],
                                                     start=True, stop=True,
                                                     tile_position=(o, 0))
                                    nc.tensor.matmul(sc[:, 256:384],
                                                     k_sb[o:o + HS, ch, toff + 128:toff + 256],
                                                     q_sb[o:o + HS, ch, toff + 128:toff + 256],
                                                     start=True, stop=True,
                                                     tile_position=(o, 0))
                                for hh, sc in enumerate(scp):
                                    h = 2 * ch + hh
                                    ex = exl.tile([128, 384], f16, tag="ex")
                                    nc.scalar.activation(out=ex, in_=sc, func=AF.Exp,
                                                         scale=SCALE)
                                    # mask diagonal blocks (cols 0:128 / 256:384) in one
                                    # op: view [128, 2, 128] block step 256, tri
                                    # broadcast along block dim (step 0)
                                    exv = ex.rearrange("p (a c) -> p a c", a=3)
                                    exd = bass.AP(exv.tensor, exv.offset,
                                                  [exv.ap[0], [2 * exv.ap[1][0], 2],
                                                   exv.ap[2]])
                                    trib = bass.AP(tri.tensor, tri.offset,
                                                   [tri.ap[0], [0, 2], tri.ap[1]])
                                    nc.vector.tensor_mul(exd, exd, trib)
                                    av = ps_av.tile([128, 2, V], f32, tag="av")
                                    nc.tensor.matmul(av[:, 0, :], ex[:, 0:128],
                                                     v_sb[:, bl * 2, h, :],
                                                     start=True, stop=True)
                                    nc.tensor.matmul(av[:, 1, :], ex[:, 128:256],
                                                     v_sb[:, bl * 2, h, :],
                                                     start=True, stop=False)
                                    nc.tensor.matmul(av[:, 1, :], ex[:, 256:384],
                                                     v_sb[:, bl * 2 + 1, h, :],
                                                     start=False, stop=True)
                                    r2 = stat.tile([128, 2], f32, tag="r2")
                                    nc.vector.reciprocal_approx_fast(
                                        out=r2, in_=av[:, :, HS:HS + 1])
                                    for tt in range(2):
                                        nc.vector.tensor_scalar_mul(
                                            att_tm[:, bl * 2 + tt, h * HS:(h + 1) * HS],
                                            av[:, tt, 0:HS], r2[:, tt:tt + 1])
                        # att -> feature-major
                        att_fm = fmg.tile([128, NC_CH, 512], f16, tag="att_fm")
                        for ci in range(NC_CH):
                            ps = ps_tr.tile([128, 512], f16, tag="tr")
                            for j in range(4):
                                nc.tensor.transpose(ps[:, j * 128:(j + 1) * 128],
                                                    att_tm[:, j, ci * 128:(ci + 1) * 128], ident)
                            nc.scalar.copy(att_fm[:, ci, :], ps)
                        # proj + residual
                        for j in range(4):
                            i = g * 4 + j
                            ps = ps_big.tile([128, 512], f32, tag="big")
                            for ci in range(NC_CH):
                                nc.tensor.matmul(ps[:, 0:C], att_fm[:, ci, j * 128:(j + 1) * 128],
                                                 projw_t[:, ci, :], start=(ci == 0),
                                                 stop=(zero_pb and ci == NC_CH - 1))
                            if not zero_pb:
                                nc.tensor.matmul(ps[:, 0:C], ones_r, projb_sb,
                                                 start=False, stop=True)
                            nc.vector.tensor_add(x_tm[:, i, :], x_tm[:, i, :], ps[:, 0:C])

                # ---- phase B: MLP ----
                with nc.named_scope(f"l{l}_mlp"):
                    for g in range(NG if do_mlp else 0):
                        nn_fm = ln_to_fm(g, ln2s_sb[:, l, :], ln2b_sb[:, l, :], "nn_fm")
                        h_sb = hp.tile([128, NF_CH, 512], f16, tag="h")
                        for f in range(NF_CH):
                            ps = ps_big.tile([128, 512], f32, tag="big")
                            for ci in range(NC_CH):
                                nc.tensor.matmul(ps, w1_t[:, ci, f * 128:(f + 1) * 128],
                                                 nn_fm[:, ci, :],
                                                 start=(ci == 0), stop=(ci == NC_CH - 1))
                            nc.scalar.activation(out=h_sb[:, f, :], in_=ps, func=AF.Relu,
                                                 bias=b1_sb[:, l, f:f + 1], scale=1.0)
                        for j in range(4):
                            i = g * 4 + j
                            ps = ps_big.tile([128, 512], f32, tag="big")
                            for fi in range(NF_CH):
                                nc.tensor.matmul(ps[:, 0:C], h_sb[:, fi, j * 128:(j + 1) * 128],
                                                 w2_t[:, fi, :], start=(fi == 0),
                                                 stop=(zero_b2 and fi == NF_CH - 1))
                            if not zero_b2:
                                nc.tensor.matmul(ps[:, 0:C], ones_r, b2_sb,
                                                 start=False, stop=True)
                            nc.vector.tensor_add(x_tm[:, i, :], x_tm[:, i, :], ps[:, 0:C])

            # ================= final LN + head + loss =================
            with nc.named_scope("head"):
                for g in range(NG):
                    xf_fm = ln_to_fm(g, lnfs_sb, lnfb_sb, "nn_fm")
                    for j in range(4):
                        i = g * 4 + j
                        psl = ps_av.tile([128, 2, V], f32, tag="av")
                        for ci in range(NC_CH):
                            nc.tensor.matmul(psl[:, 0, :], xf_fm[:, ci, j * 128:(j + 1) * 128],
                                             headw_sb[:, ci, :],
                                             start=(ci == 0), stop=False)
                        nc.tensor.matmul(psl[:, 0, :], ones_r, headb_sb,
                                         start=False, stop=True)
                        lg = outp.tile([128, V], f32, tag="lg")
                        nc.scalar.activation(out=lg, in_=psl[:, 0, :], func=AF.Copy)
                        nc.sync.dma_start(out=logits_d[i * 128:(i + 1) * 128, :], in_=lg)
                        # loss pieces: lse - logits[target]
                        sel = outp.tile([128, V], f32, tag="sel")
                        nc.vector.tensor_scalar(out=sel, in0=viota_r,
                                                scalar1=tgt_sb[:, i:i + 1], scalar2=None,
                                                op0=ALU.is_equal)
                        scr = outp.tile([128, V], f32, tag="scr")
                        pick = stat.tile([128, 1], f32, tag="pick")
                        nc.vector.tensor_mul(scr, lg, sel)
                        nc.vector.reduce_sum(pick, scr, axis=mybir.AxisListType.X)
                        ex2 = outp.tile([128, V], f32, tag="ex2")
                        se = stat.tile([128, 1], f32, tag="se")
                        nc.scalar.activation(out=ex2, in_=lg, func=AF.Exp, accum_out=se)
                        lse = stat.tile([128, 1], f32, tag="lse")
                        nc.scalar.activation(out=lse, in_=se, func=AF.Ln)
                        nc.vector.tensor_sub(loss_cols[:, i:i + 1], lse, pick)
                pst = ps_sc.tile([1, NT], f32, tag="sc")
                nc.tensor.matmul(pst, ones_c32, loss_cols, start=True, stop=True)
                tot = stat.tile([1, 1], f32, tag="tot")
                nc.vector.reduce_sum(tot, pst, axis=mybir.AxisListType.X)
                nc.sync.dma_start(out=loss_d[:, :], in_=tot)

    nc.finalize()
    return nc


def _get_program(zero_pb=False, zero_b2=False):
    key = ("nc", zero_pb, zero_b2)
    if key not in _CACHE:
        _CACHE[key] = _build_program(zero_pb=zero_pb, zero_b2=zero_b2)
    return _CACHE[key]


def _prep_weights(inputs):
    f16 = np.float16

    def hcd(w):  # [L,H,C,HS] -> [L, C, H*HS]
        return np.ascontiguousarray(w.transpose(0, 2, 1, 3)).reshape(L, C, C)

    wqkv = np.concatenate(
        [hcd(np.asarray(inputs["wq"])), hcd(np.asarray(inputs["wk"])),
         hcd(np.asarray(inputs["wv"]))], axis=2).astype(f16)
    shared = {
        "tok_emb": np.asarray(inputs["tok_emb"]).astype(f16),
        "pos_emb": np.asarray(inputs["pos_emb"]).astype(f16),
        "wqkv": np.ascontiguousarray(wqkv),
        "proj_w": np.asarray(inputs["proj_w"]).astype(f16),
        "proj_b": np.asarray(inputs["proj_b"]).astype(f16).reshape(1, L * C),
        "w1": np.asarray(inputs["w1"]).astype(f16),
        "b1": np.asarray(inputs["b1"]).astype(np.float32),
        "w2": np.asarray(inputs["w2"]).astype(f16),
        "b2": np.asarray(inputs["b2"]).astype(f16).reshape(1, L * C),
        "ln1_s": np.asarray(inputs["ln1_s"]).astype(np.float32),
        "ln1_b": np.asarray(inputs["ln1_b"]).astype(np.float32),
        "ln2_s": np.asarray(inputs["ln2_s"]).astype(np.float32),
        "ln2_b": np.asarray(inputs["ln2_b"]).astype(np.float32),
        "lnf_s": np.asarray(inputs["lnf_s"]).astype(np.float32),
        "lnf_b": np.asarray(inputs["lnf_b"]).astype(np.float32),
        "head_w": np.asarray(inputs["head_w"]).astype(f16),
        "head_b": np.asarray(inputs["head_b"]).astype(f16).reshape(1, V),
    }
    return shared


def _prepare_run(inputs):
    zero_pb = not np.any(np.asarray(inputs["proj_b"]))
    zero_b2 = not np.any(np.asarray(inputs["b2"]))
    nc = _get_program(zero_pb=zero_pb, zero_b2=zero_b2)
    shared = _prep_weights(inputs)
    if zero_pb:
        shared.pop("proj_b")
    if zero_b2:
        shared.pop("b2")
    idx = np.ascontiguousarray(np.asarray(inputs["idx"], dtype=np.int32))
    tgt = np.ascontiguousarray(np.asarray(inputs["targets"], dtype=np.int32))
    in_maps = []
    for c in range(NCORES):
        m = dict(shared)
        m["idx"] = np.ascontiguousarray(idx[c * BSH:(c + 1) * BSH])
        m["targets"] = np.ascontiguousarray(tgt[c * BSH:(c + 1) * BSH])
        in_maps.append(m)
    return nc, in_maps


def kernel(**inputs):
    from concourse.bass_utils import run_bass_kernel_spmd

    nc, in_maps = _prepare_run(inputs)
    res = run_bass_kernel_spmd(nc, in_maps, core_ids=list(range(NCORES)), trace=False)
    logits = np.concatenate([r["logits"] for r in res.results], axis=0)
    loss = sum(float(r["loss_sum"][0, 0]) for r in res.results) / (B * T)
    return logits.astype(np.float32), np.array(loss, dtype=np.float32)


# revision 34
# speedup vs baseline: 1.3792x; 1.0571x over previous
"""Bass/Trainium2 kernel for nn_BigramLanguageModel (6-layer GPT, B=128 T=256 C=384 H=6 V=65).

Strategy: pure data-parallel over batch across 8 NeuronCores (16 batch rows each),
weights replicated. Per core, a fully fused transformer forward:
  - residual stream token-major fp32 in SBUF ([128 tok, 384] tiles)
  - matmul operands fp16 (1 cy/row on PE), fp32 PSUM accumulation
  - LayerNorm stats via bn_stats/bn_aggr (token-major), scale/bias folded into the
    PSUM->SBUF copy after the PE transpose to feature-major
  - attention scores computed transposed [s, t]; softmax denominator via a ones
    column appended to V (row 64 of the AV matmul output); causal mask via
    gpsimd.affine_select; no max-subtraction (scores are tiny; softmax invariant)
  - embedding gather as one-hot matmul (one-hot via DMA-broadcast + is_equal)
  - loss (mean NLL) computed on device per core; host sums 8 partials
"""

import numpy as np

V, B, T, C, H, L = 65, 128, 256, 384, 6, 6
HS, FF = C // H, 4 * C
EPS = 1e-5
NCORES = 8
BSH = B // NCORES          # batch rows per core = 16
NTOK = BSH * T             # tokens per core = 4096
NT = NTOK // 128           # token tiles per core = 32
NG = NT // 4               # groups (512 tokens = 2 batch rows) = 8
NC_CH = C // 128           # 3 feature chunks
NF_CH = FF // 128          # 12 ff chunks
SCALE = C ** (-0.5)

_CACHE = {}


def _build_program(nlayers=L, do_attn=True, do_mlp=True, zero_pb=False, zero_b2=False):
    import concourse.bacc as bacc
    import concourse.bass as bass
    import concourse.mybir as mybir
    import concourse.tile as tile
    from concourse.masks import make_identity

    f16 = mybir.dt.float16
    f32 = mybir.dt.float32
    i32 = mybir.dt.int32
    AF = mybir.ActivationFunctionType
    ALU = mybir.AluOpType

    nc = bacc.Bacc(target_bir_lowering=False)

    # ---- DRAM I/O ----
    idx_d = nc.dram_tensor("idx", [BSH, T], i32, kind="ExternalInput")
    tgt_d = nc.dram_tensor("targets", [BSH, T], i32, kind="ExternalInput")
    tok_d = nc.dram_tensor("tok_emb", [V, C], f16, kind="ExternalInput")
    pos_d = nc.dram_tensor("pos_emb", [T, C], f16, kind="ExternalInput")
    wqkv_d = nc.dram_tensor("wqkv", [L, C, 3 * C], f16, kind="ExternalInput")
    projw_d = nc.dram_tensor("proj_w", [L, C, C], f16, kind="ExternalInput")
    projb_d = None if zero_pb else nc.dram_tensor("proj_b", [1, L * C], f16,
                                                  kind="ExternalInput")
    w1_d = nc.dram_tensor("w1", [L, C, FF], f16, kind="ExternalInput")
    b1_d = nc.dram_tensor("b1", [L, FF], f32, kind="ExternalInput")
    w2_d = nc.dram_tensor("w2", [L, FF, C], f16, kind="ExternalInput")
    b2_d = None if zero_b2 else nc.dram_tensor("b2", [1, L * C], f16,
                                               kind="ExternalInput")
    ln1s_d = nc.dram_tensor("ln1_s", [L, C], f32, kind="ExternalInput")
    ln1b_d = nc.dram_tensor("ln1_b", [L, C], f32, kind="ExternalInput")
    ln2s_d = nc.dram_tensor("ln2_s", [L, C], f32, kind="ExternalInput")
    ln2b_d = nc.dram_tensor("ln2_b", [L, C], f32, kind="ExternalInput")
    lnfs_d = nc.dram_tensor("lnf_s", [C], f32, kind="ExternalInput")
    lnfb_d = nc.dram_tensor("lnf_b", [C], f32, kind="ExternalInput")
    headw_d = nc.dram_tensor("head_w", [C, V], f16, kind="ExternalInput")
    headb_d = nc.dram_tensor("head_b", [1, V], f16, kind="ExternalInput")
    logits_d = nc.dram_tensor("logits", [NTOK, V], f32, kind="ExternalOutput")
    loss_d = nc.dram_tensor("loss_sum", [1, 1], f32, kind="ExternalOutput")

    with tile.TileContext(nc) as tc:
        import contextlib
        ctx = contextlib.ExitStack()
        with ctx:
            const = ctx.enter_context(tc.tile_pool(name="const", bufs=1))
            wA = ctx.enter_context(tc.tile_pool(name="wA", bufs=2))
            wB = ctx.enter_context(tc.tile_pool(name="wB", bufs=2))
            resid = ctx.enter_context(tc.tile_pool(name="resid", bufs=1))
            tmg = ctx.enter_context(tc.tile_pool(name="tmg", bufs=2))      # token-major group tiles
            fmg = ctx.enter_context(tc.tile_pool(name="fmg", bufs=2))      # feature-major group tiles
            qkp = ctx.enter_context(tc.tile_pool(name="qkp", bufs=2))
            vp = ctx.enter_context(tc.tile_pool(name="vp", bufs=2))
            exl = ctx.enter_context(tc.tile_pool(name="exl", bufs=3))
            hp = ctx.enter_context(tc.tile_pool(name="hp", bufs=2))
            stat = ctx.enter_context(tc.tile_pool(name="stat", bufs=4))
            small = ctx.enter_context(tc.tile_pool(name="small", bufs=2))
            outp = ctx.enter_context(tc.tile_pool(name="outp", bufs=4))
            ps_big = ctx.enter_context(tc.tile_pool(name="ps_big", bufs=4, space="PSUM"))
            ps_tr = ctx.enter_context(tc.tile_pool(name="ps_tr", bufs=2, space="PSUM"))
            ps_av = ctx.enter_context(tc.tile_pool(name="ps_av", bufs=2, space="PSUM"))

            # pin ScalarE's activation table to natural_log_exp_and_others (covers
            # Copy/Identity/Exp/Ln/Relu) so the table-load pass inserts no swaps
            nc.scalar.add_instruction(mybir.InstLoadActFuncSet(
                name=nc.get_next_instruction_name(), act_func_set_id=6, ins=[], outs=[]))

            # ---- constants / params resident in SBUF ----
            ident = const.tile([128, 128], f16)
            make_identity(nc, ident)
            ones_r = const.tile([1, 128], f16)       # K=1 lhsT for bias matmuls
            nc.vector.memset(ones_r, 1.0)
            ones_c32 = const.tile([128, 1], f32)     # K=128 lhsT for loss column-sum
            nc.vector.memset(ones_c32, 1.0)
            eps_t = const.tile([128, 1], f32)
            nc.vector.memset(eps_t, EPS)
            viota_p = const.tile([V, 512], i32)      # value = partition idx
            nc.gpsimd.iota(viota_p, pattern=[[0, 512]], base=0, channel_multiplier=1)
            viota_r = const.tile([128, V], f32)      # value = free idx (exact in f32)
            nc.gpsimd.iota(viota_r, pattern=[[1, V]], base=0, channel_multiplier=0,
                           allow_small_or_imprecise_dtypes=True)
            # causal triangle mask: tri[s, t] = 1.0 if t >= s else 0.0
            tri = const.tile([128, 128], f16)
            nc.vector.memset(tri, 1.0)
            nc.gpsimd.affine_select(out=tri, in_=tri, compare_op=ALU.is_ge, fill=0.0,
                                    base=0, pattern=[[1, 128]], channel_multiplier=-1)

            tok_sb = const.tile([V, C], f16)
            nc.sync.dma_start(out=tok_sb, in_=tok_d[:, :])
            pos_sb = const.tile([128, 2, C], f16)
            nc.sync.dma_start(out=pos_sb, in_=pos_d.rearrange("(i p) c -> p i c", p=128))
            ln1s_sb = const.tile([128, L, NC_CH], f32)
            nc.sync.dma_start(out=ln1s_sb, in_=ln1s_d.rearrange("l (i p) -> p l i", p=128))
            ln1b_sb = const.tile([128, L, NC_CH], f32)
            nc.sync.dma_start(out=ln1b_sb, in_=ln1b_d.rearrange("l (i p) -> p l i", p=128))
            ln2s_sb = const.tile([128, L, NC_CH], f32)
            nc.sync.dma_start(out=ln2s_sb, in_=ln2s_d.rearrange("l (i p) -> p l i", p=128))
            ln2b_sb = const.tile([128, L, NC_CH], f32)
            nc.sync.dma_start(out=ln2b_sb, in_=ln2b_d.rearrange("l (i p) -> p l i", p=128))
            lnfs_sb = const.tile([128, NC_CH], f32)
            nc.sync.dma_start(out=lnfs_sb, in_=lnfs_d.rearrange("(i p) -> p i", p=128))
            lnfb_sb = const.tile([128, NC_CH], f32)
            nc.sync.dma_start(out=lnfb_sb, in_=lnfb_d.rearrange("(i p) -> p i", p=128))
            b1_sb = const.tile([128, L, NF_CH], f32)
            nc.sync.dma_start(out=b1_sb, in_=b1_d.rearrange("l (i p) -> p l i", p=128))
            headw_sb = const.tile([128, NC_CH, V], f16)
            nc.sync.dma_start(out=headw_sb, in_=headw_d.rearrange("(i p) v -> p i v", p=128))
            headb_sb = const.tile([1, V], f16)
            nc.sync.dma_start(out=headb_sb, in_=headb_d[:, :])
            tgt_i = const.tile([128, NT], i32)
            nc.sync.dma_start(out=tgt_i, in_=tgt_d.rearrange("b (n p) -> p (b n)", p=128))
            tgt_sb = const.tile([128, NT], f32)
            nc.vector.tensor_copy(tgt_sb, tgt_i)
            loss_cols = const.tile([128, NT], f32)

            x_tm = resid.tile([128, NT, C], f32)     # persistent residual stream

            # ================= embedding =================
            with nc.named_scope("embed"):
                for g in range(NG):
                    idxb = small.tile([V, 512], i32, tag="idxb")
                    nc.sync.dma_start(
                        out=idxb,
                        in_=bass.AP(idx_d, g * 512, [[0, V], [1, 512]]),
                    )
                    onehot = small.tile([V, 512], f16, tag="onehot")
                    nc.vector.tensor_tensor(out=onehot, in0=idxb, in1=viota_p, op=ALU.is_equal)
                    for j in range(4):
                        i = g * 4 + j
                        ps = ps_big.tile([128, 512], f32, tag="big")
                        nc.tensor.matmul(ps[:, 0:C], onehot[:, j * 128:(j + 1) * 128], tok_sb,
                                         start=True, stop=True)
                        nc.vector.tensor_add(x_tm[:, i, :], ps[:, 0:C], pos_sb[:, i % 2, :])

            # helper: LayerNorm (token-major stats) + transpose to feature-major with
            # scale/bias folded into the PSUM->SBUF copy. Returns [128, NC_CH, 512] f16.
            # rstd = exp(-0.5*ln(var+eps)) keeps ScalarE on one activation table
            # (natural_log_exp set) - Sqrt would force a table swap per group.
            def ln_to_fm(g, s_ap, b_ap, tag):
                mv = stat.tile([128, 4, 2], f32, tag="mv")
                for j in range(4):
                    i = g * 4 + j
                    st6 = stat.tile([128, 6], f32, tag="st6")
                    nc.vector.bn_stats(out=st6, in_=x_tm[:, i, :])
                    nc.vector.bn_aggr(out=mv[:, j, :], in_=st6)
                lnv = stat.tile([128, 4], f32, tag="lnv")
                nc.scalar.activation(out=lnv, in_=mv[:, :, 1], func=AF.Ln,
                                     bias=eps_t, scale=1.0)
                rstd = stat.tile([128, 4], f32, tag="rstd")
                nc.scalar.activation(out=rstd, in_=lnv, func=AF.Exp, scale=-0.5)
                nn_tm = tmg.tile([128, 4, C], f16, tag="nn_tm")
                for j in range(4):
                    i = g * 4 + j
                    nc.vector.tensor_scalar(out=nn_tm[:, j, :], in0=x_tm[:, i, :],
                                            scalar1=mv[:, j, 0:1], scalar2=rstd[:, j:j + 1],
                                            op0=ALU.subtract, op1=ALU.mult)
                nn_fm = fmg.tile([128, NC_CH, 512], f16, tag=tag)
                for ci in range(NC_CH):
                    ps = ps_tr.tile([128, 512], f16, tag="tr")
                    for j in range(4):
                        nc.tensor.transpose(ps[:, j * 128:(j + 1) * 128],
                                            nn_tm[:, j, ci * 128:(ci + 1) * 128], ident)
                    nc.scalar.activation(out=nn_fm[:, ci, :], in_=ps, func=AF.Identity,
                                         bias=b_ap[:, ci:ci + 1], scale=s_ap[:, ci:ci + 1])
                return nn_fm

            # ================= layers =================
            for l in range(nlayers):
                wqkv_t = wA.tile([128, NC_CH, 3 * C], f16, tag="wqkv")
                nc.sync.dma_start(out=wqkv_t,
                                  in_=wqkv_d[l, :, :].rearrange("(i p) o -> p i o", p=128))
                projw_t = wA.tile([128, NC_CH, C], f16, tag="projw")
                nc.sync.dma_start(out=projw_t,
                                  in_=projw_d[l, :, :].rearrange("(i p) o -> p i o", p=128))
                w1_t = wB.tile([128, NC_CH, FF], f16, tag="w1")
                nc.sync.dma_start(out=w1_t,
                                  in_=w1_d[l, :, :].rearrange("(i p) o -> p i o", p=128))
                w2_t = wB.tile([128, NF_CH, C], f16, tag="w2")
                nc.sync.dma_start(out=w2_t,
                                  in_=w2_d[l, :, :].rearrange("(i p) o -> p i o", p=128))
                if not zero_pb:
                    projb_sb = wA.tile([1, C], f16, tag="projb")
                    nc.sync.dma_start(out=projb_sb, in_=projb_d[:, l * C:(l + 1) * C])
                if not zero_b2:
                    b2_sb = wA.tile([1, C], f16, tag="b2")
                    nc.sync.dma_start(out=b2_sb, in_=b2_d[:, l * C:(l + 1) * C])

                # ---- phase A: attention ----
                with nc.named_scope(f"l{l}_attn"):
                    for g in range(NG if do_attn else 0):
                        nn_fm = ln_to_fm(g, ln1s_sb[:, l, :], ln1b_sb[:, l, :], "nn_fm")
                        # q, k feature-major [128, ch, 512]
                        q_sb = qkp.tile([128, NC_CH, 512], f16, tag="q")
                        k_sb = qkp.tile([128, NC_CH, 512], f16, tag="k")
                        for dst, off in ((q_sb, 0), (k_sb, C)):
                            for m in range(NC_CH):
                                ps = ps_big.tile([128, 512], f32, tag="big")
                                for ci in range(NC_CH):
                                    nc.tensor.matmul(
                                        ps, wqkv_t[:, ci, off + m * 128:off + (m + 1) * 128],
                                        nn_fm[:, ci, :],
                                        start=(ci == 0), stop=(ci == NC_CH - 1))
                                nc.scalar.copy(dst[:, m, :], ps)
                        # v token-major, augmented with ones column per head [128, 4, H, HS+1]
                        v_sb = vp.tile([128, 4, H, HS + 1], f16, tag="v")
                        for j in range(4):
                            ps = ps_big.tile([128, 512], f32, tag="big")
                            for ci in range(NC_CH):
                                nc.tensor.matmul(ps[:, 0:C],
                                                 nn_fm[:, ci, j * 128:(j + 1) * 128],
                                                 wqkv_t[:, ci, 2 * C:3 * C],
                                                 start=(ci == 0), stop=(ci == NC_CH - 1))
                            nc.scalar.copy(
                                v_sb[:, j, :, 0:HS],
                                ps[:, 0:C].rearrange("p (h d) -> p h d", h=H))
                            nc.vector.memset(v_sb[:, j, :, HS:HS + 1], 1.0)
                        # attention per (batch-in-group, head) -> att token-major
                        att_tm = tmg.tile([128, 4, C], f16, tag="att_tm")
                        for bl in range(2):
                            toff = bl * 256
                            for ch in range(NC_CH):
                                # scores for head pair (2ch, 2ch+1) run concurrently in
                                # the PE array via row tiling (K=64 each at rows 0/64)
                                scp = [ps_big.tile([128, 512], f32, tag="big",
                                                   name=f"sc{hh2}")
                                       for hh2 in range(2)]
                                for hh, sc in enumerate(scp):
                                    o = hh * HS
                                    nc.tensor.matmul(sc[:, 0:256],
                                                     k_sb[o:o + HS, ch, toff:toff + 128],
                                                     q_sb[o:o + HS, ch, toff:toff + 256],
                                                     start=True, stop=True,
                                                     tile_position=(o, 0))
                                    nc.tensor.matmul(sc[:, 256:384],
                                                     k_sb[o:o + HS, ch, toff + 128:toff + 256],
                                                     q_sb[o:o + HS, ch, toff + 128:toff + 256],
                                                     start=True, stop=True,
                                                     tile_position=(o, 0))
                                for hh, sc in enumerate(scp):
                                    h = 2 * ch + hh
                                    ex = exl.tile([128, 384], f16, tag="ex")
                                    nc.scalar.activation(out=ex, in_=sc[:, 0:384], func=AF.Exp,
                                                         scale=SCALE)
                                    # mask diagonal blocks (cols 0:128 / 256:384) in one
                                    # op: view [128, 2, 128] block step 256, tri
                                    # broadcast along block dim (step 0)
                                    exv = ex.rearrange("p (a c) -> p a c", a=3)
                                    exd = bass.AP(exv.tensor, exv.offset,
                                                  [exv.ap[0], [2 * exv.ap[1][0], 2],
                                                   exv.ap[2]])
                                    trib = bass.AP(tri.tensor, tri.offset,
                                                   [tri.ap[0], [0, 2], tri.ap[1]])
                                    nc.vector.tensor_mul(exd, exd, trib)
                                    av = ps_av.tile([128, 2, V], f32, tag="av")
                                    nc.tensor.matmul(av[:, 0, :], ex[:, 0:128],
                                                     v_sb[:, bl * 2, h, :],
                                                     start=True, stop=True)
                                    nc.tensor.matmul(av[:, 1, :], ex[:, 128:256],
                                                     v_sb[:, bl * 2, h, :],
                                                     start=True, stop=False)
                                    nc.tensor.matmul(av[:, 1, :], ex[:, 256:384],
                                                     v_sb[:, bl * 2 + 1, h, :],
                                                     start=False, stop=True)
                                    r2 = stat.tile([128, 2], f32, tag="r2")
                                    nc.vector.reciprocal_approx_fast(
                                        out=r2, in_=av[:, :, HS:HS + 1])
                                    for tt in range(2):
                                        nc.vector.tensor_scalar_mul(
                                            att_tm[:, bl * 2 + tt, h * HS:(h + 1) * HS],
                                            av[:, tt, 0:HS], r2[:, tt:tt + 1])
                        # att -> feature-major
                        att_fm = fmg.tile([128, NC_CH, 512], f16, tag="att_fm")
                        for ci in range(NC_CH):
                            ps = ps_tr.tile([128, 512], f16, tag="tr")
                            for j in range(4):
                                nc.tensor.transpose(ps[:, j * 128:(j + 1) * 128],
                                                    att_tm[:, j, ci * 128:(ci + 1) * 128], ident)
                            nc.scalar.copy(att_fm[:, ci, :], ps)
                        # proj + residual
                        for j in range(4):
                            i = g * 4 + j
                            ps = ps_big.tile([128, 512], f32, tag="big")
                            for ci in range(NC_CH):
                                nc.tensor.matmul(ps[:, 0:C], att_fm[:, ci, j * 128:(j + 1) * 128],
                                                 projw_t[:, ci, :], start=(ci == 0),
                                                 stop=(zero_pb and ci == NC_CH - 1))
                            if not zero_pb:
                                nc.tensor.matmul(ps[:, 0:C], ones_r, projb_sb,
                                                 start=False, stop=True)
                            nc.vector.tensor_add(x_tm[:, i, :], x_tm[:, i, :], ps[:, 0:C])

                # ---- phase B: MLP ----
                with nc.named_scope(f"l{l}_mlp"):
                    for g in range(NG if do_mlp else 0):
                        nn_fm = ln_to_fm(g, ln2s_sb[:, l, :], ln2b_sb[:, l, :], "nn_fm")
                        h_sb = hp.tile([128, NF_CH, 512], f16, tag="h")
                        for f in range(NF_CH):
                            ps = ps_big.tile([128, 512], f32, tag="big")
                            for ci in range(NC_CH):
                                nc.tensor.matmul(ps, w1_t[:, ci, f * 128:(f + 1) * 128],
                                                 nn_fm[:, ci, :],
                                                 start=(ci == 0), stop=(ci == NC_CH - 1))
                            nc.vector.tensor_scalar(out=h_sb[:, f, :], in0=ps,
                                                    scalar1=b1_sb[:, l, f:f + 1],
                                                    scalar2=0.0, op0=ALU.add,
                                                    op1=ALU.max)
                        for j in range(4):
                            i = g * 4 + j
                            ps = ps_big.tile([128, 512], f32, tag="big")
                            for fi in range(NF_CH):
                                nc.tensor.matmul(ps[:, 0:C], h_sb[:, fi, j * 128:(j + 1) * 128],
                                                 w2_t[:, fi, :], start=(fi == 0),
                                                 stop=(zero_b2 and fi == NF_CH - 1))
                            if not zero_b2:
                                nc.tensor.matmul(ps[:, 0:C], ones_r, b2_sb,
                                                 start=False, stop=True)
                            nc.vector.tensor_add(x_tm[:, i, :], x_tm[:, i, :], ps[:, 0:C])

            # ================= final LN + head + loss =================
            with nc.named_scope("head"):
                for g in range(NG):
                    xf_fm = ln_to_fm(g, lnfs_sb, lnfb_sb, "nn_fm")
                    for j in range(4):
                        i = g * 4 + j
                        psl = ps_av.tile([128, 2, V], f32, tag="av")
                        for ci in range(NC_CH):
                            nc.tensor.matmul(psl[:, 0, :], xf_fm[:, ci, j * 128:(j + 1) * 128],
                                             headw_sb[:, ci, :],
                                             start=(ci == 0), stop=False)
                        nc.tensor.matmul(psl[:, 0, :], ones_r, headb_sb,
                                         start=False, stop=True)
                        lg = outp.tile([128, V], f32, tag="lg")
                        nc.scalar.activation(out=lg, in_=psl[:, 0, :], func=AF.Copy)
                        nc.sync.dma_start(out=logits_d[i * 128:(i + 1) * 128, :], in_=lg)
                        # loss pieces: lse - logits[target]
                        sel = outp.tile([128, V], f32, tag="sel")
                        nc.vector.tensor_scalar(out=sel, in0=viota_r,
                                                scalar1=tgt_sb[:, i:i + 1], scalar2=None,
                                                op0=ALU.is_equal)
                        scr = outp.tile([128, V], f32, tag="scr")
                        pick = stat.tile([128, 1], f32, tag="pick")
                        nc.vector.tensor_mul(scr, lg, sel)
                        nc.vector.reduce_sum(pick, scr, axis=mybir.AxisListType.X)
                        ex2 = outp.tile([128, V], f32, tag="ex2")
                        se = stat.tile([128, 1], f32, tag="se")
                        nc.scalar.activation(out=ex2, in_=lg, func=AF.Exp, accum_out=se)
                        lse = stat.tile([128, 1], f32, tag="lse")
                        nc.scalar.activation(out=lse, in_=se, func=AF.Ln)
                        nc.vector.tensor_sub(loss_cols[:, i:i + 1], lse, pick)
                pst = ps_big.tile([1, NT], f32, tag="big")
                nc.tensor.matmul(pst, ones_c32, loss_cols, start=True, stop=True)
                tot = stat.tile([1, 1], f32, tag="tot")
                nc.vector.reduce_sum(tot, pst, axis=mybir.AxisListType.X)
                nc.sync.dma_start(out=loss_d[:, :], in_=tot)

    nc.finalize()
    return nc


def _get_program(zero_pb=False, zero_b2=False):
    key = ("nc", zero_pb, zero_b2)
    if key not in _CACHE:
        _CACHE[key] = _build_program(zero_pb=zero_pb, zero_b2=zero_b2)
    return _CACHE[key]


def _prep_weights(inputs):
    f16 = np.float16

    def hcd(w):  # [L,H,C,HS] -> [L, C, H*HS]
        return np.ascontiguousarray(w.transpose(0, 2, 1, 3)).reshape(L, C, C)

    wqkv = np.concatenate(
        [hcd(np.asarray(inputs["wq"])), hcd(np.asarray(inputs["wk"])),
         hcd(np.asarray(inputs["wv"]))], axis=2).astype(f16)
    shared = {
        "tok_emb": np.asarray(inputs["tok_emb"]).astype(f16),
        "pos_emb": np.asarray(inputs["pos_emb"]).astype(f16),
        "wqkv": np.ascontiguousarray(wqkv),
        "proj_w": np.asarray(inputs["proj_w"]).astype(f16),
        "proj_b": np.asarray(inputs["proj_b"]).astype(f16).reshape(1, L * C),
        "w1": np.asarray(inputs["w1"]).astype(f16),
        "b1": np.asarray(inputs["b1"]).astype(np.float32),
        "w2": np.asarray(inputs["w2"]).astype(f16),
        "b2": np.asarray(inputs["b2"]).astype(f16).reshape(1, L * C),
        "ln1_s": np.asarray(inputs["ln1_s"]).astype(np.float32),
        "ln1_b": np.asarray(inputs["ln1_b"]).astype(np.float32),
        "ln2_s": np.asarray(inputs["ln2_s"]).astype(np.float32),
        "ln2_b": np.asarray(inputs["ln2_b"]).astype(np.float32),
        "lnf_s": np.asarray(inputs["lnf_s"]).astype(np.float32),
        "lnf_b": np.asarray(inputs["lnf_b"]).astype(np.float32),
        "head_w": np.asarray(inputs["head_w"]).astype(f16),
        "head_b": np.asarray(inputs["head_b"]).astype(f16).reshape(1, V),
    }
    return shared


def _prepare_run(inputs):
    zero_pb = not np.any(np.asarray(inputs["proj_b"]))
    zero_b2 = not np.any(np.asarray(inputs["b2"]))
    nc = _get_program(zero_pb=zero_pb, zero_b2=zero_b2)
    shared = _prep_weights(inputs)
    if zero_pb:
        shared.pop("proj_b")
    if zero_b2:
        shared.pop("b2")
    idx = np.ascontiguousarray(np.asarray(inputs["idx"], dtype=np.int32))
    tgt = np.ascontiguousarray(np.asarray(inputs["targets"], dtype=np.int32))
    in_maps = []
    for c in range(NCORES):
        m = dict(shared)
        m["idx"] = np.ascontiguousarray(idx[c * BSH:(c + 1) * BSH])
        m["targets"] = np.ascontiguousarray(tgt[c * BSH:(c + 1) * BSH])
        in_maps.append(m)
    return nc, in_maps


def kernel(**inputs):
    from concourse.bass_utils import run_bass_kernel_spmd

    nc, in_maps = _prepare_run(inputs)
    res = run_bass_kernel_spmd(nc, in_maps, core_ids=list(range(NCORES)), trace=False)
    logits = np.concatenate([r["logits"] for r in res.results], axis=0)
    loss = sum(float(r["loss_sum"][0, 0]) for r in res.results) / (B * T)
    return logits.astype(np.float32), np.array(loss, dtype=np.float32)


# revision 35
# speedup vs baseline: 1.3812x; 1.0014x over previous
"""Bass/Trainium2 kernel for nn_BigramLanguageModel (6-layer GPT, B=128 T=256 C=384 H=6 V=65).

Strategy: pure data-parallel over batch across 8 NeuronCores (16 batch rows each),
weights replicated. Per core, a fully fused transformer forward:
  - residual stream token-major fp32 in SBUF ([128 tok, 384] tiles)
  - matmul operands fp16 (1 cy/row on PE), fp32 PSUM accumulation
  - LayerNorm stats via bn_stats/bn_aggr (token-major), scale/bias folded into the
    PSUM->SBUF copy after the PE transpose to feature-major
  - attention scores computed transposed [s, t]; softmax denominator via a ones
    column appended to V (row 64 of the AV matmul output); causal mask via
    gpsimd.affine_select; no max-subtraction (scores are tiny; softmax invariant)
  - embedding gather as one-hot matmul (one-hot via DMA-broadcast + is_equal)
  - loss (mean NLL) computed on device per core; host sums 8 partials
"""

import numpy as np

V, B, T, C, H, L = 65, 128, 256, 384, 6, 6
HS, FF = C // H, 4 * C
EPS = 1e-5
NCORES = 8
BSH = B // NCORES          # batch rows per core = 16
NTOK = BSH * T             # tokens per core = 4096
NT = NTOK // 128           # token tiles per core = 32
NG = NT // 4               # groups (512 tokens = 2 batch rows) = 8
NC_CH = C // 128           # 3 feature chunks
NF_CH = FF // 128          # 12 ff chunks
SCALE = C ** (-0.5)

_CACHE = {}


def _build_program(nlayers=L, do_attn=True, do_mlp=True, zero_pb=False, zero_b2=False):
    import concourse.bacc as bacc
    import concourse.bass as bass
    import concourse.mybir as mybir
    import concourse.tile as tile
    from concourse.masks import make_identity

    f16 = mybir.dt.float16
    f32 = mybir.dt.float32
    i32 = mybir.dt.int32
    AF = mybir.ActivationFunctionType
    ALU = mybir.AluOpType

    nc = bacc.Bacc(target_bir_lowering=False)

    # ---- DRAM I/O ----
    idx_d = nc.dram_tensor("idx", [BSH, T], i32, kind="ExternalInput")
    tgt_d = nc.dram_tensor("targets", [BSH, T], i32, kind="ExternalInput")
    tok_d = nc.dram_tensor("tok_emb", [V, C], f16, kind="ExternalInput")
    pos_d = nc.dram_tensor("pos_emb", [T, C], f16, kind="ExternalInput")
    wqkv_d = nc.dram_tensor("wqkv", [L, C, 3 * C], f16, kind="ExternalInput")
    projw_d = nc.dram_tensor("proj_w", [L, C, C], f16, kind="ExternalInput")
    projb_d = None if zero_pb else nc.dram_tensor("proj_b", [1, L * C], f16,
                                                  kind="ExternalInput")
    w1_d = nc.dram_tensor("w1", [L, C, FF], f16, kind="ExternalInput")
    b1_d = nc.dram_tensor("b1", [L, FF], f32, kind="ExternalInput")
    w2_d = nc.dram_tensor("w2", [L, FF, C], f16, kind="ExternalInput")
    b2_d = None if zero_b2 else nc.dram_tensor("b2", [1, L * C], f16,
                                               kind="ExternalInput")
    ln1s_d = nc.dram_tensor("ln1_s", [L, C], f32, kind="ExternalInput")
    ln1b_d = nc.dram_tensor("ln1_b", [L, C], f32, kind="ExternalInput")
    ln2s_d = nc.dram_tensor("ln2_s", [L, C], f32, kind="ExternalInput")
    ln2b_d = nc.dram_tensor("ln2_b", [L, C], f32, kind="ExternalInput")
    lnfs_d = nc.dram_tensor("lnf_s", [C], f32, kind="ExternalInput")
    lnfb_d = nc.dram_tensor("lnf_b", [C], f32, kind="ExternalInput")
    headw_d = nc.dram_tensor("head_w", [C, V], f16, kind="ExternalInput")
    headb_d = nc.dram_tensor("head_b", [1, V], f16, kind="ExternalInput")
    logits_d = nc.dram_tensor("logits", [NTOK, V], f32, kind="ExternalOutput")
    loss_d = nc.dram_tensor("loss_sum", [1, 1], f32, kind="ExternalOutput")

    with tile.TileContext(nc) as tc:
        import contextlib
        ctx = contextlib.ExitStack()
        with ctx:
            const = ctx.enter_context(tc.tile_pool(name="const", bufs=1))
            wA = ctx.enter_context(tc.tile_pool(name="wA", bufs=2))
            wB = ctx.enter_context(tc.tile_pool(name="wB", bufs=2))
            resid = ctx.enter_context(tc.tile_pool(name="resid", bufs=1))
            tmg = ctx.enter_context(tc.tile_pool(name="tmg", bufs=2))      # token-major group tiles
            fmg = ctx.enter_context(tc.tile_pool(name="fmg", bufs=2))      # feature-major group tiles
            qkp = ctx.enter_context(tc.tile_pool(name="qkp", bufs=2))
            vp = ctx.enter_context(tc.tile_pool(name="vp", bufs=2))
            exl = ctx.enter_context(tc.tile_pool(name="exl", bufs=3))
            hp = ctx.enter_context(tc.tile_pool(name="hp", bufs=2))
            stat = ctx.enter_context(tc.tile_pool(name="stat", bufs=4))
            small = ctx.enter_context(tc.tile_pool(name="small", bufs=2))
            outp = ctx.enter_context(tc.tile_pool(name="outp", bufs=4))
            ps_big = ctx.enter_context(tc.tile_pool(name="ps_big", bufs=4, space="PSUM"))
            ps_tr = ctx.enter_context(tc.tile_pool(name="ps_tr", bufs=2, space="PSUM"))
            ps_av = ctx.enter_context(tc.tile_pool(name="ps_av", bufs=2, space="PSUM"))

            # pin ScalarE's activation table to natural_log_exp_and_others (covers
            # Copy/Identity/Exp/Ln/Relu) so the table-load pass inserts no swaps
            nc.scalar.add_instruction(mybir.InstLoadActFuncSet(
                name=nc.get_next_instruction_name(), act_func_set_id=6, ins=[], outs=[]))

            # ---- constants / params resident in SBUF ----
            ident = const.tile([128, 128], f16)
            make_identity(nc, ident)
            ones_r = const.tile([1, 128], f16)       # K=1 lhsT for bias matmuls
            nc.vector.memset(ones_r, 1.0)
            ones_c32 = const.tile([128, 1], f32)     # K=128 lhsT for loss column-sum
            nc.vector.memset(ones_c32, 1.0)
            eps_t = const.tile([128, 1], f32)
            nc.vector.memset(eps_t, EPS)
            viota_p = const.tile([V, 512], i32)      # value = partition idx
            nc.gpsimd.iota(viota_p, pattern=[[0, 512]], base=0, channel_multiplier=1)
            viota_r = const.tile([128, V], f32)      # value = free idx (exact in f32)
            nc.gpsimd.iota(viota_r, pattern=[[1, V]], base=0, channel_multiplier=0,
                           allow_small_or_imprecise_dtypes=True)
            # causal triangle mask: tri[s, t] = 1.0 if t >= s else 0.0
            tri = const.tile([128, 128], f16)
            nc.vector.memset(tri, 1.0)
            nc.gpsimd.affine_select(out=tri, in_=tri, compare_op=ALU.is_ge, fill=0.0,
                                    base=0, pattern=[[1, 128]], channel_multiplier=-1)

            tok_sb = const.tile([V, C], f16)
            nc.sync.dma_start(out=tok_sb, in_=tok_d[:, :])
            pos_sb = const.tile([128, 2, C], f16)
            nc.sync.dma_start(out=pos_sb, in_=pos_d.rearrange("(i p) c -> p i c", p=128))
            ln1s_sb = const.tile([128, L, NC_CH], f32)
            nc.sync.dma_start(out=ln1s_sb, in_=ln1s_d.rearrange("l (i p) -> p l i", p=128))
            ln1b_sb = const.tile([128, L, NC_CH], f32)
            nc.sync.dma_start(out=ln1b_sb, in_=ln1b_d.rearrange("l (i p) -> p l i", p=128))
            ln2s_sb = const.tile([128, L, NC_CH], f32)
            nc.sync.dma_start(out=ln2s_sb, in_=ln2s_d.rearrange("l (i p) -> p l i", p=128))
            ln2b_sb = const.tile([128, L, NC_CH], f32)
            nc.sync.dma_start(out=ln2b_sb, in_=ln2b_d.rearrange("l (i p) -> p l i", p=128))
            lnfs_sb = const.tile([128, NC_CH], f32)
            nc.sync.dma_start(out=lnfs_sb, in_=lnfs_d.rearrange("(i p) -> p i", p=128))
            lnfb_sb = const.tile([128, NC_CH], f32)
            nc.sync.dma_start(out=lnfb_sb, in_=lnfb_d.rearrange("(i p) -> p i", p=128))
            b1_sb = const.tile([128, L, NF_CH], f32)
            nc.sync.dma_start(out=b1_sb, in_=b1_d.rearrange("l (i p) -> p l i", p=128))
            headw_sb = const.tile([128, NC_CH, V], f16)
            nc.sync.dma_start(out=headw_sb, in_=headw_d.rearrange("(i p) v -> p i v", p=128))
            headb_sb = const.tile([1, V], f16)
            nc.sync.dma_start(out=headb_sb, in_=headb_d[:, :])
            tgt_i = const.tile([128, NT], i32)
            nc.sync.dma_start(out=tgt_i, in_=tgt_d.rearrange("b (n p) -> p (b n)", p=128))
            tgt_sb = const.tile([128, NT], f32)
            nc.vector.tensor_copy(tgt_sb, tgt_i)
            loss_cols = const.tile([128, NT], f32)

            x_tm = resid.tile([128, NT, C], f32)     # persistent residual stream

            # ================= embedding =================
            with nc.named_scope("embed"):
                for g in range(NG):
                    idxb = small.tile([V, 512], i32, tag="idxb")
                    nc.sync.dma_start(
                        out=idxb,
                        in_=bass.AP(idx_d, g * 512, [[0, V], [1, 512]]),
                    )
                    onehot = small.tile([V, 512], f16, tag="onehot")
                    nc.vector.tensor_tensor(out=onehot, in0=idxb, in1=viota_p, op=ALU.is_equal)
                    for j in range(4):
                        i = g * 4 + j
                        ps = ps_big.tile([128, 512], f32, tag="big")
                        nc.tensor.matmul(ps[:, 0:C], onehot[:, j * 128:(j + 1) * 128], tok_sb,
                                         start=True, stop=True)
                        nc.vector.tensor_add(x_tm[:, i, :], ps[:, 0:C], pos_sb[:, i % 2, :])

            # helper: LayerNorm (token-major stats) + transpose to feature-major with
            # scale/bias folded into the PSUM->SBUF copy. Returns [128, NC_CH, 512] f16.
            # rstd = exp(-0.5*ln(var+eps)) keeps ScalarE on one activation table
            # (natural_log_exp set) - Sqrt would force a table swap per group.
            def ln_to_fm(g, s_ap, b_ap, tag):
                mv = stat.tile([128, 4, 2], f32, tag="mv")
                for j in range(4):
                    i = g * 4 + j
                    st6 = stat.tile([128, 6], f32, tag="st6")
                    nc.vector.bn_stats(out=st6, in_=x_tm[:, i, :])
                    nc.vector.bn_aggr(out=mv[:, j, :], in_=st6)
                lnv = stat.tile([128, 4], f32, tag="lnv")
                nc.scalar.activation(out=lnv, in_=mv[:, :, 1], func=AF.Ln,
                                     bias=eps_t, scale=1.0)
                rstd = stat.tile([128, 4], f32, tag="rstd")
                nc.scalar.activation(out=rstd, in_=lnv, func=AF.Exp, scale=-0.5)
                nn_tm = tmg.tile([128, 4, C], f16, tag="nn_tm")
                for j in range(4):
                    i = g * 4 + j
                    nc.vector.tensor_scalar(out=nn_tm[:, j, :], in0=x_tm[:, i, :],
                                            scalar1=mv[:, j, 0:1], scalar2=rstd[:, j:j + 1],
                                            op0=ALU.subtract, op1=ALU.mult)
                nn_fm = fmg.tile([128, NC_CH, 512], f16, tag=tag)
                for ci in range(NC_CH):
                    ps = ps_tr.tile([128, 512], f32, tag="tr")
                    for j in range(4):
                        nc.tensor.matmul(ps[:, j * 128:(j + 1) * 128],
                                         nn_tm[:, j, ci * 128:(ci + 1) * 128], ident,
                                         start=True, stop=True)
                    nc.scalar.activation(out=nn_fm[:, ci, :], in_=ps, func=AF.Identity,
                                         bias=b_ap[:, ci:ci + 1], scale=s_ap[:, ci:ci + 1])
                return nn_fm

            # ================= layers =================
            for l in range(nlayers):
                wqkv_t = wA.tile([128, NC_CH, 3 * C], f16, tag="wqkv")
                nc.sync.dma_start(out=wqkv_t,
                                  in_=wqkv_d[l, :, :].rearrange("(i p) o -> p i o", p=128))
                projw_t = wA.tile([128, NC_CH, C], f16, tag="projw")
                nc.sync.dma_start(out=projw_t,
                                  in_=projw_d[l, :, :].rearrange("(i p) o -> p i o", p=128))
                w1_t = wB.tile([128, NC_CH, FF], f16, tag="w1")
                nc.sync.dma_start(out=w1_t,
                                  in_=w1_d[l, :, :].rearrange("(i p) o -> p i o", p=128))
                w2_t = wB.tile([128, NF_CH, C], f16, tag="w2")
                nc.sync.dma_start(out=w2_t,
                                  in_=w2_d[l, :, :].rearrange("(i p) o -> p i o", p=128))
                if not zero_pb:
                    projb_sb = wA.tile([1, C], f16, tag="projb")
                    nc.sync.dma_start(out=projb_sb, in_=projb_d[:, l * C:(l + 1) * C])
                if not zero_b2:
                    b2_sb = wA.tile([1, C], f16, tag="b2")
                    nc.sync.dma_start(out=b2_sb, in_=b2_d[:, l * C:(l + 1) * C])

                # ---- phase A: attention ----
                with nc.named_scope(f"l{l}_attn"):
                    for g in range(NG if do_attn else 0):
                        nn_fm = ln_to_fm(g, ln1s_sb[:, l, :], ln1b_sb[:, l, :], "nn_fm")
                        # q, k feature-major [128, ch, 512]
                        q_sb = qkp.tile([128, NC_CH, 512], f16, tag="q")
                        k_sb = qkp.tile([128, NC_CH, 512], f16, tag="k")
                        for dst, off in ((q_sb, 0), (k_sb, C)):
                            for m in range(NC_CH):
                                ps = ps_big.tile([128, 512], f32, tag="big")
                                for ci in range(NC_CH):
                                    nc.tensor.matmul(
                                        ps, wqkv_t[:, ci, off + m * 128:off + (m + 1) * 128],
                                        nn_fm[:, ci, :],
                                        start=(ci == 0), stop=(ci == NC_CH - 1))
                                nc.scalar.copy(dst[:, m, :], ps)
                        # v token-major, augmented with ones column per head [128, 4, H, HS+1]
                        v_sb = vp.tile([128, 4, H, HS + 1], f16, tag="v")
                        for j in range(4):
                            ps = ps_big.tile([128, 512], f32, tag="big")
                            for ci in range(NC_CH):
                                nc.tensor.matmul(ps[:, 0:C],
                                                 nn_fm[:, ci, j * 128:(j + 1) * 128],
                                                 wqkv_t[:, ci, 2 * C:3 * C],
                                                 start=(ci == 0), stop=(ci == NC_CH - 1))
                            nc.scalar.copy(
                                v_sb[:, j, :, 0:HS],
                                ps[:, 0:C].rearrange("p (h d) -> p h d", h=H))
                            nc.vector.memset(v_sb[:, j, :, HS:HS + 1], 1.0)
                        # attention per (batch-in-group, head) -> att token-major
                        att_tm = tmg.tile([128, 4, C], f16, tag="att_tm")
                        for bl in range(2):
                            toff = bl * 256
                            for ch in range(NC_CH):
                                # scores for head pair (2ch, 2ch+1) run concurrently in
                                # the PE array via row tiling (K=64 each at rows 0/64)
                                scp = [ps_big.tile([128, 512], f32, tag="big",
                                                   name=f"sc{hh2}")
                                       for hh2 in range(2)]
                                for hh, sc in enumerate(scp):
                                    o = hh * HS
                                    nc.tensor.matmul(sc[:, 0:256],
                                                     k_sb[o:o + HS, ch, toff:toff + 128],
                                                     q_sb[o:o + HS, ch, toff:toff + 256],
                                                     start=True, stop=True,
                                                     tile_position=(o, 0))
                                for hh, sc in enumerate(scp):
                                    o = hh * HS
                                    nc.tensor.matmul(sc[:, 256:384],
                                                     k_sb[o:o + HS, ch, toff + 128:toff + 256],
                                                     q_sb[o:o + HS, ch, toff + 128:toff + 256],
                                                     start=True, stop=True,
                                                     tile_position=(o, 0))
                                for hh, sc in enumerate(scp):
                                    h = 2 * ch + hh
                                    ex = exl.tile([128, 384], f16, tag="ex")
                                    nc.scalar.activation(out=ex, in_=sc[:, 0:384], func=AF.Exp,
                                                         scale=SCALE)
                                    # mask diagonal blocks (cols 0:128 / 256:384) in one
                                    # op: view [128, 2, 128] block step 256, tri
                                    # broadcast along block dim (step 0)
                                    exv = ex.rearrange("p (a c) -> p a c", a=3)
                                    exd = bass.AP(exv.tensor, exv.offset,
                                                  [exv.ap[0], [2 * exv.ap[1][0], 2],
                                                   exv.ap[2]])
                                    trib = bass.AP(tri.tensor, tri.offset,
                                                   [tri.ap[0], [0, 2], tri.ap[1]])
                                    nc.vector.tensor_mul(exd, exd, trib)
                                    av = ps_av.tile([128, 2, V], f32, tag="av")
                                    nc.tensor.matmul(av[:, 0, :], ex[:, 0:128],
                                                     v_sb[:, bl * 2, h, :],
                                                     start=True, stop=True)
                                    nc.tensor.matmul(av[:, 1, :], ex[:, 128:256],
                                                     v_sb[:, bl * 2, h, :],
                                                     start=True, stop=False)
                                    nc.tensor.matmul(av[:, 1, :], ex[:, 256:384],
                                                     v_sb[:, bl * 2 + 1, h, :],
                                                     start=False, stop=True)
                                    r2 = stat.tile([128, 2], f32, tag="r2")
                                    nc.vector.reciprocal_approx_fast(
                                        out=r2, in_=av[:, :, HS:HS + 1])
                                    for tt in range(2):
                                        nc.vector.tensor_scalar_mul(
                                            att_tm[:, bl * 2 + tt, h * HS:(h + 1) * HS],
                                            av[:, tt, 0:HS], r2[:, tt:tt + 1])
                        # att -> feature-major
                        att_fm = fmg.tile([128, NC_CH, 512], f16, tag="att_fm")
                        for ci in range(NC_CH):
                            ps = ps_tr.tile([128, 512], f32, tag="tr")
                            for j in range(4):
                                nc.tensor.matmul(ps[:, j * 128:(j + 1) * 128],
                                                 att_tm[:, j, ci * 128:(ci + 1) * 128], ident,
                                                 start=True, stop=True)
                            nc.scalar.copy(att_fm[:, ci, :], ps)
                        # proj + residual
                        for j in range(4):
                            i = g * 4 + j
                            ps = ps_big.tile([128, 512], f32, tag="big")
                            for ci in range(NC_CH):
                                nc.tensor.matmul(ps[:, 0:C], att_fm[:, ci, j * 128:(j + 1) * 128],
                                                 projw_t[:, ci, :], start=(ci == 0),
                                                 stop=(zero_pb and ci == NC_CH - 1))
                            if not zero_pb:
                                nc.tensor.matmul(ps[:, 0:C], ones_r, projb_sb,
                                                 start=False, stop=True)
                            nc.vector.tensor_add(x_tm[:, i, :], x_tm[:, i, :], ps[:, 0:C])

                # ---- phase B: MLP ----
                with nc.named_scope(f"l{l}_mlp"):
                    for g in range(NG if do_mlp else 0):
                        nn_fm = ln_to_fm(g, ln2s_sb[:, l, :], ln2b_sb[:, l, :], "nn_fm")
                        h_sb = hp.tile([128, NF_CH, 512], f16, tag="h")
                        for f in range(NF_CH):
                            ps = ps_big.tile([128, 512], f32, tag="big")
                            for ci in range(NC_CH):
                                nc.tensor.matmul(ps, w1_t[:, ci, f * 128:(f + 1) * 128],
                                                 nn_fm[:, ci, :],
                                                 start=(ci == 0), stop=(ci == NC_CH - 1))
                            nc.vector.tensor_scalar(out=h_sb[:, f, :], in0=ps,
                                                    scalar1=b1_sb[:, l, f:f + 1],
                                                    scalar2=0.0, op0=ALU.add,
                                                    op1=ALU.max)
                        for j in range(4):
                            i = g * 4 + j
                            ps = ps_big.tile([128, 512], f32, tag="big")
                            for fi in range(NF_CH):
                                nc.tensor.matmul(ps[:, 0:C], h_sb[:, fi, j * 128:(j + 1) * 128],
                                                 w2_t[:, fi, :], start=(fi == 0),
                                                 stop=(zero_b2 and fi == NF_CH - 1))
                            if not zero_b2:
                                nc.tensor.matmul(ps[:, 0:C], ones_r, b2_sb,
                                                 start=False, stop=True)
                            nc.vector.tensor_add(x_tm[:, i, :], x_tm[:, i, :], ps[:, 0:C])

            # ================= final LN + head + loss =================
            with nc.named_scope("head"):
                for g in range(NG):
                    xf_fm = ln_to_fm(g, lnfs_sb, lnfb_sb, "nn_fm")
                    for j in range(4):
                        i = g * 4 + j
                        psl = ps_av.tile([128, 2, V], f32, tag="av")
                        for ci in range(NC_CH):
                            nc.tensor.matmul(psl[:, 0, :], xf_fm[:, ci, j * 128:(j + 1) * 128],
                                             headw_sb[:, ci, :],
                                             start=(ci == 0), stop=False)
                        nc.tensor.matmul(psl[:, 0, :], ones_r, headb_sb,
                                         start=False, stop=True)
                        lg = outp.tile([128, V], f32, tag="lg")
                        nc.scalar.activation(out=lg, in_=psl[:, 0, :], func=AF.Copy)
                        nc.sync.dma_start(out=logits_d[i * 128:(i + 1) * 128, :], in_=lg)
                        # loss pieces: lse - logits[target]
                        sel = outp.tile([128, V], f32, tag="sel")
                        nc.vector.tensor_scalar(out=sel, in0=viota_r,
                                                scalar1=tgt_sb[:, i:i + 1], scalar2=None,
                                                op0=ALU.is_equal)
                        scr = outp.tile([128, V], f32, tag="scr")
                        pick = stat.tile([128, 1], f32, tag="pick")
                        nc.vector.tensor_mul(scr, lg, sel)
                        nc.vector.reduce_sum(pick, scr, axis=mybir.AxisListType.X)
                        ex2 = outp.tile([128, V], f32, tag="ex2")
                        se = stat.tile([128, 1], f32, tag="se")
                        nc.scalar.activation(out=ex2, in_=lg, func=AF.Exp, accum_out=se)
                        lse = stat.tile([128, 1], f32, tag="lse")
                        nc.scalar.activation(out=lse, in_=se, func=AF.Ln)
                        nc.vector.tensor_sub(loss_cols[:, i:i + 1], lse, pick)
                pst = ps_big.tile([1, NT], f32, tag="big")
                nc.tensor.matmul(pst, ones_c32, loss_cols, start=True, stop=True)
                tot = stat.tile([1, 1], f32, tag="tot")
                nc.vector.reduce_sum(tot, pst, axis=mybir.AxisListType.X)
                nc.sync.dma_start(out=loss_d[:, :], in_=tot)

    nc.finalize()
    return nc


def _get_program(zero_pb=False, zero_b2=False):
    key = ("nc", zero_pb, zero_b2)
    if key not in _CACHE:
        _CACHE[key] = _build_program(zero_pb=zero_pb, zero_b2=zero_b2)
    return _CACHE[key]


def _prep_weights(inputs):
    f16 = np.float16

    def hcd(w):  # [L,H,C,HS] -> [L, C, H*HS]
        return np.ascontiguousarray(w.transpose(0, 2, 1, 3)).reshape(L, C, C)

    wqkv = np.concatenate(
        [hcd(np.asarray(inputs["wq"])), hcd(np.asarray(inputs["wk"])),
         hcd(np.asarray(inputs["wv"]))], axis=2).astype(f16)
    shared = {
        "tok_emb": np.asarray(inputs["tok_emb"]).astype(f16),
        "pos_emb": np.asarray(inputs["pos_emb"]).astype(f16),
        "wqkv": np.ascontiguousarray(wqkv),
        "proj_w": np.asarray(inputs["proj_w"]).astype(f16),
        "proj_b": np.asarray(inputs["proj_b"]).astype(f16).reshape(1, L * C),
        "w1": np.asarray(inputs["w1"]).astype(f16),
        "b1": np.asarray(inputs["b1"]).astype(np.float32),
        "w2": np.asarray(inputs["w2"]).astype(f16),
        "b2": np.asarray(inputs["b2"]).astype(f16).reshape(1, L * C),
        "ln1_s": np.asarray(inputs["ln1_s"]).astype(np.float32),
        "ln1_b": np.asarray(inputs["ln1_b"]).astype(np.float32),
        "ln2_s": np.asarray(inputs["ln2_s"]).astype(np.float32),
        "ln2_b": np.asarray(inputs["ln2_b"]).astype(np.float32),
        "lnf_s": np.asarray(inputs["lnf_s"]).astype(np.float32),
        "lnf_b": np.asarray(inputs["lnf_b"]).astype(np.float32),
        "head_w": np.asarray(inputs["head_w"]).astype(f16),
        "head_b": np.asarray(inputs["head_b"]).astype(f16).reshape(1, V),
    }
    return shared


def _prepare_run(inputs):
    zero_pb = not np.any(np.asarray(inputs["proj_b"]))
    zero_b2 = not np.any(np.asarray(inputs["b2"]))
    nc = _get_program(zero_pb=zero_pb, zero_b2=zero_b2)
    shared = _prep_weights(inputs)
    if zero_pb:
        shared.pop("proj_b")
    if zero_b2:
        shared.pop("b2")
    idx = np.ascontiguousarray(np.asarray(inputs["idx"], dtype=np.int32))
    tgt = np.ascontiguousarray(np.asarray(inputs["targets"], dtype=np.int32))
    in_maps = []
    for c in range(NCORES):
        m = dict(shared)
        m["idx"] = np.ascontiguousarray(idx[c * BSH:(c + 1) * BSH])
        m["targets"] = np.ascontiguousarray(tgt[c * BSH:(c + 1) * BSH])
        in_maps.append(m)
    return nc, in_maps


def kernel(**inputs):
    from concourse.bass_utils import run_bass_kernel_spmd

    nc, in_maps = _prepare_run(inputs)
    res = run_bass_kernel_spmd(nc, in_maps, core_ids=list(range(NCORES)), trace=False)
    logits = np.concatenate([r["logits"] for r in res.results], axis=0)
    loss = sum(float(r["loss_sum"][0, 0]) for r in res.results) / (B * T)
    return logits.astype(np.float32), np.array(loss, dtype=np.float32)


# revision 36
# speedup vs baseline: 1.3837x; 1.0018x over previous
"""Bass/Trainium2 kernel for nn_BigramLanguageModel (6-layer GPT, B=128 T=256 C=384 H=6 V=65).

Strategy: pure data-parallel over batch across 8 NeuronCores (16 batch rows each),
weights replicated. Per core, a fully fused transformer forward:
  - residual stream token-major fp32 in SBUF ([128 tok, 384] tiles)
  - matmul operands fp16 (1 cy/row on PE), fp32 PSUM accumulation
  - LayerNorm stats via bn_stats/bn_aggr (token-major), scale/bias folded into the
    PSUM->SBUF copy after the PE transpose to feature-major
  - attention scores computed transposed [s, t]; softmax denominator via a ones
    column appended to V (row 64 of the AV matmul output); causal mask via
    gpsimd.affine_select; no max-subtraction (scores are tiny; softmax invariant)
  - embedding gather as one-hot matmul (one-hot via DMA-broadcast + is_equal)
  - loss (mean NLL) computed on device per core; host sums 8 partials
"""

import numpy as np

V, B, T, C, H, L = 65, 128, 256, 384, 6, 6
HS, FF = C // H, 4 * C
EPS = 1e-5
NCORES = 8
BSH = B // NCORES          # batch rows per core = 16
NTOK = BSH * T             # tokens per core = 4096
NT = NTOK // 128           # token tiles per core = 32
NG = NT // 4               # groups (512 tokens = 2 batch rows) = 8
NC_CH = C // 128           # 3 feature chunks
NF_CH = FF // 128          # 12 ff chunks
SCALE = C ** (-0.5)

_CACHE = {}


def _build_program(nlayers=L, do_attn=True, do_mlp=True, zero_pb=False, zero_b2=False):
    import concourse.bacc as bacc
    import concourse.bass as bass
    import concourse.mybir as mybir
    import concourse.tile as tile
    from concourse.masks import make_identity

    f16 = mybir.dt.float16
    f32 = mybir.dt.float32
    i32 = mybir.dt.int32
    AF = mybir.ActivationFunctionType
    ALU = mybir.AluOpType

    nc = bacc.Bacc(target_bir_lowering=False)

    # ---- DRAM I/O ----
    idx_d = nc.dram_tensor("idx", [BSH, T], i32, kind="ExternalInput")
    tgt_d = nc.dram_tensor("targets", [BSH, T], i32, kind="ExternalInput")
    tok_d = nc.dram_tensor("tok_emb", [V, C], f16, kind="ExternalInput")
    pos_d = nc.dram_tensor("pos_emb", [T, C], f16, kind="ExternalInput")
    wqkv_d = nc.dram_tensor("wqkv", [L, C, 3 * C], f16, kind="ExternalInput")
    projw_d = nc.dram_tensor("proj_w", [L, C, C], f16, kind="ExternalInput")
    projb_d = None if zero_pb else nc.dram_tensor("proj_b", [1, L * C], f16,
                                                  kind="ExternalInput")
    w1_d = nc.dram_tensor("w1", [L, C, FF], f16, kind="ExternalInput")
    b1_d = nc.dram_tensor("b1", [L, FF], f32, kind="ExternalInput")
    w2_d = nc.dram_tensor("w2", [L, FF, C], f16, kind="ExternalInput")
    b2_d = None if zero_b2 else nc.dram_tensor("b2", [1, L * C], f16,
                                               kind="ExternalInput")
    ln1s_d = nc.dram_tensor("ln1_s", [L, C], f32, kind="ExternalInput")
    ln1b_d = nc.dram_tensor("ln1_b", [L, C], f32, kind="ExternalInput")
    ln2s_d = nc.dram_tensor("ln2_s", [L, C], f32, kind="ExternalInput")
    ln2b_d = nc.dram_tensor("ln2_b", [L, C], f32, kind="ExternalInput")
    lnfs_d = nc.dram_tensor("lnf_s", [C], f32, kind="ExternalInput")
    lnfb_d = nc.dram_tensor("lnf_b", [C], f32, kind="ExternalInput")
    headw_d = nc.dram_tensor("head_w", [C, V], f16, kind="ExternalInput")
    headb_d = nc.dram_tensor("head_b", [1, V], f16, kind="ExternalInput")
    logits_d = nc.dram_tensor("logits", [NTOK, V], f32, kind="ExternalOutput")
    loss_d = nc.dram_tensor("loss_sum", [1, 1], f32, kind="ExternalOutput")

    with tile.TileContext(nc) as tc:
        import contextlib
        ctx = contextlib.ExitStack()
        with ctx:
            const = ctx.enter_context(tc.tile_pool(name="const", bufs=1))
            wA = ctx.enter_context(tc.tile_pool(name="wA", bufs=2))
            wB = ctx.enter_context(tc.tile_pool(name="wB", bufs=2))
            resid = ctx.enter_context(tc.tile_pool(name="resid", bufs=1))
            tmg = ctx.enter_context(tc.tile_pool(name="tmg", bufs=2))      # token-major group tiles
            fmg = ctx.enter_context(tc.tile_pool(name="fmg", bufs=2))      # feature-major group tiles
            qkp = ctx.enter_context(tc.tile_pool(name="qkp", bufs=2))
            vp = ctx.enter_context(tc.tile_pool(name="vp", bufs=2))
            exl = ctx.enter_context(tc.tile_pool(name="exl", bufs=3))
            hp = ctx.enter_context(tc.tile_pool(name="hp", bufs=2))
            stat = ctx.enter_context(tc.tile_pool(name="stat", bufs=4))
            small = ctx.enter_context(tc.tile_pool(name="small", bufs=2))
            outp = ctx.enter_context(tc.tile_pool(name="outp", bufs=4))
            ps_big = ctx.enter_context(tc.tile_pool(name="ps_big", bufs=4, space="PSUM"))
            ps_tr = ctx.enter_context(tc.tile_pool(name="ps_tr", bufs=2, space="PSUM"))
            ps_av = ctx.enter_context(tc.tile_pool(name="ps_av", bufs=2, space="PSUM"))

            # pin ScalarE's activation table to natural_log_exp_and_others (covers
            # Copy/Identity/Exp/Ln/Relu) so the table-load pass inserts no swaps
            nc.scalar.add_instruction(mybir.InstLoadActFuncSet(
                name=nc.get_next_instruction_name(), act_func_set_id=6, ins=[], outs=[]))

            # ---- constants / params resident in SBUF ----
            ident = const.tile([128, 128], f16)
            make_identity(nc, ident)
            ones_r = const.tile([1, 128], f16)       # K=1 lhsT for bias matmuls
            nc.vector.memset(ones_r, 1.0)
            ones_c32 = const.tile([128, 1], f32)     # K=128 lhsT for loss column-sum
            nc.vector.memset(ones_c32, 1.0)
            eps_t = const.tile([128, 1], f32)
            nc.vector.memset(eps_t, EPS)
            viota_p = const.tile([V, 512], i32)      # value = partition idx
            nc.gpsimd.iota(viota_p, pattern=[[0, 512]], base=0, channel_multiplier=1)
            viota_r = const.tile([128, V], f32)      # value = free idx (exact in f32)
            nc.gpsimd.iota(viota_r, pattern=[[1, V]], base=0, channel_multiplier=0,
                           allow_small_or_imprecise_dtypes=True)
            # causal triangle mask: tri[s, t] = 1.0 if t >= s else 0.0
            tri = const.tile([128, 128], f16)
            nc.vector.memset(tri, 1.0)
            nc.gpsimd.affine_select(out=tri, in_=tri, compare_op=ALU.is_ge, fill=0.0,
                                    base=0, pattern=[[1, 128]], channel_multiplier=-1)

            tok_sb = const.tile([V, C], f16)
            nc.sync.dma_start(out=tok_sb, in_=tok_d[:, :])
            pos_sb = const.tile([128, 2, C], f16)
            nc.sync.dma_start(out=pos_sb, in_=pos_d.rearrange("(i p) c -> p i c", p=128))
            ln1s_sb = const.tile([128, L, NC_CH], f32)
            nc.sync.dma_start(out=ln1s_sb, in_=ln1s_d.rearrange("l (i p) -> p l i", p=128))
            ln1b_sb = const.tile([128, L, NC_CH], f32)
            nc.sync.dma_start(out=ln1b_sb, in_=ln1b_d.rearrange("l (i p) -> p l i", p=128))
            ln2s_sb = const.tile([128, L, NC_CH], f32)
            nc.sync.dma_start(out=ln2s_sb, in_=ln2s_d.rearrange("l (i p) -> p l i", p=128))
            ln2b_sb = const.tile([128, L, NC_CH], f32)
            nc.sync.dma_start(out=ln2b_sb, in_=ln2b_d.rearrange("l (i p) -> p l i", p=128))
            lnfs_sb = const.tile([128, NC_CH], f32)
            nc.sync.dma_start(out=lnfs_sb, in_=lnfs_d.rearrange("(i p) -> p i", p=128))
            lnfb_sb = const.tile([128, NC_CH], f32)
            nc.sync.dma_start(out=lnfb_sb, in_=lnfb_d.rearrange("(i p) -> p i", p=128))
            b1_sb = const.tile([128, L, NF_CH], f32)
            nc.sync.dma_start(out=b1_sb, in_=b1_d.rearrange("l (i p) -> p l i", p=128))
            headw_sb = const.tile([128, NC_CH, V], f16)
            nc.sync.dma_start(out=headw_sb, in_=headw_d.rearrange("(i p) v -> p i v", p=128))
            headb_sb = const.tile([1, V], f16)
            nc.sync.dma_start(out=headb_sb, in_=headb_d[:, :])
            tgt_i = const.tile([128, NT], i32)
            nc.sync.dma_start(out=tgt_i, in_=tgt_d.rearrange("b (n p) -> p (b n)", p=128))
            tgt_sb = const.tile([128, NT], f32)
            nc.vector.tensor_copy(tgt_sb, tgt_i)
            loss_cols = const.tile([128, NT], f32)

            x_tm = resid.tile([128, NT, C], f32)     # persistent residual stream

            # ================= embedding =================
            with nc.named_scope("embed"):
                for g in range(NG):
                    idxb = small.tile([V, 512], i32, tag="idxb")
                    nc.sync.dma_start(
                        out=idxb,
                        in_=bass.AP(idx_d, g * 512, [[0, V], [1, 512]]),
                    )
                    onehot = small.tile([V, 512], f16, tag="onehot")
                    nc.vector.tensor_tensor(out=onehot, in0=idxb, in1=viota_p, op=ALU.is_equal)
                    for j in range(4):
                        i = g * 4 + j
                        ps = ps_big.tile([128, 512], f32, tag="big")
                        nc.tensor.matmul(ps[:, 0:C], onehot[:, j * 128:(j + 1) * 128], tok_sb,
                                         start=True, stop=True)
                        nc.vector.tensor_add(x_tm[:, i, :], ps[:, 0:C], pos_sb[:, i % 2, :])

            # helper: LayerNorm (token-major stats) + transpose to feature-major with
            # scale/bias folded into the PSUM->SBUF copy. Returns [128, NC_CH, 512] f16.
            # rstd = exp(-0.5*ln(var+eps)) keeps ScalarE on one activation table
            # (natural_log_exp set) - Sqrt would force a table swap per group.
            def ln_to_fm(g, s_ap, b_ap, tag):
                mv = stat.tile([128, 4, 2], f32, tag="mv")
                for j in range(4):
                    i = g * 4 + j
                    st6 = stat.tile([128, 6], f32, tag="st6")
                    nc.vector.bn_stats(out=st6, in_=x_tm[:, i, :])
                    nc.vector.bn_aggr(out=mv[:, j, :], in_=st6)
                lnv = stat.tile([128, 4], f32, tag="lnv")
                nc.scalar.activation(out=lnv, in_=mv[:, :, 1], func=AF.Ln,
                                     bias=eps_t, scale=1.0)
                rstd = stat.tile([128, 4], f32, tag="rstd")
                nc.scalar.activation(out=rstd, in_=lnv, func=AF.Exp, scale=-0.5)
                nn_tm = tmg.tile([128, 4, C], f16, tag="nn_tm")
                for j in range(4):
                    i = g * 4 + j
                    nc.vector.tensor_scalar(out=nn_tm[:, j, :], in0=x_tm[:, i, :],
                                            scalar1=mv[:, j, 0:1], scalar2=rstd[:, j:j + 1],
                                            op0=ALU.subtract, op1=ALU.mult)
                nn_fm = fmg.tile([128, NC_CH, 512], f16, tag=tag)
                for ci in range(NC_CH):
                    ps = ps_tr.tile([128, 512], f32, tag="tr")
                    for j in range(4):
                        nc.tensor.matmul(ps[:, j * 128:(j + 1) * 128],
                                         nn_tm[:, j, ci * 128:(ci + 1) * 128], ident,
                                         start=True, stop=True)
                    nc.scalar.activation(out=nn_fm[:, ci, :], in_=ps, func=AF.Identity,
                                         bias=b_ap[:, ci:ci + 1], scale=s_ap[:, ci:ci + 1])
                return nn_fm

            # ================= layers =================
            for l in range(nlayers):
                wqkv_t = wA.tile([128, NC_CH, 3 * C], f16, tag="wqkv")
                nc.sync.dma_start(out=wqkv_t,
                                  in_=wqkv_d[l, :, :].rearrange("(i p) o -> p i o", p=128))
                projw_t = wA.tile([128, NC_CH, C], f16, tag="projw")
                nc.sync.dma_start(out=projw_t,
                                  in_=projw_d[l, :, :].rearrange("(i p) o -> p i o", p=128))
                w1_t = wB.tile([128, NC_CH, FF], f16, tag="w1")
                nc.sync.dma_start(out=w1_t,
                                  in_=w1_d[l, :, :].rearrange("(i p) o -> p i o", p=128))
                w2_t = wB.tile([128, NF_CH, C], f16, tag="w2")
                nc.sync.dma_start(out=w2_t,
                                  in_=w2_d[l, :, :].rearrange("(i p) o -> p i o", p=128))
                if not zero_pb:
                    projb_sb = wA.tile([1, C], f16, tag="projb")
                    nc.sync.dma_start(out=projb_sb, in_=projb_d[:, l * C:(l + 1) * C])
                if not zero_b2:
                    b2_sb = wA.tile([1, C], f16, tag="b2")
                    nc.sync.dma_start(out=b2_sb, in_=b2_d[:, l * C:(l + 1) * C])

                # ---- phase A: attention ----
                with nc.named_scope(f"l{l}_attn"):
                    for g in range(NG if do_attn else 0):
                        nn_fm = ln_to_fm(g, ln1s_sb[:, l, :], ln1b_sb[:, l, :], "nn_fm")
                        # q, k feature-major [128, ch, 512]
                        q_sb = qkp.tile([128, NC_CH, 512], f16, tag="q")
                        k_sb = qkp.tile([128, NC_CH, 512], f16, tag="k")
                        for dst, off in ((q_sb, 0), (k_sb, C)):
                            for m in range(NC_CH):
                                ps = ps_big.tile([128, 512], f32, tag="big")
                                for ci in range(NC_CH):
                                    nc.tensor.matmul(
                                        ps, wqkv_t[:, ci, off + m * 128:off + (m + 1) * 128],
                                        nn_fm[:, ci, :],
                                        start=(ci == 0), stop=(ci == NC_CH - 1))
                                nc.scalar.copy(dst[:, m, :], ps)
                        # v token-major, augmented with ones column per head [128, 4, H, HS+1]
                        v_sb = vp.tile([128, 4, H, HS + 1], f16, tag="v")
                        for j in range(4):
                            ps = ps_big.tile([128, 512], f32, tag="big")
                            for ci in range(NC_CH):
                                nc.tensor.matmul(ps[:, 0:C],
                                                 nn_fm[:, ci, j * 128:(j + 1) * 128],
                                                 wqkv_t[:, ci, 2 * C:3 * C],
                                                 start=(ci == 0), stop=(ci == NC_CH - 1))
                            nc.scalar.copy(
                                v_sb[:, j, :, 0:HS],
                                ps[:, 0:C].rearrange("p (h d) -> p h d", h=H))
                            nc.vector.memset(v_sb[:, j, :, HS:HS + 1], 1.0)
                        # attention per (batch-in-group, head) -> att token-major
                        att_tm = tmg.tile([128, 4, C], f16, tag="att_tm")
                        for bl in range(2):
                            toff = bl * 256
                            for ch in range(NC_CH):
                                # scores for head pair (2ch, 2ch+1) run concurrently in
                                # the PE array via row tiling (K=64 each at rows 0/64)
                                scp = [ps_big.tile([128, 512], f32, tag="big",
                                                   name=f"sc{hh2}")
                                       for hh2 in range(2)]
                                for hh, sc in enumerate(scp):
                                    o = hh * HS
                                    nc.tensor.matmul(sc[:, 0:256],
                                                     k_sb[o:o + HS, ch, toff:toff + 128],
                                                     q_sb[o:o + HS, ch, toff:toff + 256],
                                                     start=True, stop=True,
                                                     tile_position=(o, 0))
                                for hh, sc in enumerate(scp):
                                    o = hh * HS
                                    nc.tensor.matmul(sc[:, 256:384],
                                                     k_sb[o:o + HS, ch, toff + 128:toff + 256],
                                                     q_sb[o:o + HS, ch, toff + 128:toff + 256],
                                                     start=True, stop=True,
                                                     tile_position=(o, 0))
                                for hh, sc in enumerate(scp):
                                    h = 2 * ch + hh
                                    ex = exl.tile([128, 384], f16, tag="ex")
                                    nc.scalar.activation(out=ex, in_=sc[:, 0:384], func=AF.Exp,
                                                         scale=SCALE)
                                    # mask diagonal blocks (cols 0:128 / 256:384) in one
                                    # op: view [128, 2, 128] block step 256, tri
                                    # broadcast along block dim (step 0)
                                    exv = ex.rearrange("p (a c) -> p a c", a=3)
                                    exd = bass.AP(exv.tensor, exv.offset,
                                                  [exv.ap[0], [2 * exv.ap[1][0], 2],
                                                   exv.ap[2]])
                                    trib = bass.AP(tri.tensor, tri.offset,
                                                   [tri.ap[0], [0, 2], tri.ap[1]])
                                    nc.vector.tensor_mul(exd, exd, trib)
                                    av = ps_av.tile([128, 2, V], f32, tag="av")
                                    nc.tensor.matmul(av[:, 0, :], ex[:, 0:128],
                                                     v_sb[:, bl * 2, h, :],
                                                     start=True, stop=True)
                                    nc.tensor.matmul(av[:, 1, :], ex[:, 128:256],
                                                     v_sb[:, bl * 2, h, :],
                                                     start=True, stop=False)
                                    nc.tensor.matmul(av[:, 1, :], ex[:, 256:384],
                                                     v_sb[:, bl * 2 + 1, h, :],
                                                     start=False, stop=True)
                                    r2 = stat.tile([128, 2], f32, tag="r2")
                                    nc.vector.reciprocal_approx_fast(
                                        out=r2, in_=av[:, :, HS:HS + 1])
                                    for tt in range(2):
                                        nc.vector.tensor_scalar_mul(
                                            att_tm[:, bl * 2 + tt, h * HS:(h + 1) * HS],
                                            av[:, tt, 0:HS], r2[:, tt:tt + 1])
                        # att -> feature-major
                        att_fm = fmg.tile([128, NC_CH, 512], f16, tag="att_fm")
                        for ci in range(NC_CH):
                            ps = ps_tr.tile([128, 512], f32, tag="tr")
                            for j in range(4):
                                nc.tensor.matmul(ps[:, j * 128:(j + 1) * 128],
                                                 att_tm[:, j, ci * 128:(ci + 1) * 128], ident,
                                                 start=True, stop=True)
                            nc.scalar.copy(att_fm[:, ci, :], ps)
                        # proj + residual
                        for j in range(4):
                            i = g * 4 + j
                            ps = ps_big.tile([128, 512], f32, tag="big")
                            for ci in range(NC_CH):
                                nc.tensor.matmul(ps[:, 0:C], att_fm[:, ci, j * 128:(j + 1) * 128],
                                                 projw_t[:, ci, :], start=(ci == 0),
                                                 stop=(zero_pb and ci == NC_CH - 1))
                            if not zero_pb:
                                nc.tensor.matmul(ps[:, 0:C], ones_r, projb_sb,
                                                 start=False, stop=True)
                            nc.vector.tensor_add(x_tm[:, i, :], x_tm[:, i, :], ps[:, 0:C])

                # ---- phase B: MLP ----
                with nc.named_scope(f"l{l}_mlp"):
                    for g in range(NG if do_mlp else 0):
                        nn_fm = ln_to_fm(g, ln2s_sb[:, l, :], ln2b_sb[:, l, :], "nn_fm")
                        h_sb = hp.tile([128, NF_CH, 512], f16, tag="h")
                        for f in range(NF_CH):
                            ps = ps_big.tile([128, 512], f32, tag="big")
                            for ci in range(NC_CH):
                                nc.tensor.matmul(ps, w1_t[:, ci, f * 128:(f + 1) * 128],
                                                 nn_fm[:, ci, :],
                                                 start=(ci == 0), stop=(ci == NC_CH - 1))
                            if f % 2 == 0:
                                nc.vector.tensor_scalar(out=h_sb[:, f, :], in0=ps,
                                                        scalar1=b1_sb[:, l, f:f + 1],
                                                        scalar2=0.0, op0=ALU.add,
                                                        op1=ALU.max)
                            else:
                                nc.scalar.activation(out=h_sb[:, f, :], in_=ps,
                                                     func=AF.Relu,
                                                     bias=b1_sb[:, l, f:f + 1],
                                                     scale=1.0)
                        for j in range(4):
                            i = g * 4 + j
                            ps = ps_big.tile([128, 512], f32, tag="big")
                            for fi in range(NF_CH):
                                nc.tensor.matmul(ps[:, 0:C], h_sb[:, fi, j * 128:(j + 1) * 128],
                                                 w2_t[:, fi, :], start=(fi == 0),
                                                 stop=(zero_b2 and fi == NF_CH - 1))
                            if not zero_b2:
                                nc.tensor.matmul(ps[:, 0:C], ones_r, b2_sb,
                                                 start=False, stop=True)
                            nc.vector.tensor_add(x_tm[:, i, :], x_tm[:, i, :], ps[:, 0:C])

            # ================= final LN + head + loss =================
            with nc.named_scope("head"):
                for g in range(NG):
                    xf_fm = ln_to_fm(g, lnfs_sb, lnfb_sb, "nn_fm")
                    for j in range(4):
                        i = g * 4 + j
                        psl = ps_av.tile([128, 2, V], f32, tag="av")
                        for ci in range(NC_CH):
                            nc.tensor.matmul(psl[:, 0, :], xf_fm[:, ci, j * 128:(j + 1) * 128],
                                             headw_sb[:, ci, :],
                                             start=(ci == 0), stop=False)
                        nc.tensor.matmul(psl[:, 0, :], ones_r, headb_sb,
                                         start=False, stop=True)
                        lg = outp.tile([128, V], f32, tag="lg")
                        nc.scalar.activation(out=lg, in_=psl[:, 0, :], func=AF.Copy)
                        nc.sync.dma_start(out=logits_d[i * 128:(i + 1) * 128, :], in_=lg)
                        # loss pieces: lse - logits[target]
                        sel = outp.tile([128, V], f32, tag="sel")
                        nc.vector.tensor_scalar(out=sel, in0=viota_r,
                                                scalar1=tgt_sb[:, i:i + 1], scalar2=None,
                                                op0=ALU.is_equal)
                        scr = outp.tile([128, V], f32, tag="scr")
                        pick = stat.tile([128, 1], f32, tag="pick")
                        nc.vector.tensor_mul(scr, lg, sel)
                        nc.vector.reduce_sum(pick, scr, axis=mybir.AxisListType.X)
                        ex2 = outp.tile([128, V], f32, tag="ex2")
                        se = stat.tile([128, 1], f32, tag="se")
                        nc.scalar.activation(out=ex2, in_=lg, func=AF.Exp, accum_out=se)
                        lse = stat.tile([128, 1], f32, tag="lse")
                        nc.scalar.activation(out=lse, in_=se, func=AF.Ln)
                        nc.vector.tensor_sub(loss_cols[:, i:i + 1], lse, pick)
                pst = ps_big.tile([1, NT], f32, tag="big")
                nc.tensor.matmul(pst, ones_c32, loss_cols, start=True, stop=True)
                tot = stat.tile([1, 1], f32, tag="tot")
                nc.vector.reduce_sum(tot, pst, axis=mybir.AxisListType.X)
                nc.sync.dma_start(out=loss_d[:, :], in_=tot)

    nc.finalize()
    return nc


def _get_program(zero_pb=False, zero_b2=False):
    key = ("nc", zero_pb, zero_b2)
    if key not in _CACHE:
        _CACHE[key] = _build_program(zero_pb=zero_pb, zero_b2=zero_b2)
    return _CACHE[key]


def _prep_weights(inputs):
    f16 = np.float16

    def hcd(w):  # [L,H,C,HS] -> [L, C, H*HS]
        return np.ascontiguousarray(w.transpose(0, 2, 1, 3)).reshape(L, C, C)

    wqkv = np.concatenate(
        [hcd(np.asarray(inputs["wq"])), hcd(np.asarray(inputs["wk"])),
         hcd(np.asarray(inputs["wv"]))], axis=2).astype(f16)
    shared = {
        "tok_emb": np.asarray(inputs["tok_emb"]).astype(f16),
        "pos_emb": np.asarray(inputs["pos_emb"]).astype(f16),
        "wqkv": np.ascontiguousarray(wqkv),
        "proj_w": np.asarray(inputs["proj_w"]).astype(f16),
        "proj_b": np.asarray(inputs["proj_b"]).astype(f16).reshape(1, L * C),
        "w1": np.asarray(inputs["w1"]).astype(f16),
        "b1": np.asarray(inputs["b1"]).astype(np.float32),
        "w2": np.asarray(inputs["w2"]).astype(f16),
        "b2": np.asarray(inputs["b2"]).astype(f16).reshape(1, L * C),
        "ln1_s": np.asarray(inputs["ln1_s"]).astype(np.float32),
        "ln1_b": np.asarray(inputs["ln1_b"]).astype(np.float32),
        "ln2_s": np.asarray(inputs["ln2_s"]).astype(np.float32),
        "ln2_b": np.asarray(inputs["ln2_b"]).astype(np.float32),
        "lnf_s": np.asarray(inputs["lnf_s"]).astype(np.float32),
        "lnf_b": np.asarray(inputs["lnf_b"]).astype(np.float32),
        "head_w": np.asarray(inputs["head_w"]).astype(f16),
        "head_b": np.asarray(inputs["head_b"]).astype(f16).reshape(1, V),
    }
    return shared


def _prepare_run(inputs):
    zero_pb = not np.any(np.asarray(inputs["proj_b"]))
    zero_b2 = not np.any(np.asarray(inputs["b2"]))
    nc = _get_program(zero_pb=zero_pb, zero_b2=zero_b2)
    shared = _prep_weights(inputs)
    if zero_pb:
        shared.pop("proj_b")
    if zero_b2:
        shared.pop("b2")
    idx = np.ascontiguousarray(np.asarray(inputs["idx"], dtype=np.int32))
    tgt = np.ascontiguousarray(np.asarray(inputs["targets"], dtype=np.int32))
    in_maps = []
    for c in range(NCORES):
        m = dict(shared)
        m["idx"] = np.ascontiguousarray(idx[c * BSH:(c + 1) * BSH])
        m["targets"] = np.ascontiguousarray(tgt[c * BSH:(c + 1) * BSH])
        in_maps.append(m)
    return nc, in_maps


def kernel(**inputs):
    from concourse.bass_utils import run_bass_kernel_spmd

    nc, in_maps = _prepare_run(inputs)
    res = run_bass_kernel_spmd(nc, in_maps, core_ids=list(range(NCORES)), trace=False)
    logits = np.concatenate([r["logits"] for r in res.results], axis=0)
    loss = sum(float(r["loss_sum"][0, 0]) for r in res.results) / (B * T)
    return logits.astype(np.float32), np.array(loss, dtype=np.float32)


# revision 37
# speedup vs baseline: 1.4045x; 1.0150x over previous
"""Bass/Trainium2 kernel for nn_BigramLanguageModel (6-layer GPT, B=128 T=256 C=384 H=6 V=65).

Strategy: pure data-parallel over batch across 8 NeuronCores (16 batch rows each),
weights replicated. Per core, a fully fused transformer forward:
  - residual stream token-major fp32 in SBUF ([128 tok, 384] tiles)
  - matmul operands fp16 (1 cy/row on PE), fp32 PSUM accumulation
  - LayerNorm stats via bn_stats/bn_aggr (token-major), scale/bias folded into the
    PSUM->SBUF copy after the PE transpose to feature-major
  - attention scores computed transposed [s, t]; softmax denominator via a ones
    column appended to V (row 64 of the AV matmul output); causal mask via
    gpsimd.affine_select; no max-subtraction (scores are tiny; softmax invariant)
  - embedding gather as one-hot matmul (one-hot via DMA-broadcast + is_equal)
  - loss (mean NLL) computed on device per core; host sums 8 partials
"""

import numpy as np

V, B, T, C, H, L = 65, 128, 256, 384, 6, 6
HS, FF = C // H, 4 * C
EPS = 1e-5
NCORES = 8
BSH = B // NCORES          # batch rows per core = 16
NTOK = BSH * T             # tokens per core = 4096
NT = NTOK // 128           # token tiles per core = 32
NG = NT // 4               # groups (512 tokens = 2 batch rows) = 8
NC_CH = C // 128           # 3 feature chunks
NF_CH = FF // 128          # 12 ff chunks
SCALE = C ** (-0.5)

_CACHE = {}


def _build_program(nlayers=L, do_attn=True, do_mlp=True, zero_pb=False, zero_b2=False):
    import concourse.bacc as bacc
    import concourse.bass as bass
    import concourse.mybir as mybir
    import concourse.tile as tile
    from concourse.masks import make_identity

    f16 = mybir.dt.float16
    f32 = mybir.dt.float32
    i32 = mybir.dt.int32
    AF = mybir.ActivationFunctionType
    ALU = mybir.AluOpType

    nc = bacc.Bacc(target_bir_lowering=False)

    # ---- DRAM I/O ----
    idx_d = nc.dram_tensor("idx", [BSH, T], i32, kind="ExternalInput")
    tgt_d = nc.dram_tensor("targets", [BSH, T], i32, kind="ExternalInput")
    tok_d = nc.dram_tensor("tok_emb", [V, C], f16, kind="ExternalInput")
    pos_d = nc.dram_tensor("pos_emb", [T, C], f16, kind="ExternalInput")
    wqkv_d = nc.dram_tensor("wqkv", [L, C, 3 * C], f16, kind="ExternalInput")
    projw_d = nc.dram_tensor("proj_w", [L, C, C], f16, kind="ExternalInput")
    projb_d = None if zero_pb else nc.dram_tensor("proj_b", [1, L * C], f16,
                                                  kind="ExternalInput")
    w1_d = nc.dram_tensor("w1", [L, C, FF], f16, kind="ExternalInput")
    b1_d = nc.dram_tensor("b1", [L, FF], f32, kind="ExternalInput")
    w2_d = nc.dram_tensor("w2", [L, FF, C], f16, kind="ExternalInput")
    b2_d = None if zero_b2 else nc.dram_tensor("b2", [1, L * C], f16,
                                               kind="ExternalInput")
    ln1s_d = nc.dram_tensor("ln1_s", [L, C], f32, kind="ExternalInput")
    ln1b_d = nc.dram_tensor("ln1_b", [L, C], f32, kind="ExternalInput")
    ln2s_d = nc.dram_tensor("ln2_s", [L, C], f32, kind="ExternalInput")
    ln2b_d = nc.dram_tensor("ln2_b", [L, C], f32, kind="ExternalInput")
    lnfs_d = nc.dram_tensor("lnf_s", [C], f32, kind="ExternalInput")
    lnfb_d = nc.dram_tensor("lnf_b", [C], f32, kind="ExternalInput")
    headw_d = nc.dram_tensor("head_w", [C, V], f16, kind="ExternalInput")
    headb_d = nc.dram_tensor("head_b", [1, V], f16, kind="ExternalInput")
    logits_d = nc.dram_tensor("logits", [NTOK, V], f32, kind="ExternalOutput")
    loss_d = nc.dram_tensor("loss_sum", [1, 1], f32, kind="ExternalOutput")

    with tile.TileContext(nc) as tc:
        import contextlib
        ctx = contextlib.ExitStack()
        with ctx:
            const = ctx.enter_context(tc.tile_pool(name="const", bufs=1))
            wA = ctx.enter_context(tc.tile_pool(name="wA", bufs=2))
            wB = ctx.enter_context(tc.tile_pool(name="wB", bufs=2))
            resid = ctx.enter_context(tc.tile_pool(name="resid", bufs=1))
            tmg = ctx.enter_context(tc.tile_pool(name="tmg", bufs=2))      # token-major group tiles
            fmg = ctx.enter_context(tc.tile_pool(name="fmg", bufs=2))      # feature-major group tiles
            qkp = ctx.enter_context(tc.tile_pool(name="qkp", bufs=2))
            vp = ctx.enter_context(tc.tile_pool(name="vp", bufs=2))
            exl = ctx.enter_context(tc.tile_pool(name="exl", bufs=4))
            hp = ctx.enter_context(tc.tile_pool(name="hp", bufs=2))
            stat = ctx.enter_context(tc.tile_pool(name="stat", bufs=6))
            small = ctx.enter_context(tc.tile_pool(name="small", bufs=2))
            outp = ctx.enter_context(tc.tile_pool(name="outp", bufs=6))
            ps_big = ctx.enter_context(tc.tile_pool(name="ps_big", bufs=4, space="PSUM"))
            ps_tr = ctx.enter_context(tc.tile_pool(name="ps_tr", bufs=2, space="PSUM"))
            ps_av = ctx.enter_context(tc.tile_pool(name="ps_av", bufs=2, space="PSUM"))

            # pin ScalarE's activation table to natural_log_exp_and_others (covers
            # Copy/Identity/Exp/Ln/Relu) so the table-load pass inserts no swaps
            nc.scalar.add_instruction(mybir.InstLoadActFuncSet(
                name=nc.get_next_instruction_name(), act_func_set_id=6, ins=[], outs=[]))

            # ---- constants / params resident in SBUF ----
            ident = const.tile([128, 128], f16)
            make_identity(nc, ident)
            ones_r = const.tile([1, 128], f16)       # K=1 lhsT for bias matmuls
            nc.vector.memset(ones_r, 1.0)
            ones_c32 = const.tile([128, 1], f32)     # K=128 lhsT for loss column-sum
            nc.vector.memset(ones_c32, 1.0)
            eps_t = const.tile([128, 1], f32)
            nc.vector.memset(eps_t, EPS)
            viota_p = const.tile([V, 512], i32)      # value = partition idx
            nc.gpsimd.iota(viota_p, pattern=[[0, 512]], base=0, channel_multiplier=1)
            viota_r = const.tile([128, V], f32)      # value = free idx (exact in f32)
            nc.gpsimd.iota(viota_r, pattern=[[1, V]], base=0, channel_multiplier=0,
                           allow_small_or_imprecise_dtypes=True)
            # causal triangle mask: tri[s, t] = 1.0 if t >= s else 0.0
            tri = const.tile([128, 128], f16)
            nc.vector.memset(tri, 1.0)
            nc.gpsimd.affine_select(out=tri, in_=tri, compare_op=ALU.is_ge, fill=0.0,
                                    base=0, pattern=[[1, 128]], channel_multiplier=-1)

            tok_sb = const.tile([V, C], f16)
            nc.sync.dma_start(out=tok_sb, in_=tok_d[:, :])
            pos_sb = const.tile([128, 2, C], f16)
            nc.sync.dma_start(out=pos_sb, in_=pos_d.rearrange("(i p) c -> p i c", p=128))
            ln1s_sb = const.tile([128, L, NC_CH], f32)
            nc.sync.dma_start(out=ln1s_sb, in_=ln1s_d.rearrange("l (i p) -> p l i", p=128))
            ln1b_sb = const.tile([128, L, NC_CH], f32)
            nc.sync.dma_start(out=ln1b_sb, in_=ln1b_d.rearrange("l (i p) -> p l i", p=128))
            ln2s_sb = const.tile([128, L, NC_CH], f32)
            nc.sync.dma_start(out=ln2s_sb, in_=ln2s_d.rearrange("l (i p) -> p l i", p=128))
            ln2b_sb = const.tile([128, L, NC_CH], f32)
            nc.sync.dma_start(out=ln2b_sb, in_=ln2b_d.rearrange("l (i p) -> p l i", p=128))
            lnfs_sb = const.tile([128, NC_CH], f32)
            nc.sync.dma_start(out=lnfs_sb, in_=lnfs_d.rearrange("(i p) -> p i", p=128))
            lnfb_sb = const.tile([128, NC_CH], f32)
            nc.sync.dma_start(out=lnfb_sb, in_=lnfb_d.rearrange("(i p) -> p i", p=128))
            b1_sb = const.tile([128, L, NF_CH], f32)
            nc.sync.dma_start(out=b1_sb, in_=b1_d.rearrange("l (i p) -> p l i", p=128))
            headw_sb = const.tile([128, NC_CH, V], f16)
            nc.sync.dma_start(out=headw_sb, in_=headw_d.rearrange("(i p) v -> p i v", p=128))
            headb_sb = const.tile([1, V], f16)
            nc.sync.dma_start(out=headb_sb, in_=headb_d[:, :])
            tgt_i = const.tile([128, NT], i32)
            nc.sync.dma_start(out=tgt_i, in_=tgt_d.rearrange("b (n p) -> p (b n)", p=128))
            tgt_sb = const.tile([128, NT], f32)
            nc.vector.tensor_copy(tgt_sb, tgt_i)
            loss_cols = const.tile([128, NT], f32)

            x_tm = resid.tile([128, NT, C], f32)     # persistent residual stream

            # ================= embedding =================
            with nc.named_scope("embed"):
                for g in range(NG):
                    idxb = small.tile([V, 512], i32, tag="idxb")
                    nc.sync.dma_start(
                        out=idxb,
                        in_=bass.AP(idx_d, g * 512, [[0, V], [1, 512]]),
                    )
                    onehot = small.tile([V, 512], f16, tag="onehot")
                    nc.vector.tensor_tensor(out=onehot, in0=idxb, in1=viota_p, op=ALU.is_equal)
                    for j in range(4):
                        i = g * 4 + j
                        ps = ps_big.tile([128, 512], f32, tag="big")
                        nc.tensor.matmul(ps[:, 0:C], onehot[:, j * 128:(j + 1) * 128], tok_sb,
                                         start=True, stop=True)
                        nc.vector.tensor_add(x_tm[:, i, :], ps[:, 0:C], pos_sb[:, i % 2, :])

            # helper: LayerNorm (token-major stats) + transpose to feature-major with
            # scale/bias folded into the PSUM->SBUF copy. Returns [128, NC_CH, 512] f16.
            # rstd = exp(-0.5*ln(var+eps)) keeps ScalarE on one activation table
            # (natural_log_exp set) - Sqrt would force a table swap per group.
            def ln_to_fm(g, s_ap, b_ap, tag):
                mv = stat.tile([128, 4, 2], f32, tag="mv")
                for j in range(4):
                    i = g * 4 + j
                    st6 = stat.tile([128, 6], f32, tag="st6")
                    nc.vector.bn_stats(out=st6, in_=x_tm[:, i, :])
                    nc.vector.bn_aggr(out=mv[:, j, :], in_=st6)
                lnv = stat.tile([128, 4], f32, tag="lnv")
                nc.scalar.activation(out=lnv, in_=mv[:, :, 1], func=AF.Ln,
                                     bias=eps_t, scale=1.0)
                rstd = stat.tile([128, 4], f32, tag="rstd")
                nc.scalar.activation(out=rstd, in_=lnv, func=AF.Exp, scale=-0.5)
                nn_tm = tmg.tile([128, 4, C], f16, tag="nn_tm")
                for j in range(4):
                    i = g * 4 + j
                    nc.vector.tensor_scalar(out=nn_tm[:, j, :], in0=x_tm[:, i, :],
                                            scalar1=mv[:, j, 0:1], scalar2=rstd[:, j:j + 1],
                                            op0=ALU.subtract, op1=ALU.mult)
                nn_fm = fmg.tile([128, NC_CH, 512], f16, tag=tag)
                for ci in range(NC_CH):
                    ps = ps_tr.tile([128, 512], f32, tag="tr")
                    for j in range(4):
                        nc.tensor.matmul(ps[:, j * 128:(j + 1) * 128],
                                         nn_tm[:, j, ci * 128:(ci + 1) * 128], ident,
                                         start=True, stop=True)
                    nc.scalar.activation(out=nn_fm[:, ci, :], in_=ps, func=AF.Identity,
                                         bias=b_ap[:, ci:ci + 1], scale=s_ap[:, ci:ci + 1])
                return nn_fm

            # ================= layers =================
            for l in range(nlayers):
                wqkv_t = wA.tile([128, NC_CH, 3 * C], f16, tag="wqkv")
                nc.sync.dma_start(out=wqkv_t,
                                  in_=wqkv_d[l, :, :].rearrange("(i p) o -> p i o", p=128))
                projw_t = wA.tile([128, NC_CH, C], f16, tag="projw")
                nc.sync.dma_start(out=projw_t,
                                  in_=projw_d[l, :, :].rearrange("(i p) o -> p i o", p=128))
                w1_t = wB.tile([128, NC_CH, FF], f16, tag="w1")
                nc.sync.dma_start(out=w1_t,
                                  in_=w1_d[l, :, :].rearrange("(i p) o -> p i o", p=128))
                w2_t = wB.tile([128, NF_CH, C], f16, tag="w2")
                nc.sync.dma_start(out=w2_t,
                                  in_=w2_d[l, :, :].rearrange("(i p) o -> p i o", p=128))
                if not zero_pb:
                    projb_sb = wA.tile([1, C], f16, tag="projb")
                    nc.sync.dma_start(out=projb_sb, in_=projb_d[:, l * C:(l + 1) * C])
                if not zero_b2:
                    b2_sb = wA.tile([1, C], f16, tag="b2")
                    nc.sync.dma_start(out=b2_sb, in_=b2_d[:, l * C:(l + 1) * C])

                # ---- phase A: attention ----
                with nc.named_scope(f"l{l}_attn"):
                    for g in range(NG if do_attn else 0):
                        nn_fm = ln_to_fm(g, ln1s_sb[:, l, :], ln1b_sb[:, l, :], "nn_fm")
                        # q, k feature-major [128, ch, 512]
                        q_sb = qkp.tile([128, NC_CH, 512], f16, tag="q")
                        k_sb = qkp.tile([128, NC_CH, 512], f16, tag="k")
                        for dst, off in ((q_sb, 0), (k_sb, C)):
                            for m in range(NC_CH):
                                ps = ps_big.tile([128, 512], f32, tag="big")
                                for ci in range(NC_CH):
                                    nc.tensor.matmul(
                                        ps, wqkv_t[:, ci, off + m * 128:off + (m + 1) * 128],
                                        nn_fm[:, ci, :],
                                        start=(ci == 0), stop=(ci == NC_CH - 1))
                                nc.scalar.copy(dst[:, m, :], ps)
                        # v token-major, augmented with ones column per head [128, 4, H, HS+1]
                        v_sb = vp.tile([128, 4, H, HS + 1], f16, tag="v")
                        for j in range(4):
                            ps = ps_big.tile([128, 512], f32, tag="big")
                            for ci in range(NC_CH):
                                nc.tensor.matmul(ps[:, 0:C],
                                                 nn_fm[:, ci, j * 128:(j + 1) * 128],
                                                 wqkv_t[:, ci, 2 * C:3 * C],
                                                 start=(ci == 0), stop=(ci == NC_CH - 1))
                            nc.scalar.copy(
                                v_sb[:, j, :, 0:HS],
                                ps[:, 0:C].rearrange("p (h d) -> p h d", h=H))
                            nc.vector.memset(v_sb[:, j, :, HS:HS + 1], 1.0)
                        # attention per (batch-in-group, head) -> att token-major
                        att_tm = tmg.tile([128, 4, C], f16, tag="att_tm")
                        for bl in range(2):
                            toff = bl * 256
                            for ch in range(NC_CH):
                                # scores for head pair (2ch, 2ch+1) run concurrently in
                                # the PE array via row tiling (K=64 each at rows 0/64)
                                scp = [ps_big.tile([128, 512], f32, tag="big",
                                                   name=f"sc{hh2}")
                                       for hh2 in range(2)]
                                for hh, sc in enumerate(scp):
                                    o = hh * HS
                                    nc.tensor.matmul(sc[:, 0:256],
                                                     k_sb[o:o + HS, ch, toff:toff + 128],
                                                     q_sb[o:o + HS, ch, toff:toff + 256],
                                                     start=True, stop=True,
                                                     tile_position=(o, 0))
                                for hh, sc in enumerate(scp):
                                    o = hh * HS
                                    nc.tensor.matmul(sc[:, 256:384],
                                                     k_sb[o:o + HS, ch, toff + 128:toff + 256],
                                                     q_sb[o:o + HS, ch, toff + 128:toff + 256],
                                                     start=True, stop=True,
                                                     tile_position=(o, 0))
                                for hh, sc in enumerate(scp):
                                    h = 2 * ch + hh
                                    ex = exl.tile([128, 384], f16, tag="ex")
                                    nc.scalar.activation(out=ex, in_=sc[:, 0:384], func=AF.Exp,
                                                         scale=SCALE)
                                    # mask diagonal blocks (cols 0:128 / 256:384) in one
                                    # op: view [128, 2, 128] block step 256, tri
                                    # broadcast along block dim (step 0)
                                    exv = ex.rearrange("p (a c) -> p a c", a=3)
                                    exd = bass.AP(exv.tensor, exv.offset,
                                                  [exv.ap[0], [2 * exv.ap[1][0], 2],
                                                   exv.ap[2]])
                                    trib = bass.AP(tri.tensor, tri.offset,
                                                   [tri.ap[0], [0, 2], tri.ap[1]])
                                    nc.vector.tensor_mul(exd, exd, trib)
                                    av = ps_av.tile([128, 2, V], f32, tag="av")
                                    nc.tensor.matmul(av[:, 0, :], ex[:, 0:128],
                                                     v_sb[:, bl * 2, h, :],
                                                     start=True, stop=True)
                                    nc.tensor.matmul(av[:, 1, :], ex[:, 128:256],
                                                     v_sb[:, bl * 2, h, :],
                                                     start=True, stop=False)
                                    nc.tensor.matmul(av[:, 1, :], ex[:, 256:384],
                                                     v_sb[:, bl * 2 + 1, h, :],
                                                     start=False, stop=True)
                                    r2 = stat.tile([128, 2], f32, tag="r2")
                                    nc.vector.reciprocal_approx_fast(
                                        out=r2, in_=av[:, :, HS:HS + 1])
                                    for tt in range(2):
                                        nc.vector.tensor_scalar_mul(
                                            att_tm[:, bl * 2 + tt, h * HS:(h + 1) * HS],
                                            av[:, tt, 0:HS], r2[:, tt:tt + 1])
                        # att -> feature-major
                        att_fm = fmg.tile([128, NC_CH, 512], f16, tag="att_fm")
                        for ci in range(NC_CH):
                            ps = ps_tr.tile([128, 512], f32, tag="tr")
                            for j in range(4):
                                nc.tensor.matmul(ps[:, j * 128:(j + 1) * 128],
                                                 att_tm[:, j, ci * 128:(ci + 1) * 128], ident,
                                                 start=True, stop=True)
                            nc.scalar.copy(att_fm[:, ci, :], ps)
                        # proj + residual
                        for j in range(4):
                            i = g * 4 + j
                            ps = ps_big.tile([128, 512], f32, tag="big")
                            for ci in range(NC_CH):
                                nc.tensor.matmul(ps[:, 0:C], att_fm[:, ci, j * 128:(j + 1) * 128],
                                                 projw_t[:, ci, :], start=(ci == 0),
                                                 stop=(zero_pb and ci == NC_CH - 1))
                            if not zero_pb:
                                nc.tensor.matmul(ps[:, 0:C], ones_r, projb_sb,
                                                 start=False, stop=True)
                            nc.vector.tensor_add(x_tm[:, i, :], x_tm[:, i, :], ps[:, 0:C])

                # ---- phase B: MLP ----
                with nc.named_scope(f"l{l}_mlp"):
                    for g in range(NG if do_mlp else 0):
                        nn_fm = ln_to_fm(g, ln2s_sb[:, l, :], ln2b_sb[:, l, :], "nn_fm")
                        h_sb = hp.tile([128, NF_CH, 512], f16, tag="h")
                        for f in range(NF_CH):
                            ps = ps_big.tile([128, 512], f32, tag="big")
                            for ci in range(NC_CH):
                                nc.tensor.matmul(ps, w1_t[:, ci, f * 128:(f + 1) * 128],
                                                 nn_fm[:, ci, :],
                                                 start=(ci == 0), stop=(ci == NC_CH - 1))
                            if f % 2 == 0:
                                nc.vector.tensor_scalar(out=h_sb[:, f, :], in0=ps,
                                                        scalar1=b1_sb[:, l, f:f + 1],
                                                        scalar2=0.0, op0=ALU.add,
                                                        op1=ALU.max)
                            else:
                                nc.scalar.activation(out=h_sb[:, f, :], in_=ps,
                                                     func=AF.Relu,
                                                     bias=b1_sb[:, l, f:f + 1],
                                                     scale=1.0)
                        for j in range(4):
                            i = g * 4 + j
                            ps = ps_big.tile([128, 512], f32, tag="big")
                            for fi in range(NF_CH):
                                nc.tensor.matmul(ps[:, 0:C], h_sb[:, fi, j * 128:(j + 1) * 128],
                                                 w2_t[:, fi, :], start=(fi == 0),
                                                 stop=(zero_b2 and fi == NF_CH - 1))
                            if not zero_b2:
                                nc.tensor.matmul(ps[:, 0:C], ones_r, b2_sb,
                                                 start=False, stop=True)
                            nc.vector.tensor_add(x_tm[:, i, :], x_tm[:, i, :], ps[:, 0:C])

            # ================= final LN + head + loss =================
            with nc.named_scope("head"):
                for g in range(NG):
                    xf_fm = ln_to_fm(g, lnfs_sb, lnfb_sb, "nn_fm")
                    for j in range(4):
                        i = g * 4 + j
                        psl = ps_av.tile([128, 2, V], f32, tag="av")
                        for ci in range(NC_CH):
                            nc.tensor.matmul(psl[:, 0, :], xf_fm[:, ci, j * 128:(j + 1) * 128],
                                             headw_sb[:, ci, :],
                                             start=(ci == 0), stop=False)
                        nc.tensor.matmul(psl[:, 0, :], ones_r, headb_sb,
                                         start=False, stop=True)
                        lg = outp.tile([128, V], f32, tag="lg")
                        nc.scalar.activation(out=lg, in_=psl[:, 0, :], func=AF.Copy)
                        nc.sync.dma_start(out=logits_d[i * 128:(i + 1) * 128, :], in_=lg)
                        # loss pieces: lse - logits[target]
                        sel = outp.tile([128, V], f32, tag="sel")
                        nc.vector.tensor_scalar(out=sel, in0=viota_r,
                                                scalar1=tgt_sb[:, i:i + 1], scalar2=None,
                                                op0=ALU.is_equal)
                        scr = outp.tile([128, V], f32, tag="scr")
                        pick = stat.tile([128, 1], f32, tag="pick")
                        nc.vector.tensor_mul(scr, lg, sel)
                        nc.vector.reduce_sum(pick, scr, axis=mybir.AxisListType.X)
                        ex2 = outp.tile([128, V], f32, tag="ex2")
                        se = stat.tile([128, 1], f32, tag="se")
                        nc.scalar.activation(out=ex2, in_=lg, func=AF.Exp, accum_out=se)
                        lse = stat.tile([128, 1], f32, tag="lse")
                        nc.scalar.activation(out=lse, in_=se, func=AF.Ln)
                        nc.vector.tensor_sub(loss_cols[:, i:i + 1], lse, pick)
                pst = ps_big.tile([1, NT], f32, tag="big")
                nc.tensor.matmul(pst, ones_c32, loss_cols, start=True, stop=True)
                tot = stat.tile([1, 1], f32, tag="tot")
                nc.vector.reduce_sum(tot, pst, axis=mybir.AxisListType.X)
                nc.sync.dma_start(out=loss_d[:, :], in_=tot)

    nc.finalize()
    return nc


def _get_program(zero_pb=False, zero_b2=False):
    key = ("nc", zero_pb, zero_b2)
    if key not in _CACHE:
        _CACHE[key] = _build_program(zero_pb=zero_pb, zero_b2=zero_b2)
    return _CACHE[key]


def _prep_weights(inputs):
    f16 = np.float16

    def hcd(w):  # [L,H,C,HS] -> [L, C, H*HS]
        return np.ascontiguousarray(w.transpose(0, 2, 1, 3)).reshape(L, C, C)

    wqkv = np.concatenate(
        [hcd(np.asarray(inputs["wq"])), hcd(np.asarray(inputs["wk"])),
         hcd(np.asarray(inputs["wv"]))], axis=2).astype(f16)
    shared = {
        "tok_emb": np.asarray(inputs["tok_emb"]).astype(f16),
        "pos_emb": np.asarray(inputs["pos_emb"]).astype(f16),
        "wqkv": np.ascontiguousarray(wqkv),
        "proj_w": np.asarray(inputs["proj_w"]).astype(f16),
        "proj_b": np.asarray(inputs["proj_b"]).astype(f16).reshape(1, L * C),
        "w1": np.asarray(inputs["w1"]).astype(f16),
        "b1": np.asarray(inputs["b1"]).astype(np.float32),
        "w2": np.asarray(inputs["w2"]).astype(f16),
        "b2": np.asarray(inputs["b2"]).astype(f16).reshape(1, L * C),
        "ln1_s": np.asarray(inputs["ln1_s"]).astype(np.float32),
        "ln1_b": np.asarray(inputs["ln1_b"]).astype(np.float32),
        "ln2_s": np.asarray(inputs["ln2_s"]).astype(np.float32),
        "ln2_b": np.asarray(inputs["ln2_b"]).astype(np.float32),
        "lnf_s": np.asarray(inputs["lnf_s"]).astype(np.float32),
        "lnf_b": np.asarray(inputs["lnf_b"]).astype(np.float32),
        "head_w": np.asarray(inputs["head_w"]).astype(f16),
        "head_b": np.asarray(inputs["head_b"]).astype(f16).reshape(1, V),
    }
    return shared


def _prepare_run(inputs):
    zero_pb = not np.any(np.asarray(inputs["proj_b"]))
    zero_b2 = not np.any(np.asarray(inputs["b2"]))
    nc = _get_program(zero_pb=zero_pb, zero_b2=zero_b2)
    shared = _prep_weights(inputs)
    if zero_pb:
        shared.pop("proj_b")
    if zero_b2:
        shared.pop("b2")
    idx = np.ascontiguousarray(np.asarray(inputs["idx"], dtype=np.int32))
    tgt = np.ascontiguousarray(np.asarray(inputs["targets"], dtype=np.int32))
    in_maps = []
    for c in range(NCORES):
        m = dict(shared)
        m["idx"] = np.ascontiguousarray(idx[c * BSH:(c + 1) * BSH])
        m["targets"] = np.ascontiguousarray(tgt[c * BSH:(c + 1) * BSH])
        in_maps.append(m)
    return nc, in_maps


def kernel(**inputs):
    from concourse.bass_utils import run_bass_kernel_spmd

    nc, in_maps = _prepare_run(inputs)
    res = run_bass_kernel_spmd(nc, in_maps, core_ids=list(range(NCORES)), trace=False)
    logits = np.concatenate([r["logits"] for r in res.results], axis=0)
    loss = sum(float(r["loss_sum"][0, 0]) for r in res.results) / (B * T)
    return logits.astype(np.float32), np.array(loss, dtype=np.float32)
